# revision 37
# baseline (speedup 1.0000x reference)
"""Trainium2 Bass kernel for nn_CNNGenerator (frame CNN + FC + window-sum + FC).

Key algebraic facts exploited (validated vs the reference):
  * softmax over a size-1 axis == 1.0, so the whole attention_conv stack is
    dead code; the bmm reduces to an 8-wide sliding-window sum of ffc.
  * The per-window stride-2 conv stack collapses into global conv streams:
    an "interior" stream g{1,2,3} and a "left-edge" stream e{1,2,3} per
    layer, plus a 2-tap combine (z).  Per t:
      g1[s] = b1 + sum_k W1k x[s+k-8]          e1[t] = b1 + W11 x[t-7] + W12 x[t-6]
      g2[s] = b2 + V0 G1[s-2] + V1 G1[s] + V2 G1[s+2]
      e2[t] = b2 + V1 E1[t] + V2 G1[t+2]
      g3[s] = b3 + U0 G2[s-4] + U1 G2[s] + U2 G2[s+4]
      e3[t] = b3 + U1 E2[t] + U2 G2[t+4]
      z[t]  = b4 + T1 E3[t] + T2 G3[t+8]
    (capitals = leaky-activated streams), then fc1/fc2/fc3+tanh,
    ws[t] = sum_{d=-3..4} ffc[t+d], out = clip(fcw @ ws, 0, 1).

Sharding: pure data parallel, 2 batch elements per core on 8 cores.
On-chip layout: time axis split in 4 chunks of 2048; 32-channel streams pack
4 chunks x 32ch on the 128 partitions, 64-channel streams pack 2 chunks x 64ch
(two tiles).  All matmul operands are bf16 (fp32 PSUM accumulate); evacuation
work is split between the Activation engine (Prelu/Tanh) and DVE (one-op
leaky via scalar_tensor_tensor).  G3 runs as 2 matmuls (not 3) and E3 as 1
(not 2) against DMA-assembled tiles that stack a stream with a column-shifted
copy on the partition axis, doubling effective contraction per column.
"""
import sys

if '/opt/trn_rl_repo' not in sys.path:
    sys.path.insert(0, '/opt/trn_rl_repo')

import numpy as np
import ml_dtypes

BF = ml_dtypes.bfloat16

B, C, T = 16, 29, 8192
NCORES = 8
BPC = B // NCORES          # batch per core
Tc = T // 4                # time chunk
HL = 16                    # left halo: tile col u <-> global idx c*Tc + u - HL
W = Tc + 40                # per-batch stream tile width
NSLOT = 17                 # 128-col lhsT slots in the weight pack

_PROG = {}
PS_GROUP = 1024
PS_BUFS = 4
ST_BUFS = 22

# evacuation routing: which streams drain on DVE instead of Activation.
# 'dve1' = one-op leaky (psum already contains bias), 'dve2' = bias-add +
# leaky (two DVE ops), 'act' = Activation Prelu.
ROUTE = {
    'G1': 'act', 'E1': 'act',
    'G2': 'act', 'E2': 'dve2',
    'G3_0': 'dve2', 'G3_1': 'act', 'E3': 'act', 'H': 'act',
    'H1_0': 'dve2', 'H1_1': 'act', 'H1_2': 'dve2', 'H1_3': 'act',
    'A2': 'act', 'FFC': 'act',
}
SKEW = 3


def _blockdiag(blocks):
    k = sum(b.shape[0] for b in blocks)
    m = sum(b.shape[1] for b in blocks)
    out = np.zeros((k, m), np.float32)
    i = j = 0
    for b in blocks:
        out[i:i + b.shape[0], j:j + b.shape[1]] = b
        i += b.shape[0]
        j += b.shape[1]
    return out


def _pack_weights(inp):
    wp = np.zeros((128, NSLOT * 128), np.float32)

    def put(slot, mat):
        wp[:mat.shape[0], slot * 128: slot * 128 + mat.shape[1]] = mat

    w1 = inp['w1'].astype(np.float32)  # [32, 29, 3]
    b1 = inp['b1'].astype(np.float32)
    # slots 0-2: g1 taps: blockdiag4 of [30, 32]: 29 in-ch rows + bias row
    for k in range(3):
        blk = np.zeros((30, 32), np.float32)
        blk[:29] = w1[:, :, k].T
        if k == 1:
            blk[29] = b1
        put(k, _blockdiag([blk] * 4))
    w2 = inp['w2'].astype(np.float32)
    for k in range(3):  # slots 3-5
        put(3 + k, _blockdiag([w2[:, :, k].T.astype(np.float32)] * 4))
    w3 = inp['w3'].astype(np.float32)
    w3t = [_blockdiag([w3[:, :, k].T.astype(np.float32)] * 2) for k in range(3)]
    # slot 6: G3 merged tap: [G2p ; G2p<<8] -> U0 on rows 0:64, U2 on 64:128
    put(6, np.concatenate([w3t[0], w3t[2]], axis=0))
    # slot 7: G3 center tap U1 on G3d rows 0:64
    put(7, w3t[1])
    # slot 8: E3 merged tap: [E2p ; G2p<<4] -> U1 rows 0:64, U2 rows 64:128
    put(8, np.concatenate([w3t[1], w3t[2]], axis=0))
    w4 = inp['w4'].astype(np.float32)
    for j in (1, 2):    # slots 9-10: [128, 128]
        put(9 + j - 1, _blockdiag([w4[:, :, j].T.astype(np.float32)] * 2))
    fw1t = inp['fw1'].T.astype(np.float32)
    put(11, np.concatenate([fw1t, fw1t], axis=0))               # [64,128] x2 rows
    # fc2 / fc3 as M=128 with zero column-halves: psum accumulation composes
    # the two chunk-halves onto partitions 0:64 / 64:128 without col-tiling.
    fw2t = inp['fw2'].T.astype(np.float32)          # [128, 64]
    z64 = np.zeros_like(fw2t)
    put(12, np.concatenate([fw2t, z64], axis=1))    # fc2_lo [128, 128]
    put(13, np.concatenate([z64, fw2t], axis=1))    # fc2_hi
    fw3t = _blockdiag([inp['fw3'].T.astype(np.float32)] * 2)       # [128, 64]
    z64b = np.zeros_like(fw3t)
    put(14, np.concatenate([fw3t, z64b], axis=1))   # fc3_lo
    put(15, np.concatenate([z64b, fw3t], axis=1))   # fc3_hi
    put(16, _blockdiag([inp['fcw'].T.astype(np.float32)] * 4))     # [128, 64]

    wb = np.zeros((128, 8), np.float32)
    wb[:, 0] = np.tile(inp['b2'], 4)
    wb[:, 1] = np.tile(inp['b3'], 2)
    wb[:, 2] = np.tile(inp['b4'], 2)
    wb[:, 3] = inp['fb1']
    wb[:, 4] = np.tile(inp['fb2'], 2)
    wb[:, 5] = np.tile(inp['fb3'], 4)
    return wp.astype(BF), wb


def _split(lo, hi, step=512):
    return [(a, min(a + step, hi)) for a in range(lo, hi, step)]


def _build_program(reps=1):
    import concourse.bacc as bacc
    import concourse.mybir as mybir
    import concourse.tile as tile

    F32 = mybir.dt.float32
    BF16 = mybir.dt.bfloat16

    nc = bacc.Bacc("TRN2", target_bir_lowering=False, debug=False)
    x_d = nc.dram_tensor("x", [BPC, 4, C + 1, Tc + 22], BF16, kind="ExternalInput").ap()
    w_d = nc.dram_tensor("wpack", [128, NSLOT * 128], BF16, kind="ExternalInput").ap()
    wb_d = nc.dram_tensor("wbias", [128, 8], F32, kind="ExternalInput").ap()
    o_d = nc.dram_tensor("out", [BPC, 4, 16, Tc], BF16, kind="ExternalOutput").ap()

    with tile.TileContext(nc) as tc:
        with tc.tile_pool(name="wp", bufs=1) as wpool, \
             tc.tile_pool(name="xp", bufs=1) as xpool, \
             tc.tile_pool(name="yp", bufs=1) as ypool, \
             tc.tile_pool(name="st", bufs=ST_BUFS) as spool, \
             tc.tile_pool(name="psa", bufs=PS_BUFS, space="PSUM") as ppool_a:

            wsb = wpool.tile([128, NSLOT * 128], BF16, tag="w")
            wbb = wpool.tile([128, 8], F32, tag="wb")

            for _rep in range(reps):
                _emit_body(nc, tc, mybir, F32, BF16, wsb, wbb,
                           xpool, ypool, spool, (ppool_a, ppool_a), x_d, o_d,
                           w_d, wb_d)
    nc.finalize()
    return nc


def _emit_body(nc, tc, mybir, F32, BF16, wsb, wbb,
               xpool, ypool, spool, ppool, x_d, o_d):
    AF = mybir.ActivationFunctionType
    OP = mybir.AluOpType

    def lhsT(slot, k=128, m=128, base=0):
        return wsb[base:base + k, slot * 128: slot * 128 + m]

    def bias(i):
        return wbb[:, i: i + 1]

    # ---------------- input load: per batch [120p = 4 x (29ch + ones), W]
    # host pre-pads x to [4, 30, Tc+22] (chunk-major so one DMA fills all 120
    # partitions); each load covers tile cols [6, Tc+28) in 4 column pieces.
    X = [None] * BPC
    for b in range(BPC):
        X[b] = xpool.tile([120, W], BF16, tag="x", name=f"x{b}")
    XP = [(0, 1050), (1050, Tc + 22)]
    nc.sync.dma_start(out=wsb[:, 0:384], in_=w_d[:, 0:384])
    nc.sync.dma_start(out=wbb[:], in_=wb_d[:])
    for (s0, s1) in XP:
        nc.sync.dma_start(out=X[0][0:120, 6 + s0: 6 + s1],
                          in_=x_d[0, :, :, s0: s1])
    nc.sync.dma_start(out=wsb[:, 384:], in_=w_d[:, 384:])
    for (s0, s1) in XP:
        nc.sync.dma_start(out=X[1][0:120, 6 + s0: 6 + s1],
                          in_=x_d[1, :, :, s0: s1])

    Y = ypool.tile([64, BPC * Tc], BF16, tag="y")

    CB = 1037  # half boundary (tile col)

    def make_layers(b):
        ST = lambda nm: spool.tile([128, W], BF16, tag="st", name=f"{nm}_{b}")  # noqa: E731

        def conv_pass(out_tile, rng, taps, evac, pool, half):
            """taps: list of (lhsT_ap, rhs_tile, rp0, rp1, delta)."""
            lo = rng[0] if half == 0 else CB
            hi = CB if half == 0 else rng[1]
            for (glo, ghi) in _split(lo, hi, PS_GROUP):
                gn = ghi - glo
                ps = pool.tile([128, PS_GROUP], F32, tag="ps", name="ps")
                for (lo, hi) in _split(glo, ghi, 512):
                    n, off = hi - lo, lo - glo
                    for i, (lw, rt, rp0, rp1, d) in enumerate(taps):
                        nc.tensor.matmul(
                            ps[0:lw.shape[-1], off:off + n], lw,
                            rt[rp0:rp1, lo + d: hi + d],
                            start=(i == 0), stop=(i == len(taps) - 1))
                evac(ps[:, 0:gn], out_tile[:, glo:ghi])

        def pool_for(name):
            return ppool_a if ROUTE.get(name, 'act') == 'act' else ppool_d

        def psg_for(name):
            return PSA_GROUP if ROUTE.get(name, 'act') == 'act' else PS_GROUP

        def evac_for(name, alpha, bias_i, half=0):
            route = ROUTE.get(name, 'act')
            # tail: the last job's DVE-routed drains serialize behind the
            # window-sum/clip chain while Act idles; shift them to Act
            if TAILACT and b == BPC - 1 and half == len(CBS) and route in ('dve2',):
                route = 'act'

            def act(ps, ot):
                nc.scalar.activation(ot, ps, AF.Prelu,
                                     bias=bias(bias_i) if bias_i is not None else 0.0,
                                     scale=1.0, alpha=alpha)

            def dve1(ps, ot):
                # one PSUM read max per instruction: copy out, then leaky in SBUF
                nc.vector.tensor_scalar(ot, ps, 0.0, None, OP.add)
                nc.vector.scalar_tensor_tensor(ot, ot, alpha, ot, OP.mult, OP.max)

            def dve2(ps, ot):
                nc.vector.tensor_scalar(ot, ps, bias(bias_i), None, OP.add)
                nc.vector.scalar_tensor_tensor(ot, ot, alpha, ot, OP.mult, OP.max)

            def dver(ps, ot):
                nc.vector.tensor_scalar(
                    ot, ps, bias(bias_i) if bias_i is not None else 0.0, 0.0,
                    OP.add, OP.max)

            return {'act': act, 'dve1': dve1, 'dve2': dve2, 'dver': dver}[route]

        tiles = {}

        def getst(nm):
            if nm not in tiles:
                tiles[nm] = ST(nm)
            return tiles[nm]

        def l0(half):  # G1 + E1
            G1 = getst("G1")
            conv_pass(G1, (14, Tc + 34),
                      [(lhsT(k, 120), X[b], 0, 120, k - 8) for k in range(3)],
                      evac_for('G1', 0.02, None, half), pool_for('G1'), half, psg_for('G1'))
            E1 = getst("E1")
            conv_pass(E1, (13, Tc + 21),
                      [(lhsT(k, 120), X[b], 0, 120, k - 8) for k in (1, 2)],
                      evac_for('E1', 0.02, None, half), pool_for('E1'), half, psg_for('E1'))

        def l1(half):  # G2 + E2
            G1, E1 = tiles['G1'], tiles['E1']
            G2 = getst("G2")
            conv_pass(G2, (17, Tc + 33),
                      [(lhsT(3 + k), G1, 0, 128, 2 * (k - 1)) for k in range(3)],
                      evac_for('G2', 0.02, 0, half), pool_for('G2'), half, psg_for('G2'))
            E2 = getst("E2")
            conv_pass(E2, (13, Tc + 21),
                      [(lhsT(4), E1, 0, 128, 0), (lhsT(5), G1, 0, 128, 2)],
                      evac_for('E2', 0.02, 0, half), pool_for('E2'), half, psg_for('E2'))

        def l2(half):  # assemble [stream ; shifted-copy] tiles
            G2, E2 = tiles['G2'], tiles['E2']
            G3d = [getst("G3d0"), getst("G3d1")]
            EG3d = [getst("EG3d0"), getst("EG3d1")]
            tiles['G3d'], tiles['EG3d'] = G3d, EG3d
            for p in range(2):
                (a0, a1) = (17, CB) if half == 0 else (CB, Tc + 29)
                nc.sync.dma_start(out=G3d[p][0:64, a0: a1],
                                  in_=G2[64 * p:64 * p + 64, a0: a1])
                (a0, a1) = (17, CB) if half == 0 else (CB, Tc + 25)
                nc.sync.dma_start(out=G3d[p][64:128, a0: a1],
                                  in_=G2[64 * p:64 * p + 64, a0 + 8: a1 + 8])
                (a0, a1) = (13, CB) if half == 0 else (CB, Tc + 21)
                nc.sync.dma_start(out=EG3d[p][0:64, a0: a1],
                                  in_=E2[64 * p:64 * p + 64, a0: a1])
                nc.sync.dma_start(out=EG3d[p][64:128, a0: a1],
                                  in_=G2[64 * p:64 * p + 64, a0 + 4: a1 + 4])

        def l3(half):  # G3 + E3
            G3d, EG3d = tiles['G3d'], tiles['EG3d']
            tiles['G3'] = G3 = [getst("G3a"), getst("G3b")]
            for p in range(2):
                conv_pass(G3[p], (21, Tc + 29),
                          [(lhsT(6), G3d[p], 0, 128, -4),
                           (lhsT(7, 64), G3d[p], 0, 64, 0)],
                          evac_for(f'G3_{p}', 0.2, 1, half), pool_for(f'G3_{p}'), half, psg_for(f'G3_{p}'))
            tiles['E3'] = E3 = [getst("E3a"), getst("E3b")]
            for p in range(2):
                conv_pass(E3[p], (13, Tc + 21),
                          [(lhsT(8), EG3d[p], 0, 128, 0)],
                          evac_for('E3', 0.2, 1, half), pool_for('E3'), half, psg_for('E3'))

        def l4(half):  # H
            G3, E3 = tiles['G3'], tiles['E3']
            tiles['H'] = H = [getst("Ha"), getst("Hb")]
            for p in range(2):
                conv_pass(H[p], (13, Tc + 21),
                          [(lhsT(9), E3[p], 0, 128, 0),
                           (lhsT(10), G3[p], 0, 128, 8)],
                          evac_for('H', 0.2, 2, half), pool_for('H'), half, psg_for('H'))

        def l5(half):  # H1 (fc1)
            H = tiles['H']
            tiles['H1'] = H1 = [getst("H1" + str(c)) for c in range(4)]
            for cidx in range(4):
                p, ph = cidx // 2, cidx % 2
                conv_pass(H1[cidx], (13, Tc + 21),
                          [(lhsT(11, 64, base=64 * ph), H[p],
                            64 * ph, 64 * ph + 64, 0)],
                          evac_for(f'H1_{cidx}', 0.02, 3, half), pool_for(f'H1_{cidx}'), half, psg_for(f'H1_{cidx}'))

        def l6(half):  # A2 (fc2)
            H1 = tiles['H1']
            tiles['A2'] = A2 = [getst("A2a"), getst("A2b")]
            for p in range(2):
                conv_pass(A2[p], (13, Tc + 21),
                          [(lhsT(12), H1[2 * p], 0, 128, 0),
                           (lhsT(13), H1[2 * p + 1], 0, 128, 0)],
                          evac_for('A2', 0.02, 4, half), pool_for('A2'), half, psg_for('A2'))

        def l7(half):  # FFC (fc3 + tanh)
            A2 = tiles['A2']
            tiles['FFC'] = FFC = getst("FFC")

            def tanh_evac(ps, ot):
                nc.scalar.activation(ot, ps, AF.Tanh, bias=bias(5), scale=1.0)

            conv_pass(FFC, (13, Tc + 21),
                      [(lhsT(14), A2[0], 0, 128, 0),
                       (lhsT(15), A2[1], 0, 128, 0)],
                      tanh_evac, ppool_a, half, PSA_GROUP)
            # phantom edge values must read as zero in the window sum
            if half == 0:
                nc.gpsimd.memset(FFC[0:32, 13:16], 0.0)
            else:
                nc.gpsimd.memset(FFC[96:128, Tc + 16: Tc + 21], 0.0)

        def l8(half):  # window-sum tree (8-wide): piece-split for pipelining
            # half 0 owns cols [.., CB), half 1 [CB, ..); ops at the boundary
            # read a few columns across it (producer half finished earlier).
            FFC = tiles['FFC']
            S1 = getst("S1")
            tiles['S1'] = S1
            SSTEP = 512
            # Pool is slow (no 16-bit speedup); the final job's sums sit on
            # the critical tail, so run those on DVE instead.
            eng = nc.gpsimd if (b == 0 and half == 0) else nc.vector
            # staggered split points: each op's half-0 range ends before the
            # columns that would read the NEXT op stage across the boundary
            for (p0, p1) in _split(13 if half == 0 else CB + 3,
                                   CB + 3 if half == 0 else Tc + 19, SSTEP):
                eng.tensor_tensor(S1[:, p0: p1], FFC[:, p0: p1],
                                  FFC[:, p0 + 1: p1 + 1], OP.add)
            for (p0, p1) in _split(13 if half == 0 else CB + 1,
                                   CB + 1 if half == 0 else Tc + 17, SSTEP):
                eng.tensor_tensor(FFC[:, p0: p1], S1[:, p0: p1],
                                  S1[:, p0 + 2: p1 + 2], OP.add)
            for (p0, p1) in _split(16 if half == 0 else CB,
                                   CB if half == 0 else Tc + 16, SSTEP):
                eng.tensor_tensor(S1[:, p0: p1], FFC[:, p0 - 3: p1 - 3],
                                  FFC[:, p0 + 1: p1 + 1], OP.add)

        def l9(half):  # final fc + clip + output DMA
            S1 = tiles['S1']
            for (glo, ghi) in _split(16 if half == 0 else CB,
                                     CB if half == 0 else Tc + 16, PS_GROUP):
                ps = ppool_d.tile([128, PS_GROUP], F32, tag=f"ps{PS_GROUP}", name="ps",
                                  bufs=PS_BUFS)
                for (lo, hi) in _split(glo, ghi, 512):
                    n, off = hi - lo, lo - glo
                    nc.tensor.matmul(ps[0:64, off:off + n], lhsT(16, 128, 64),
                                     S1[:, lo: hi], start=True, stop=True)
                    nc.vector.tensor_scalar(
                        Y[:, b * Tc + lo - 16: b * Tc + hi - 16],
                        ps[0:64, off:off + n], 0.0, 1.0, OP.max, OP.min)
            (h0, h1) = (0, CB - 16) if half == 0 else (CB - 16, Tc)
            nc.sync.dma_start(out=o_d[b, :, :, h0:h1],
                              in_=Y[0:64, b * Tc + h0: b * Tc + h1])

        return [l0, l1, l2, l3, l4, l5, l6, l7, l8, l9]

    batch_layers = [make_layers(b) for b in range(BPC)]
    jobs = [(0, 0), (0, 1), (1, 0), (1, 1)]
    L = len(batch_layers[0])
    for k in range(L + SKEW * (len(jobs) - 1)):
        # emit deeper-pipelined jobs first: a half-0 layer reads a few
        # boundary columns from the next job's previous layer, which must
        # appear earlier in program order for the dependency to register
        for j, (b, h) in reversed(list(enumerate(jobs))):
            kk = k - SKEW * j
            if 0 <= kk < L:
                batch_layers[b][kk](h)


def _get_program(reps=1):
    global _PROG
    if _PROG is None:
        _PROG = {}
    if reps not in _PROG:
        _PROG[reps] = _build_program(reps)
    return _PROG[reps]


def _prepare_inputs(inputs):
    x = np.asarray(inputs['speech_features'], np.float32)
    xp = np.zeros((B, C + 1, T + 22), np.float32)
    xp[:, :C, 10:10 + T] = x
    xp[:, C, :] = 1.0
    # chunk-major: [B, 4, 30, Tc+22]; chunk c covers padded cols [c*Tc, c*Tc+Tc+22)
    xa = np.zeros((B, 4, C + 1, Tc + 22), np.float32)
    for c in range(4):
        xa[:, c] = xp[:, :, c * Tc: c * Tc + Tc + 22]
    xa = xa.astype(BF)
    wp, wb = _pack_weights({k: np.asarray(v, np.float32)
                            for k, v in inputs.items() if k != 'speech_features'})
    return [{"x": xa[i * BPC:(i + 1) * BPC], "wpack": wp, "wbias": wb}
            for i in range(NCORES)]


def kernel(**inputs):
    from concourse.bass_utils import run_bass_kernel_spmd

    in_maps = _prepare_inputs(inputs)
    nc = _get_program()
    res = run_bass_kernel_spmd(nc, in_maps, core_ids=list(range(NCORES)))
    outs = []
    for r in res.results:
        o = np.asarray(r["out"]).astype(np.float32)      # [BPC, 4, 16, Tc]
        o = o.transpose(0, 1, 3, 2).reshape(BPC, T, 16)  # [BPC, T, 16]
        outs.append(o)
    return np.ascontiguousarray(np.concatenate(outs, axis=0))


# revision 38
# speedup vs baseline: 1.0080x; 1.0080x over previous
"""Trainium2 Bass kernel for nn_CNNGenerator (frame CNN + FC + window-sum + FC).

Key algebraic facts exploited (validated vs the reference):
  * softmax over a size-1 axis == 1.0, so the whole attention_conv stack is
    dead code; the bmm reduces to an 8-wide sliding-window sum of ffc.
  * The per-window stride-2 conv stack collapses into global conv streams:
    an "interior" stream g{1,2,3} and a "left-edge" stream e{1,2,3} per
    layer, plus a 2-tap combine (z).  Per t:
      g1[s] = b1 + sum_k W1k x[s+k-8]          e1[t] = b1 + W11 x[t-7] + W12 x[t-6]
      g2[s] = b2 + V0 G1[s-2] + V1 G1[s] + V2 G1[s+2]
      e2[t] = b2 + V1 E1[t] + V2 G1[t+2]
      g3[s] = b3 + U0 G2[s-4] + U1 G2[s] + U2 G2[s+4]
      e3[t] = b3 + U1 E2[t] + U2 G2[t+4]
      z[t]  = b4 + T1 E3[t] + T2 G3[t+8]
    (capitals = leaky-activated streams), then fc1/fc2/fc3+tanh,
    ws[t] = sum_{d=-3..4} ffc[t+d], out = clip(fcw @ ws, 0, 1).

Sharding: pure data parallel, 2 batch elements per core on 8 cores.
On-chip layout: time axis split in 4 chunks of 2048; 32-channel streams pack
4 chunks x 32ch on the 128 partitions, 64-channel streams pack 2 chunks x 64ch
(two tiles).  All matmul operands are bf16 (fp32 PSUM accumulate); evacuation
work is split between the Activation engine (Prelu/Tanh) and DVE (one-op
leaky via scalar_tensor_tensor).  G3 runs as 2 matmuls (not 3) and E3 as 1
(not 2) against DMA-assembled tiles that stack a stream with a column-shifted
copy on the partition axis, doubling effective contraction per column.
"""
import sys

if '/opt/trn_rl_repo' not in sys.path:
    sys.path.insert(0, '/opt/trn_rl_repo')

import numpy as np
import ml_dtypes

BF = ml_dtypes.bfloat16

B, C, T = 16, 29, 8192
NCORES = 8
BPC = B // NCORES          # batch per core
Tc = T // 4                # time chunk
HL = 16                    # left halo: tile col u <-> global idx c*Tc + u - HL
W = Tc + 40                # per-batch stream tile width
NSLOT = 17                 # 128-col lhsT slots in the weight pack

_PROG = {}
PS_GROUP = 1024
PS_BUFS = 4
ST_BUFS = 22

# evacuation routing: which streams drain on DVE instead of Activation.
# 'dve1' = one-op leaky (psum already contains bias), 'dve2' = bias-add +
# leaky (two DVE ops), 'act' = Activation Prelu.
ROUTE = {
    'G1': 'act', 'E1': 'act',
    'G2': 'act', 'E2': 'dve2',
    'G3_0': 'dve2', 'G3_1': 'act', 'E3': 'act', 'H': 'act',
    'H1_0': 'dve2', 'H1_1': 'act', 'H1_2': 'dve2', 'H1_3': 'act',
    'A2': 'act', 'FFC': 'act',
}
SKEW = 3


def _blockdiag(blocks):
    k = sum(b.shape[0] for b in blocks)
    m = sum(b.shape[1] for b in blocks)
    out = np.zeros((k, m), np.float32)
    i = j = 0
    for b in blocks:
        out[i:i + b.shape[0], j:j + b.shape[1]] = b
        i += b.shape[0]
        j += b.shape[1]
    return out


def _pack_weights(inp):
    wp = np.zeros((128, NSLOT * 128), np.float32)

    def put(slot, mat):
        wp[:mat.shape[0], slot * 128: slot * 128 + mat.shape[1]] = mat

    w1 = inp['w1'].astype(np.float32)  # [32, 29, 3]
    b1 = inp['b1'].astype(np.float32)
    # slots 0-2: g1 taps: blockdiag4 of [30, 32]: 29 in-ch rows + bias row
    for k in range(3):
        blk = np.zeros((30, 32), np.float32)
        blk[:29] = w1[:, :, k].T
        if k == 1:
            blk[29] = b1
        put(k, _blockdiag([blk] * 4))
    w2 = inp['w2'].astype(np.float32)
    for k in range(3):  # slots 3-5
        put(3 + k, _blockdiag([w2[:, :, k].T.astype(np.float32)] * 4))
    w3 = inp['w3'].astype(np.float32)
    w3t = [_blockdiag([w3[:, :, k].T.astype(np.float32)] * 2) for k in range(3)]
    # slot 6: G3 merged tap: [G2p ; G2p<<8] -> U0 on rows 0:64, U2 on 64:128
    put(6, np.concatenate([w3t[0], w3t[2]], axis=0))
    # slot 7: G3 center tap U1 on G3d rows 0:64
    put(7, w3t[1])
    # slot 8: E3 merged tap: [E2p ; G2p<<4] -> U1 rows 0:64, U2 rows 64:128
    put(8, np.concatenate([w3t[1], w3t[2]], axis=0))
    w4 = inp['w4'].astype(np.float32)
    for j in (1, 2):    # slots 9-10: [128, 128]
        put(9 + j - 1, _blockdiag([w4[:, :, j].T.astype(np.float32)] * 2))
    fw1t = inp['fw1'].T.astype(np.float32)
    put(11, np.concatenate([fw1t, fw1t], axis=0))               # [64,128] x2 rows
    # fc2 / fc3 as M=128 with zero column-halves: psum accumulation composes
    # the two chunk-halves onto partitions 0:64 / 64:128 without col-tiling.
    fw2t = inp['fw2'].T.astype(np.float32)          # [128, 64]
    z64 = np.zeros_like(fw2t)
    put(12, np.concatenate([fw2t, z64], axis=1))    # fc2_lo [128, 128]
    put(13, np.concatenate([z64, fw2t], axis=1))    # fc2_hi
    fw3t = _blockdiag([inp['fw3'].T.astype(np.float32)] * 2)       # [128, 64]
    z64b = np.zeros_like(fw3t)
    put(14, np.concatenate([fw3t, z64b], axis=1))   # fc3_lo
    put(15, np.concatenate([z64b, fw3t], axis=1))   # fc3_hi
    put(16, _blockdiag([inp['fcw'].T.astype(np.float32)] * 4))     # [128, 64]

    wb = np.zeros((128, 8), np.float32)
    wb[:, 0] = np.tile(inp['b2'], 4)
    wb[:, 1] = np.tile(inp['b3'], 2)
    wb[:, 2] = np.tile(inp['b4'], 2)
    wb[:, 3] = inp['fb1']
    wb[:, 4] = np.tile(inp['fb2'], 2)
    wb[:, 5] = np.tile(inp['fb3'], 4)
    return wp.astype(BF), wb


def _split(lo, hi, step=512):
    return [(a, min(a + step, hi)) for a in range(lo, hi, step)]


def _build_program(reps=1):
    import concourse.bacc as bacc
    import concourse.mybir as mybir
    import concourse.tile as tile

    F32 = mybir.dt.float32
    BF16 = mybir.dt.bfloat16

    nc = bacc.Bacc("TRN2", target_bir_lowering=False, debug=False)
    x_d = nc.dram_tensor("x", [BPC, 4, C + 1, Tc + 22], BF16, kind="ExternalInput").ap()
    w_d = nc.dram_tensor("wpack", [128, NSLOT * 128], BF16, kind="ExternalInput").ap()
    wb_d = nc.dram_tensor("wbias", [128, 8], F32, kind="ExternalInput").ap()
    o_d = nc.dram_tensor("out", [BPC, 4, 16, Tc], BF16, kind="ExternalOutput").ap()

    with tile.TileContext(nc) as tc:
        with tc.tile_pool(name="wp", bufs=1) as wpool, \
             tc.tile_pool(name="xp", bufs=1) as xpool, \
             tc.tile_pool(name="yp", bufs=1) as ypool, \
             tc.tile_pool(name="st", bufs=ST_BUFS) as spool, \
             tc.tile_pool(name="psa", bufs=PS_BUFS, space="PSUM") as ppool_a:

            wsb = wpool.tile([128, NSLOT * 128], BF16, tag="w")
            wbb = wpool.tile([128, 8], F32, tag="wb")

            for _rep in range(reps):
                _emit_body(nc, tc, mybir, F32, BF16, wsb, wbb,
                           xpool, ypool, spool, (ppool_a, ppool_a), x_d, o_d,
                           w_d, wb_d)
    nc.finalize()
    return nc


def _emit_body(nc, tc, mybir, F32, BF16, wsb, wbb,
               xpool, ypool, spool, ppool, x_d, o_d):
    AF = mybir.ActivationFunctionType
    OP = mybir.AluOpType

    def lhsT(slot, k=128, m=128, base=0):
        return wsb[base:base + k, slot * 128: slot * 128 + m]

    def bias(i):
        return wbb[:, i: i + 1]

    # ---------------- input load: per batch [120p = 4 x (29ch + ones), W]
    # host pre-pads x to [4, 30, Tc+22] (chunk-major so one DMA fills all 120
    # partitions); each load covers tile cols [6, Tc+28) in 4 column pieces.
    X = [None] * BPC
    for b in range(BPC):
        X[b] = xpool.tile([120, W], BF16, tag="x", name=f"x{b}")
    XP = [(0, 1050), (1050, Tc + 22)]
    nc.sync.dma_start(out=wsb[:, 0:384], in_=w_d[:, 0:384])
    for (s0, s1) in XP:
        nc.sync.dma_start(out=X[0][0:120, 6 + s0: 6 + s1],
                          in_=x_d[0, :, :, s0: s1])
    nc.sync.dma_start(out=wbb[:], in_=wb_d[:])
    nc.sync.dma_start(out=wsb[:, 384:], in_=w_d[:, 384:])
    for (s0, s1) in XP:
        nc.sync.dma_start(out=X[1][0:120, 6 + s0: 6 + s1],
                          in_=x_d[1, :, :, s0: s1])

    Y = ypool.tile([64, BPC * Tc], BF16, tag="y")

    CB = 1037  # half boundary (tile col)

    def make_layers(b):
        ST = lambda nm: spool.tile([128, W], BF16, tag="st", name=f"{nm}_{b}")  # noqa: E731

        def conv_pass(out_tile, rng, taps, evac, pool, half):
            """taps: list of (lhsT_ap, rhs_tile, rp0, rp1, delta)."""
            lo = rng[0] if half == 0 else CB
            hi = CB if half == 0 else rng[1]
            for (glo, ghi) in _split(lo, hi, PS_GROUP):
                gn = ghi - glo
                ps = pool.tile([128, PS_GROUP], F32, tag="ps", name="ps")
                for (lo, hi) in _split(glo, ghi, 512):
                    n, off = hi - lo, lo - glo
                    for i, (lw, rt, rp0, rp1, d) in enumerate(taps):
                        nc.tensor.matmul(
                            ps[0:lw.shape[-1], off:off + n], lw,
                            rt[rp0:rp1, lo + d: hi + d],
                            start=(i == 0), stop=(i == len(taps) - 1))
                evac(ps[:, 0:gn], out_tile[:, glo:ghi])

        def pool_for(name):
            return ppool_a if ROUTE.get(name, 'act') == 'act' else ppool_d

        def psg_for(name):
            return PSA_GROUP if ROUTE.get(name, 'act') == 'act' else PS_GROUP

        def evac_for(name, alpha, bias_i, half=0):
            route = ROUTE.get(name, 'act')
            # tail: the last job's DVE-routed drains serialize behind the
            # window-sum/clip chain while Act idles; shift them to Act
            if TAILACT and b == BPC - 1 and half == len(CBS) and route in ('dve2',):
                route = 'act'

            def act(ps, ot):
                nc.scalar.activation(ot, ps, AF.Prelu,
                                     bias=bias(bias_i) if bias_i is not None else 0.0,
                                     scale=1.0, alpha=alpha)

            def dve1(ps, ot):
                # one PSUM read max per instruction: copy out, then leaky in SBUF
                nc.vector.tensor_scalar(ot, ps, 0.0, None, OP.add)
                nc.vector.scalar_tensor_tensor(ot, ot, alpha, ot, OP.mult, OP.max)

            def dve2(ps, ot):
                nc.vector.tensor_scalar(ot, ps, bias(bias_i), None, OP.add)
                nc.vector.scalar_tensor_tensor(ot, ot, alpha, ot, OP.mult, OP.max)

            def dver(ps, ot):
                nc.vector.tensor_scalar(
                    ot, ps, bias(bias_i) if bias_i is not None else 0.0, 0.0,
                    OP.add, OP.max)

            return {'act': act, 'dve1': dve1, 'dve2': dve2, 'dver': dver}[route]

        tiles = {}

        def getst(nm):
            if nm not in tiles:
                tiles[nm] = ST(nm)
            return tiles[nm]

        def l0(half):  # G1 + E1
            G1 = getst("G1")
            conv_pass(G1, (14, Tc + 34),
                      [(lhsT(k, 120), X[b], 0, 120, k - 8) for k in range(3)],
                      evac_for('G1', 0.02, None, half), pool_for('G1'), half, psg_for('G1'))
            E1 = getst("E1")
            conv_pass(E1, (13, Tc + 21),
                      [(lhsT(k, 120), X[b], 0, 120, k - 8) for k in (1, 2)],
                      evac_for('E1', 0.02, None, half), pool_for('E1'), half, psg_for('E1'))

        def l1(half):  # G2 + E2
            G1, E1 = tiles['G1'], tiles['E1']
            G2 = getst("G2")
            conv_pass(G2, (17, Tc + 33),
                      [(lhsT(3 + k), G1, 0, 128, 2 * (k - 1)) for k in range(3)],
                      evac_for('G2', 0.02, 0, half), pool_for('G2'), half, psg_for('G2'))
            E2 = getst("E2")
            conv_pass(E2, (13, Tc + 21),
                      [(lhsT(4), E1, 0, 128, 0), (lhsT(5), G1, 0, 128, 2)],
                      evac_for('E2', 0.02, 0, half), pool_for('E2'), half, psg_for('E2'))

        def l2(half):  # assemble [stream ; shifted-copy] tiles
            G2, E2 = tiles['G2'], tiles['E2']
            G3d = [getst("G3d0"), getst("G3d1")]
            EG3d = [getst("EG3d0"), getst("EG3d1")]
            tiles['G3d'], tiles['EG3d'] = G3d, EG3d
            for p in range(2):
                (a0, a1) = (17, CB) if half == 0 else (CB, Tc + 29)
                nc.sync.dma_start(out=G3d[p][0:64, a0: a1],
                                  in_=G2[64 * p:64 * p + 64, a0: a1])
                (a0, a1) = (17, CB) if half == 0 else (CB, Tc + 25)
                nc.sync.dma_start(out=G3d[p][64:128, a0: a1],
                                  in_=G2[64 * p:64 * p + 64, a0 + 8: a1 + 8])
                (a0, a1) = (13, CB) if half == 0 else (CB, Tc + 21)
                nc.sync.dma_start(out=EG3d[p][0:64, a0: a1],
                                  in_=E2[64 * p:64 * p + 64, a0: a1])
                nc.sync.dma_start(out=EG3d[p][64:128, a0: a1],
                                  in_=G2[64 * p:64 * p + 64, a0 + 4: a1 + 4])

        def l3(half):  # G3 + E3
            G3d, EG3d = tiles['G3d'], tiles['EG3d']
            tiles['G3'] = G3 = [getst("G3a"), getst("G3b")]
            for p in range(2):
                conv_pass(G3[p], (21, Tc + 29),
                          [(lhsT(6), G3d[p], 0, 128, -4),
                           (lhsT(7, 64), G3d[p], 0, 64, 0)],
                          evac_for(f'G3_{p}', 0.2, 1, half), pool_for(f'G3_{p}'), half, psg_for(f'G3_{p}'))
            tiles['E3'] = E3 = [getst("E3a"), getst("E3b")]
            for p in range(2):
                conv_pass(E3[p], (13, Tc + 21),
                          [(lhsT(8), EG3d[p], 0, 128, 0)],
                          evac_for('E3', 0.2, 1, half), pool_for('E3'), half, psg_for('E3'))

        def l4(half):  # H
            G3, E3 = tiles['G3'], tiles['E3']
            tiles['H'] = H = [getst("Ha"), getst("Hb")]
            for p in range(2):
                conv_pass(H[p], (13, Tc + 21),
                          [(lhsT(9), E3[p], 0, 128, 0),
                           (lhsT(10), G3[p], 0, 128, 8)],
                          evac_for('H', 0.2, 2, half), pool_for('H'), half, psg_for('H'))

        def l5(half):  # H1 (fc1)
            H = tiles['H']
            tiles['H1'] = H1 = [getst("H1" + str(c)) for c in range(4)]
            for cidx in range(4):
                p, ph = cidx // 2, cidx % 2
                conv_pass(H1[cidx], (13, Tc + 21),
                          [(lhsT(11, 64, base=64 * ph), H[p],
                            64 * ph, 64 * ph + 64, 0)],
                          evac_for(f'H1_{cidx}', 0.02, 3, half), pool_for(f'H1_{cidx}'), half, psg_for(f'H1_{cidx}'))

        def l6(half):  # A2 (fc2)
            H1 = tiles['H1']
            tiles['A2'] = A2 = [getst("A2a"), getst("A2b")]
            for p in range(2):
                conv_pass(A2[p], (13, Tc + 21),
                          [(lhsT(12), H1[2 * p], 0, 128, 0),
                           (lhsT(13), H1[2 * p + 1], 0, 128, 0)],
                          evac_for('A2', 0.02, 4, half), pool_for('A2'), half, psg_for('A2'))

        def l7(half):  # FFC (fc3 + tanh)
            A2 = tiles['A2']
            tiles['FFC'] = FFC = getst("FFC")

            def tanh_evac(ps, ot):
                nc.scalar.activation(ot, ps, AF.Tanh, bias=bias(5), scale=1.0)

            conv_pass(FFC, (13, Tc + 21),
                      [(lhsT(14), A2[0], 0, 128, 0),
                       (lhsT(15), A2[1], 0, 128, 0)],
                      tanh_evac, ppool_a, half, PSA_GROUP)
            # phantom edge values must read as zero in the window sum
            if half == 0:
                nc.gpsimd.memset(FFC[0:32, 13:16], 0.0)
            else:
                nc.gpsimd.memset(FFC[96:128, Tc + 16: Tc + 21], 0.0)

        def l8(half):  # window-sum tree (8-wide): piece-split for pipelining
            # half 0 owns cols [.., CB), half 1 [CB, ..); ops at the boundary
            # read a few columns across it (producer half finished earlier).
            FFC = tiles['FFC']
            S1 = getst("S1")
            tiles['S1'] = S1
            SSTEP = 512
            # Pool is slow (no 16-bit speedup); the final job's sums sit on
            # the critical tail, so run those on DVE instead.
            eng = nc.gpsimd if (b == 0 and half == 0) else nc.vector
            # staggered split points: each op's half-0 range ends before the
            # columns that would read the NEXT op stage across the boundary
            for (p0, p1) in _split(13 if half == 0 else CB + 3,
                                   CB + 3 if half == 0 else Tc + 19, SSTEP):
                eng.tensor_tensor(S1[:, p0: p1], FFC[:, p0: p1],
                                  FFC[:, p0 + 1: p1 + 1], OP.add)
            for (p0, p1) in _split(13 if half == 0 else CB + 1,
                                   CB + 1 if half == 0 else Tc + 17, SSTEP):
                eng.tensor_tensor(FFC[:, p0: p1], S1[:, p0: p1],
                                  S1[:, p0 + 2: p1 + 2], OP.add)
            for (p0, p1) in _split(16 if half == 0 else CB,
                                   CB if half == 0 else Tc + 16, SSTEP):
                eng.tensor_tensor(S1[:, p0: p1], FFC[:, p0 - 3: p1 - 3],
                                  FFC[:, p0 + 1: p1 + 1], OP.add)

        def l9(half):  # final fc + clip + output DMA
            S1 = tiles['S1']
            for (glo, ghi) in _split(16 if half == 0 else CB,
                                     CB if half == 0 else Tc + 16, PS_GROUP):
                ps = ppool_d.tile([128, PS_GROUP], F32, tag=f"ps{PS_GROUP}", name="ps",
                                  bufs=PS_BUFS)
                for (lo, hi) in _split(glo, ghi, 512):
                    n, off = hi - lo, lo - glo
                    nc.tensor.matmul(ps[0:64, off:off + n], lhsT(16, 128, 64),
                                     S1[:, lo: hi], start=True, stop=True)
                    nc.vector.tensor_scalar(
                        Y[:, b * Tc + lo - 16: b * Tc + hi - 16],
                        ps[0:64, off:off + n], 0.0, 1.0, OP.max, OP.min)
            (h0, h1) = (0, CB - 16) if half == 0 else (CB - 16, Tc)
            nc.sync.dma_start(out=o_d[b, :, :, h0:h1],
                              in_=Y[0:64, b * Tc + h0: b * Tc + h1])

        return [l0, l1, l2, l3, l4, l5, l6, l7, l8, l9]

    batch_layers = [make_layers(b) for b in range(BPC)]
    jobs = [(0, 0), (0, 1), (1, 0), (1, 1)]
    L = len(batch_layers[0])
    for k in range(L + SKEW * (len(jobs) - 1)):
        # emit deeper-pipelined jobs first: a half-0 layer reads a few
        # boundary columns from the next job's previous layer, which must
        # appear earlier in program order for the dependency to register
        for j, (b, h) in reversed(list(enumerate(jobs))):
            kk = k - SKEW * j
            if 0 <= kk < L:
                batch_layers[b][kk](h)


def _get_program(reps=1):
    global _PROG
    if _PROG is None:
        _PROG = {}
    if reps not in _PROG:
        _PROG[reps] = _build_program(reps)
    return _PROG[reps]


def _prepare_inputs(inputs):
    x = np.asarray(inputs['speech_features'], np.float32)
    xp = np.zeros((B, C + 1, T + 22), np.float32)
    xp[:, :C, 10:10 + T] = x
    xp[:, C, :] = 1.0
    # chunk-major: [B, 4, 30, Tc+22]; chunk c covers padded cols [c*Tc, c*Tc+Tc+22)
    xa = np.zeros((B, 4, C + 1, Tc + 22), np.float32)
    for c in range(4):
        xa[:, c] = xp[:, :, c * Tc: c * Tc + Tc + 22]
    xa = xa.astype(BF)
    wp, wb = _pack_weights({k: np.asarray(v, np.float32)
                            for k, v in inputs.items() if k != 'speech_features'})
    return [{"x": xa[i * BPC:(i + 1) * BPC], "wpack": wp, "wbias": wb}
            for i in range(NCORES)]


def kernel(**inputs):
    from concourse.bass_utils import run_bass_kernel_spmd

    in_maps = _prepare_inputs(inputs)
    nc = _get_program()
    res = run_bass_kernel_spmd(nc, in_maps, core_ids=list(range(NCORES)))
    outs = []
    for r in res.results:
        o = np.asarray(r["out"]).astype(np.float32)      # [BPC, 4, 16, Tc]
        o = o.transpose(0, 1, 3, 2).reshape(BPC, T, 16)  # [BPC, T, 16]
        outs.append(o)
    return np.ascontiguousarray(np.concatenate(outs, axis=0))


# revision 40
# speedup vs baseline: 1.0102x; 1.0022x over previous
"""Trainium2 Bass kernel for nn_CNNGenerator (frame CNN + FC + window-sum + FC).

Key algebraic facts exploited (validated vs the reference):
  * softmax over a size-1 axis == 1.0, so the whole attention_conv stack is
    dead code; the bmm reduces to an 8-wide sliding-window sum of ffc.
  * The per-window stride-2 conv stack collapses into global conv streams:
    an "interior" stream g{1,2,3} and a "left-edge" stream e{1,2,3} per
    layer, plus a 2-tap combine (z).  Per t:
      g1[s] = b1 + sum_k W1k x[s+k-8]          e1[t] = b1 + W11 x[t-7] + W12 x[t-6]
      g2[s] = b2 + V0 G1[s-2] + V1 G1[s] + V2 G1[s+2]
      e2[t] = b2 + V1 E1[t] + V2 G1[t+2]
      g3[s] = b3 + U0 G2[s-4] + U1 G2[s] + U2 G2[s+4]
      e3[t] = b3 + U1 E2[t] + U2 G2[t+4]
      z[t]  = b4 + T1 E3[t] + T2 G3[t+8]
    (capitals = leaky-activated streams), then fc1/fc2/fc3+tanh,
    ws[t] = sum_{d=-3..4} ffc[t+d], out = clip(fcw @ ws, 0, 1).

Sharding: pure data parallel, 2 batch elements per core on 8 cores.
On-chip layout: time axis split in 4 chunks of 2048; 32-channel streams pack
4 chunks x 32ch on the 128 partitions, 64-channel streams pack 2 chunks x 64ch
(two tiles).  All matmul operands are bf16 (fp32 PSUM accumulate); evacuation
work is split between the Activation engine (Prelu/Tanh) and DVE (one-op
leaky via scalar_tensor_tensor).  G3 runs as 2 matmuls (not 3) and E3 as 1
(not 2) against DMA-assembled tiles that stack a stream with a column-shifted
copy on the partition axis, doubling effective contraction per column.
"""
import sys

if '/opt/trn_rl_repo' not in sys.path:
    sys.path.insert(0, '/opt/trn_rl_repo')

import numpy as np
import ml_dtypes

BF = ml_dtypes.bfloat16

B, C, T = 16, 29, 8192
NCORES = 8
BPC = B // NCORES          # batch per core
Tc = T // 4                # time chunk
HL = 16                    # left halo: tile col u <-> global idx c*Tc + u - HL
W = Tc + 40                # per-batch stream tile width
NSLOT = 17                 # 128-col lhsT slots in the weight pack

_PROG = {}
PS_GROUP = 1024
PS_BUFS = 4
ST_BUFS = 22

# evacuation routing: which streams drain on DVE instead of Activation.
# 'dve1' = one-op leaky (psum already contains bias), 'dve2' = bias-add +
# leaky (two DVE ops), 'act' = Activation Prelu.
ROUTE = {
    'G1': 'act', 'E1': 'act',
    'G2': 'act', 'E2': 'dve2',
    'G3_0': 'dve2', 'G3_1': 'act', 'E3': 'act', 'H': 'act',
    'H1_0': 'dve2', 'H1_1': 'act', 'H1_2': 'dve2', 'H1_3': 'act',
    'A2': 'act', 'FFC': 'act',
}
SKEW = 3


def _blockdiag(blocks):
    k = sum(b.shape[0] for b in blocks)
    m = sum(b.shape[1] for b in blocks)
    out = np.zeros((k, m), np.float32)
    i = j = 0
    for b in blocks:
        out[i:i + b.shape[0], j:j + b.shape[1]] = b
        i += b.shape[0]
        j += b.shape[1]
    return out


def _pack_weights(inp):
    wp = np.zeros((128, NSLOT * 128), np.float32)

    def put(slot, mat):
        wp[:mat.shape[0], slot * 128: slot * 128 + mat.shape[1]] = mat

    w1 = inp['w1'].astype(np.float32)  # [32, 29, 3]
    b1 = inp['b1'].astype(np.float32)
    # slots 0-2: g1 taps: blockdiag4 of [30, 32]: 29 in-ch rows + bias row
    for k in range(3):
        blk = np.zeros((30, 32), np.float32)
        blk[:29] = w1[:, :, k].T
        if k == 1:
            blk[29] = b1
        put(k, _blockdiag([blk] * 4))
    w2 = inp['w2'].astype(np.float32)
    for k in range(3):  # slots 3-5
        put(3 + k, _blockdiag([w2[:, :, k].T.astype(np.float32)] * 4))
    w3 = inp['w3'].astype(np.float32)
    w3t = [_blockdiag([w3[:, :, k].T.astype(np.float32)] * 2) for k in range(3)]
    # slot 6: G3 merged tap: [G2p ; G2p<<8] -> U0 on rows 0:64, U2 on 64:128
    put(6, np.concatenate([w3t[0], w3t[2]], axis=0))
    # slot 7: G3 center tap U1 on G3d rows 0:64
    put(7, w3t[1])
    # slot 8: E3 merged tap: [E2p ; G2p<<4] -> U1 rows 0:64, U2 rows 64:128
    put(8, np.concatenate([w3t[1], w3t[2]], axis=0))
    w4 = inp['w4'].astype(np.float32)
    for j in (1, 2):    # slots 9-10: [128, 128]
        put(9 + j - 1, _blockdiag([w4[:, :, j].T.astype(np.float32)] * 2))
    fw1t = inp['fw1'].T.astype(np.float32)
    put(11, np.concatenate([fw1t, fw1t], axis=0))               # [64,128] x2 rows
    # fc2 / fc3 as M=128 with zero column-halves: psum accumulation composes
    # the two chunk-halves onto partitions 0:64 / 64:128 without col-tiling.
    fw2t = inp['fw2'].T.astype(np.float32)          # [128, 64]
    z64 = np.zeros_like(fw2t)
    put(12, np.concatenate([fw2t, z64], axis=1))    # fc2_lo [128, 128]
    put(13, np.concatenate([z64, fw2t], axis=1))    # fc2_hi
    fw3t = _blockdiag([inp['fw3'].T.astype(np.float32)] * 2)       # [128, 64]
    z64b = np.zeros_like(fw3t)
    put(14, np.concatenate([fw3t, z64b], axis=1))   # fc3_lo
    put(15, np.concatenate([z64b, fw3t], axis=1))   # fc3_hi
    put(16, _blockdiag([inp['fcw'].T.astype(np.float32)] * 4))     # [128, 64]

    wb = np.zeros((128, 8), np.float32)
    wb[:, 0] = np.tile(inp['b2'], 4)
    wb[:, 1] = np.tile(inp['b3'], 2)
    wb[:, 2] = np.tile(inp['b4'], 2)
    wb[:, 3] = inp['fb1']
    wb[:, 4] = np.tile(inp['fb2'], 2)
    wb[:, 5] = np.tile(inp['fb3'], 4)
    return wp.astype(BF), wb


def _split(lo, hi, step=512):
    return [(a, min(a + step, hi)) for a in range(lo, hi, step)]


def _build_program(reps=1):
    import concourse.bacc as bacc
    import concourse.mybir as mybir
    import concourse.tile as tile

    F32 = mybir.dt.float32
    BF16 = mybir.dt.bfloat16

    nc = bacc.Bacc("TRN2", target_bir_lowering=False, debug=False)
    x_d = nc.dram_tensor("x", [BPC, 4, C + 1, Tc + 22], BF16, kind="ExternalInput").ap()
    w_d = nc.dram_tensor("wpack", [128, NSLOT * 128], BF16, kind="ExternalInput").ap()
    wb_d = nc.dram_tensor("wbias", [128, 8], F32, kind="ExternalInput").ap()
    o_d = nc.dram_tensor("out", [BPC, 4, 16, Tc], BF16, kind="ExternalOutput").ap()

    with tile.TileContext(nc) as tc:
        with tc.tile_pool(name="wp", bufs=1) as wpool, \
             tc.tile_pool(name="xp", bufs=1) as xpool, \
             tc.tile_pool(name="yp", bufs=1) as ypool, \
             tc.tile_pool(name="st", bufs=ST_BUFS) as spool, \
             tc.tile_pool(name="psa", bufs=PS_BUFS, space="PSUM") as ppool_a:

            wsb = wpool.tile([128, NSLOT * 128], BF16, tag="w")
            wbb = wpool.tile([128, 8], F32, tag="wb")

            for _rep in range(reps):
                _emit_body(nc, tc, mybir, F32, BF16, wsb, wbb,
                           xpool, ypool, spool, (ppool_a, ppool_a), x_d, o_d,
                           w_d, wb_d)
    nc.finalize()
    return nc


def _emit_body(nc, tc, mybir, F32, BF16, wsb, wbb,
               xpool, ypool, spool, ppool, x_d, o_d):
    AF = mybir.ActivationFunctionType
    OP = mybir.AluOpType

    def lhsT(slot, k=128, m=128, base=0):
        return wsb[base:base + k, slot * 128: slot * 128 + m]

    def bias(i):
        return wbb[:, i: i + 1]

    # ---------------- input load: per batch [120p = 4 x (29ch + ones), W]
    # host pre-pads x to [4, 30, Tc+22] (chunk-major so one DMA fills all 120
    # partitions); each load covers tile cols [6, Tc+28) in 4 column pieces.
    X = [None] * BPC
    for b in range(BPC):
        X[b] = xpool.tile([120, W], BF16, tag="x", name=f"x{b}")
    XP = [(0, 1050), (1050, Tc + 22)]
    nc.sync.dma_start(out=wsb[:, 0:384], in_=w_d[:, 0:384])
    for (s0, s1) in XP:
        nc.sync.dma_start(out=X[0][0:120, 6 + s0: 6 + s1],
                          in_=x_d[0, :, :, s0: s1])
    nc.sync.dma_start(out=wbb[:], in_=wb_d[:])
    nc.sync.dma_start(out=wsb[:, 384:], in_=w_d[:, 384:])
    for (s0, s1) in XP:
        nc.sync.dma_start(out=X[1][0:120, 6 + s0: 6 + s1],
                          in_=x_d[1, :, :, s0: s1])

    Y = ypool.tile([64, BPC * Tc], BF16, tag="y")

    CB = 1037  # half boundary (tile col)

    def make_layers(b):
        ST = lambda nm: spool.tile([128, W], BF16, tag="st", name=f"{nm}_{b}")  # noqa: E731

        def conv_pass(out_tile, rng, taps, evac, pool, half):
            """taps: list of (lhsT_ap, rhs_tile, rp0, rp1, delta)."""
            lo = rng[0] if half == 0 else CB
            hi = CB if half == 0 else rng[1]
            for (glo, ghi) in _split(lo, hi, PS_GROUP):
                gn = ghi - glo
                ps = pool.tile([128, PS_GROUP], F32, tag="ps", name="ps")
                for (lo, hi) in _split(glo, ghi, 512):
                    n, off = hi - lo, lo - glo
                    for i, (lw, rt, rp0, rp1, d) in enumerate(taps):
                        nc.tensor.matmul(
                            ps[0:lw.shape[-1], off:off + n], lw,
                            rt[rp0:rp1, lo + d: hi + d],
                            start=(i == 0), stop=(i == len(taps) - 1))
                evac(ps[:, 0:gn], out_tile[:, glo:ghi])

        def pool_for(name):
            return ppool_a if ROUTE.get(name, 'act') == 'act' else ppool_d

        def psg_for(name):
            return PSA_GROUP if ROUTE.get(name, 'act') == 'act' else PS_GROUP

        def evac_for(name, alpha, bias_i, half=0):
            route = ROUTE.get(name, 'act')
            # tail: the last job's DVE-routed drains serialize behind the
            # window-sum/clip chain while Act idles; shift them to Act
            if TAILACT and b == BPC - 1 and half == len(CBS) and route in ('dve2',):
                route = 'act'

            def act(ps, ot):
                nc.scalar.activation(ot, ps, AF.Prelu,
                                     bias=bias(bias_i) if bias_i is not None else 0.0,
                                     scale=1.0, alpha=alpha)

            def dve1(ps, ot):
                # one PSUM read max per instruction: copy out, then leaky in SBUF
                nc.vector.tensor_scalar(ot, ps, 0.0, None, OP.add)
                nc.vector.scalar_tensor_tensor(ot, ot, alpha, ot, OP.mult, OP.max)

            def dve2(ps, ot):
                nc.vector.tensor_scalar(ot, ps, bias(bias_i), None, OP.add)
                nc.vector.scalar_tensor_tensor(ot, ot, alpha, ot, OP.mult, OP.max)

            def dver(ps, ot):
                nc.vector.tensor_scalar(
                    ot, ps, bias(bias_i) if bias_i is not None else 0.0, 0.0,
                    OP.add, OP.max)

            return {'act': act, 'dve1': dve1, 'dve2': dve2, 'dver': dver}[route]

        tiles = {}

        def getst(nm):
            if nm not in tiles:
                tiles[nm] = ST(nm)
            return tiles[nm]

        def l0(half):  # G1 + E1
            G1 = getst("G1")
            conv_pass(G1, (14, Tc + 34),
                      [(lhsT(k, 120), X[b], 0, 120, k - 8) for k in range(3)],
                      evac_for('G1', 0.02, None, half), pool_for('G1'), half, psg_for('G1'))
            E1 = getst("E1")
            conv_pass(E1, (13, Tc + 21),
                      [(lhsT(k, 120), X[b], 0, 120, k - 8) for k in (1, 2)],
                      evac_for('E1', 0.02, None, half), pool_for('E1'), half, psg_for('E1'))

        def l1(half):  # G2 + E2
            G1, E1 = tiles['G1'], tiles['E1']
            G2 = getst("G2")
            conv_pass(G2, (17, Tc + 33),
                      [(lhsT(3 + k), G1, 0, 128, 2 * (k - 1)) for k in range(3)],
                      evac_for('G2', 0.02, 0, half), pool_for('G2'), half, psg_for('G2'))
            E2 = getst("E2")
            conv_pass(E2, (13, Tc + 21),
                      [(lhsT(4), E1, 0, 128, 0), (lhsT(5), G1, 0, 128, 2)],
                      evac_for('E2', 0.02, 0, half), pool_for('E2'), half, psg_for('E2'))

        def l2(half):  # assemble [stream ; shifted-copy] tiles
            G2, E2 = tiles['G2'], tiles['E2']
            G3d = [getst("G3d0"), getst("G3d1")]
            EG3d = [getst("EG3d0"), getst("EG3d1")]
            tiles['G3d'], tiles['EG3d'] = G3d, EG3d
            for p in range(2):
                (a0, a1) = (17, CB) if half == 0 else (CB, Tc + 29)
                nc.gpsimd.dma_start(out=G3d[p][0:64, a0: a1],
                                  in_=G2[64 * p:64 * p + 64, a0: a1])
                (a0, a1) = (17, CB) if half == 0 else (CB, Tc + 25)
                nc.gpsimd.dma_start(out=G3d[p][64:128, a0: a1],
                                  in_=G2[64 * p:64 * p + 64, a0 + 8: a1 + 8])
                (a0, a1) = (13, CB) if half == 0 else (CB, Tc + 21)
                nc.sync.dma_start(out=EG3d[p][0:64, a0: a1],
                                  in_=E2[64 * p:64 * p + 64, a0: a1])
                nc.sync.dma_start(out=EG3d[p][64:128, a0: a1],
                                  in_=G2[64 * p:64 * p + 64, a0 + 4: a1 + 4])

        def l3(half):  # G3 + E3
            G3d, EG3d = tiles['G3d'], tiles['EG3d']
            tiles['G3'] = G3 = [getst("G3a"), getst("G3b")]
            for p in range(2):
                conv_pass(G3[p], (21, Tc + 29),
                          [(lhsT(6), G3d[p], 0, 128, -4),
                           (lhsT(7, 64), G3d[p], 0, 64, 0)],
                          evac_for(f'G3_{p}', 0.2, 1, half), pool_for(f'G3_{p}'), half, psg_for(f'G3_{p}'))
            tiles['E3'] = E3 = [getst("E3a"), getst("E3b")]
            for p in range(2):
                conv_pass(E3[p], (13, Tc + 21),
                          [(lhsT(8), EG3d[p], 0, 128, 0)],
                          evac_for('E3', 0.2, 1, half), pool_for('E3'), half, psg_for('E3'))

        def l4(half):  # H
            G3, E3 = tiles['G3'], tiles['E3']
            tiles['H'] = H = [getst("Ha"), getst("Hb")]
            for p in range(2):
                conv_pass(H[p], (13, Tc + 21),
                          [(lhsT(9), E3[p], 0, 128, 0),
                           (lhsT(10), G3[p], 0, 128, 8)],
                          evac_for('H', 0.2, 2, half), pool_for('H'), half, psg_for('H'))

        def l5(half):  # H1 (fc1)
            H = tiles['H']
            tiles['H1'] = H1 = [getst("H1" + str(c)) for c in range(4)]
            for cidx in range(4):
                p, ph = cidx // 2, cidx % 2
                conv_pass(H1[cidx], (13, Tc + 21),
                          [(lhsT(11, 64, base=64 * ph), H[p],
                            64 * ph, 64 * ph + 64, 0)],
                          evac_for(f'H1_{cidx}', 0.02, 3, half), pool_for(f'H1_{cidx}'), half, psg_for(f'H1_{cidx}'))

        def l6(half):  # A2 (fc2)
            H1 = tiles['H1']
            tiles['A2'] = A2 = [getst("A2a"), getst("A2b")]
            for p in range(2):
                conv_pass(A2[p], (13, Tc + 21),
                          [(lhsT(12), H1[2 * p], 0, 128, 0),
                           (lhsT(13), H1[2 * p + 1], 0, 128, 0)],
                          evac_for('A2', 0.02, 4, half), pool_for('A2'), half, psg_for('A2'))

        def l7(half):  # FFC (fc3 + tanh)
            A2 = tiles['A2']
            tiles['FFC'] = FFC = getst("FFC")

            def tanh_evac(ps, ot):
                nc.scalar.activation(ot, ps, AF.Tanh, bias=bias(5), scale=1.0)

            conv_pass(FFC, (13, Tc + 21),
                      [(lhsT(14), A2[0], 0, 128, 0),
                       (lhsT(15), A2[1], 0, 128, 0)],
                      tanh_evac, ppool_a, half, PSA_GROUP)
            # phantom edge values must read as zero in the window sum
            if half == 0:
                nc.gpsimd.memset(FFC[0:32, 13:16], 0.0)
            else:
                nc.gpsimd.memset(FFC[96:128, Tc + 16: Tc + 21], 0.0)

        def l8(half):  # window-sum tree (8-wide): piece-split for pipelining
            # half 0 owns cols [.., CB), half 1 [CB, ..); ops at the boundary
            # read a few columns across it (producer half finished earlier).
            FFC = tiles['FFC']
            S1 = getst("S1")
            tiles['S1'] = S1
            SSTEP = 512
            # Pool is slow (no 16-bit speedup); the final job's sums sit on
            # the critical tail, so run those on DVE instead.
            eng = nc.gpsimd if (b == 0 and half == 0) else nc.vector
            # staggered split points: each op's half-0 range ends before the
            # columns that would read the NEXT op stage across the boundary
            for (p0, p1) in _split(13 if half == 0 else CB + 3,
                                   CB + 3 if half == 0 else Tc + 19, SSTEP):
                eng.tensor_tensor(S1[:, p0: p1], FFC[:, p0: p1],
                                  FFC[:, p0 + 1: p1 + 1], OP.add)
            for (p0, p1) in _split(13 if half == 0 else CB + 1,
                                   CB + 1 if half == 0 else Tc + 17, SSTEP):
                eng.tensor_tensor(FFC[:, p0: p1], S1[:, p0: p1],
                                  S1[:, p0 + 2: p1 + 2], OP.add)
            for (p0, p1) in _split(16 if half == 0 else CB,
                                   CB if half == 0 else Tc + 16, SSTEP):
                eng.tensor_tensor(S1[:, p0: p1], FFC[:, p0 - 3: p1 - 3],
                                  FFC[:, p0 + 1: p1 + 1], OP.add)

        def l9(half):  # final fc + clip + output DMA
            S1 = tiles['S1']
            for (glo, ghi) in _split(16 if half == 0 else CB,
                                     CB if half == 0 else Tc + 16, PS_GROUP):
                ps = ppool_d.tile([128, PS_GROUP], F32, tag=f"ps{PS_GROUP}", name="ps",
                                  bufs=PS_BUFS)
                for (lo, hi) in _split(glo, ghi, 512):
                    n, off = hi - lo, lo - glo
                    nc.tensor.matmul(ps[0:64, off:off + n], lhsT(16, 128, 64),
                                     S1[:, lo: hi], start=True, stop=True)
                    nc.vector.tensor_scalar(
                        Y[:, b * Tc + lo - 16: b * Tc + hi - 16],
                        ps[0:64, off:off + n], 0.0, 1.0, OP.max, OP.min)
            (h0, h1) = (0, CB - 16) if half == 0 else (CB - 16, Tc)
            nc.sync.dma_start(out=o_d[b, :, :, h0:h1],
                              in_=Y[0:64, b * Tc + h0: b * Tc + h1])

        return [l0, l1, l2, l3, l4, l5, l6, l7, l8, l9]

    batch_layers = [make_layers(b) for b in range(BPC)]
    jobs = [(0, 0), (0, 1), (1, 0), (1, 1)]
    L = len(batch_layers[0])
    for k in range(L + SKEW * (len(jobs) - 1)):
        # emit deeper-pipelined jobs first: a half-0 layer reads a few
        # boundary columns from the next job's previous layer, which must
        # appear earlier in program order for the dependency to register
        for j, (b, h) in reversed(list(enumerate(jobs))):
            kk = k - SKEW * j
            if 0 <= kk < L:
                batch_layers[b][kk](h)


def _get_program(reps=1):
    global _PROG
    if _PROG is None:
        _PROG = {}
    if reps not in _PROG:
        _PROG[reps] = _build_program(reps)
    return _PROG[reps]


def _prepare_inputs(inputs):
    x = np.asarray(inputs['speech_features'], np.float32)
    xp = np.zeros((B, C + 1, T + 22), np.float32)
    xp[:, :C, 10:10 + T] = x
    xp[:, C, :] = 1.0
    # chunk-major: [B, 4, 30, Tc+22]; chunk c covers padded cols [c*Tc, c*Tc+Tc+22)
    xa = np.zeros((B, 4, C + 1, Tc + 22), np.float32)
    for c in range(4):
        xa[:, c] = xp[:, :, c * Tc: c * Tc + Tc + 22]
    xa = xa.astype(BF)
    wp, wb = _pack_weights({k: np.asarray(v, np.float32)
                            for k, v in inputs.items() if k != 'speech_features'})
    return [{"x": xa[i * BPC:(i + 1) * BPC], "wpack": wp, "wbias": wb}
            for i in range(NCORES)]


def kernel(**inputs):
    from concourse.bass_utils import run_bass_kernel_spmd

    in_maps = _prepare_inputs(inputs)
    nc = _get_program()
    res = run_bass_kernel_spmd(nc, in_maps, core_ids=list(range(NCORES)))
    outs = []
    for r in res.results:
        o = np.asarray(r["out"]).astype(np.float32)      # [BPC, 4, 16, Tc]
        o = o.transpose(0, 1, 3, 2).reshape(BPC, T, 16)  # [BPC, T, 16]
        outs.append(o)
    return np.ascontiguousarray(np.concatenate(outs, axis=0))


# revision 42
# speedup vs baseline: 1.0136x; 1.0033x over previous
"""Trainium2 Bass kernel for nn_CNNGenerator (frame CNN + FC + window-sum + FC).

Key algebraic facts exploited (validated vs the reference):
  * softmax over a size-1 axis == 1.0, so the whole attention_conv stack is
    dead code; the bmm reduces to an 8-wide sliding-window sum of ffc.
  * The per-window stride-2 conv stack collapses into global conv streams:
    an "interior" stream g{1,2,3} and a "left-edge" stream e{1,2,3} per
    layer, plus a 2-tap combine (z).  Per t:
      g1[s] = b1 + sum_k W1k x[s+k-8]          e1[t] = b1 + W11 x[t-7] + W12 x[t-6]
      g2[s] = b2 + V0 G1[s-2] + V1 G1[s] + V2 G1[s+2]
      e2[t] = b2 + V1 E1[t] + V2 G1[t+2]
      g3[s] = b3 + U0 G2[s-4] + U1 G2[s] + U2 G2[s+4]
      e3[t] = b3 + U1 E2[t] + U2 G2[t+4]
      z[t]  = b4 + T1 E3[t] + T2 G3[t+8]
    (capitals = leaky-activated streams), then fc1/fc2/fc3+tanh,
    ws[t] = sum_{d=-3..4} ffc[t+d], out = clip(fcw @ ws, 0, 1).

Sharding: pure data parallel, 2 batch elements per core on 8 cores.
On-chip layout: time axis split in 4 chunks of 2048; 32-channel streams pack
4 chunks x 32ch on the 128 partitions, 64-channel streams pack 2 chunks x 64ch
(two tiles).  All matmul operands are bf16 (fp32 PSUM accumulate); evacuation
work is split between the Activation engine (Prelu/Tanh) and DVE (one-op
leaky via scalar_tensor_tensor).  G3 runs as 2 matmuls (not 3) and E3 as 1
(not 2) against DMA-assembled tiles that stack a stream with a column-shifted
copy on the partition axis, doubling effective contraction per column.
"""
import sys

if '/opt/trn_rl_repo' not in sys.path:
    sys.path.insert(0, '/opt/trn_rl_repo')

import numpy as np
import ml_dtypes

BF = ml_dtypes.bfloat16

B, C, T = 16, 29, 8192
NCORES = 8
BPC = B // NCORES          # batch per core
Tc = T // 4                # time chunk
HL = 16                    # left halo: tile col u <-> global idx c*Tc + u - HL
W = Tc + 40                # per-batch stream tile width
NSLOT = 17                 # 128-col lhsT slots in the weight pack

_PROG = {}
PS_GROUP = 1024
PS_BUFS = 4
ST_BUFS = 22

# evacuation routing: which streams drain on DVE instead of Activation.
# 'dve1' = one-op leaky (psum already contains bias), 'dve2' = bias-add +
# leaky (two DVE ops), 'act' = Activation Prelu.
ROUTE = {
    'G1': 'act', 'E1': 'act',
    'G2': 'act', 'E2': 'dve2',
    'G3_0': 'dve2', 'G3_1': 'act', 'E3': 'act', 'H': 'act',
    'H1_0': 'dve2', 'H1_1': 'act', 'H1_2': 'dve2', 'H1_3': 'act',
    'A2': 'act', 'FFC': 'act',
}
SKEW = 3


def _blockdiag(blocks):
    k = sum(b.shape[0] for b in blocks)
    m = sum(b.shape[1] for b in blocks)
    out = np.zeros((k, m), np.float32)
    i = j = 0
    for b in blocks:
        out[i:i + b.shape[0], j:j + b.shape[1]] = b
        i += b.shape[0]
        j += b.shape[1]
    return out


def _pack_weights(inp):
    wp = np.zeros((128, NSLOT * 128), np.float32)

    def put(slot, mat):
        wp[:mat.shape[0], slot * 128: slot * 128 + mat.shape[1]] = mat

    w1 = inp['w1'].astype(np.float32)  # [32, 29, 3]
    b1 = inp['b1'].astype(np.float32)
    # slots 0-2: g1 taps: blockdiag4 of [30, 32]: 29 in-ch rows + bias row
    for k in range(3):
        blk = np.zeros((30, 32), np.float32)
        blk[:29] = w1[:, :, k].T
        if k == 1:
            blk[29] = b1
        put(k, _blockdiag([blk] * 4))
    w2 = inp['w2'].astype(np.float32)
    for k in range(3):  # slots 3-5
        put(3 + k, _blockdiag([w2[:, :, k].T.astype(np.float32)] * 4))
    w3 = inp['w3'].astype(np.float32)
    w3t = [_blockdiag([w3[:, :, k].T.astype(np.float32)] * 2) for k in range(3)]
    # slot 6: G3 merged tap: [G2p ; G2p<<8] -> U0 on rows 0:64, U2 on 64:128
    put(6, np.concatenate([w3t[0], w3t[2]], axis=0))
    # slot 7: G3 center tap U1 on G3d rows 0:64
    put(7, w3t[1])
    # slot 8: E3 merged tap: [E2p ; G2p<<4] -> U1 rows 0:64, U2 rows 64:128
    put(8, np.concatenate([w3t[1], w3t[2]], axis=0))
    w4 = inp['w4'].astype(np.float32)
    for j in (1, 2):    # slots 9-10: [128, 128]
        put(9 + j - 1, _blockdiag([w4[:, :, j].T.astype(np.float32)] * 2))
    fw1t = inp['fw1'].T.astype(np.float32)
    put(11, np.concatenate([fw1t, fw1t], axis=0))               # [64,128] x2 rows
    # fc2 / fc3 as M=128 with zero column-halves: psum accumulation composes
    # the two chunk-halves onto partitions 0:64 / 64:128 without col-tiling.
    fw2t = inp['fw2'].T.astype(np.float32)          # [128, 64]
    z64 = np.zeros_like(fw2t)
    put(12, np.concatenate([fw2t, z64], axis=1))    # fc2_lo [128, 128]
    put(13, np.concatenate([z64, fw2t], axis=1))    # fc2_hi
    fw3t = _blockdiag([inp['fw3'].T.astype(np.float32)] * 2)       # [128, 64]
    z64b = np.zeros_like(fw3t)
    put(14, np.concatenate([fw3t, z64b], axis=1))   # fc3_lo
    put(15, np.concatenate([z64b, fw3t], axis=1))   # fc3_hi
    put(16, _blockdiag([inp['fcw'].T.astype(np.float32)] * 4))     # [128, 64]

    wb = np.zeros((128, 8), np.float32)
    wb[:, 0] = np.tile(inp['b2'], 4)
    wb[:, 1] = np.tile(inp['b3'], 2)
    wb[:, 2] = np.tile(inp['b4'], 2)
    wb[:, 3] = inp['fb1']
    wb[:, 4] = np.tile(inp['fb2'], 2)
    wb[:, 5] = np.tile(inp['fb3'], 4)
    return wp.astype(BF), wb


def _split(lo, hi, step=512):
    return [(a, min(a + step, hi)) for a in range(lo, hi, step)]


def _build_program(reps=1):
    import concourse.bacc as bacc
    import concourse.mybir as mybir
    import concourse.tile as tile

    F32 = mybir.dt.float32
    BF16 = mybir.dt.bfloat16

    nc = bacc.Bacc("TRN2", target_bir_lowering=False, debug=False)
    x_d = nc.dram_tensor("x", [BPC, 4, C + 1, Tc + 22], BF16, kind="ExternalInput").ap()
    w_d = nc.dram_tensor("wpack", [128, NSLOT * 128], BF16, kind="ExternalInput").ap()
    wb_d = nc.dram_tensor("wbias", [128, 8], F32, kind="ExternalInput").ap()
    o_d = nc.dram_tensor("out", [BPC, 4, 16, Tc], BF16, kind="ExternalOutput").ap()

    with tile.TileContext(nc) as tc:
        with tc.tile_pool(name="wp", bufs=1) as wpool, \
             tc.tile_pool(name="xp", bufs=1) as xpool, \
             tc.tile_pool(name="yp", bufs=1) as ypool, \
             tc.tile_pool(name="st", bufs=ST_BUFS) as spool, \
             tc.tile_pool(name="psa", bufs=PS_BUFS, space="PSUM") as ppool_a:

            wsb = wpool.tile([128, NSLOT * 128], BF16, tag="w")
            wbb = wpool.tile([128, 8], F32, tag="wb")

            for _rep in range(reps):
                _emit_body(nc, tc, mybir, F32, BF16, wsb, wbb,
                           xpool, ypool, spool, (ppool_a, ppool_a), x_d, o_d,
                           w_d, wb_d)
    nc.finalize()
    return nc


def _emit_body(nc, tc, mybir, F32, BF16, wsb, wbb,
               xpool, ypool, spool, ppool, x_d, o_d):
    AF = mybir.ActivationFunctionType
    OP = mybir.AluOpType

    def lhsT(slot, k=128, m=128, base=0):
        return wsb[base:base + k, slot * 128: slot * 128 + m]

    def bias(i):
        return wbb[:, i: i + 1]

    # ---------------- input load: per batch [120p = 4 x (29ch + ones), W]
    # host pre-pads x to [4, 30, Tc+22] (chunk-major so one DMA fills all 120
    # partitions); each load covers tile cols [6, Tc+28) in 4 column pieces.
    X = [None] * BPC
    for b in range(BPC):
        X[b] = xpool.tile([120, W], BF16, tag="x", name=f"x{b}")
    XP = [(0, 1050), (1050, Tc + 22)]
    nc.sync.dma_start(out=wsb[:, 0:384], in_=w_d[:, 0:384])
    for (s0, s1) in XP:
        nc.sync.dma_start(out=X[0][0:120, 6 + s0: 6 + s1],
                          in_=x_d[0, :, :, s0: s1])
    nc.sync.dma_start(out=wbb[:], in_=wb_d[:])
    nc.sync.dma_start(out=wsb[:, 384:], in_=w_d[:, 384:])
    for (s0, s1) in XP:
        nc.sync.dma_start(out=X[1][0:120, 6 + s0: 6 + s1],
                          in_=x_d[1, :, :, s0: s1])

    Y = ypool.tile([64, BPC * Tc], BF16, tag="y")

    CB = 1037  # half boundary (tile col)

    def make_layers(b):
        ST = lambda nm: spool.tile([128, W], BF16, tag="st", name=f"{nm}_{b}")  # noqa: E731

        def conv_pass(out_tile, rng, taps, evac, pool, half):
            """taps: list of (lhsT_ap, rhs_tile, rp0, rp1, delta)."""
            lo = rng[0] if half == 0 else CB
            hi = CB if half == 0 else rng[1]
            for (glo, ghi) in _split(lo, hi, PS_GROUP):
                gn = ghi - glo
                ps = pool.tile([128, PS_GROUP], F32, tag="ps", name="ps")
                for (lo, hi) in _split(glo, ghi, 512):
                    n, off = hi - lo, lo - glo
                    for i, (lw, rt, rp0, rp1, d) in enumerate(taps):
                        nc.tensor.matmul(
                            ps[0:lw.shape[-1], off:off + n], lw,
                            rt[rp0:rp1, lo + d: hi + d],
                            start=(i == 0), stop=(i == len(taps) - 1))
                evac(ps[:, 0:gn], out_tile[:, glo:ghi])

        def pool_for(name):
            return ppool_a if ROUTE.get(name, 'act') == 'act' else ppool_d

        def psg_for(name):
            return PSA_GROUP if ROUTE.get(name, 'act') == 'act' else PS_GROUP

        def evac_for(name, alpha, bias_i, half=0):
            route = ROUTE.get(name, 'act')
            # tail: the last job's DVE-routed drains serialize behind the
            # window-sum/clip chain while Act idles; shift them to Act
            if TAILACT and b == BPC - 1 and half == len(CBS) and route in ('dve2',):
                route = 'act'

            def act(ps, ot):
                nc.scalar.activation(ot, ps, AF.Prelu,
                                     bias=bias(bias_i) if bias_i is not None else 0.0,
                                     scale=1.0, alpha=alpha)

            def dve1(ps, ot):
                # one PSUM read max per instruction: copy out, then leaky in SBUF
                nc.vector.tensor_scalar(ot, ps, 0.0, None, OP.add)
                nc.vector.scalar_tensor_tensor(ot, ot, alpha, ot, OP.mult, OP.max)

            def dve2(ps, ot):
                nc.vector.tensor_scalar(ot, ps, bias(bias_i), None, OP.add)
                nc.vector.scalar_tensor_tensor(ot, ot, alpha, ot, OP.mult, OP.max)

            def dver(ps, ot):
                nc.vector.tensor_scalar(
                    ot, ps, bias(bias_i) if bias_i is not None else 0.0, 0.0,
                    OP.add, OP.max)

            return {'act': act, 'dve1': dve1, 'dve2': dve2, 'dver': dver}[route]

        tiles = {}

        def getst(nm):
            if nm not in tiles:
                tiles[nm] = ST(nm)
            return tiles[nm]

        def l0(half):  # G1 + E1
            G1 = getst("G1")
            conv_pass(G1, (14, Tc + 34),
                      [(lhsT(k, 120), X[b], 0, 120, k - 8) for k in range(3)],
                      evac_for('G1', 0.02, None, half), pool_for('G1'), half, psg_for('G1'))
            E1 = getst("E1")
            conv_pass(E1, (13, Tc + 21),
                      [(lhsT(k, 120), X[b], 0, 120, k - 8) for k in (1, 2)],
                      evac_for('E1', 0.02, None, half), pool_for('E1'), half, psg_for('E1'))

        def l1(half):  # G2 + E2
            G1, E1 = tiles['G1'], tiles['E1']
            G2 = getst("G2")
            conv_pass(G2, (17, Tc + 33),
                      [(lhsT(3 + k), G1, 0, 128, 2 * (k - 1)) for k in range(3)],
                      evac_for('G2', 0.02, 0, half), pool_for('G2'), half, psg_for('G2'))
            E2 = getst("E2")
            conv_pass(E2, (13, Tc + 21),
                      [(lhsT(4), E1, 0, 128, 0), (lhsT(5), G1, 0, 128, 2)],
                      evac_for('E2', 0.02, 0, half), pool_for('E2'), half, psg_for('E2'))

        def l2(half):  # assemble [stream ; shifted-copy] tiles
            G2, E2 = tiles['G2'], tiles['E2']
            G3d = [getst("G3d0"), getst("G3d1")]
            EG3d = [getst("EG3d0"), getst("EG3d1")]
            tiles['G3d'], tiles['EG3d'] = G3d, EG3d
            for p in range(2):
                (a0, a1) = (17, CB) if half == 0 else (CB, Tc + 29)
                nc.gpsimd.dma_start(out=G3d[p][0:64, a0: a1],
                                  in_=G2[64 * p:64 * p + 64, a0: a1])
                (a0, a1) = (17, CB) if half == 0 else (CB, Tc + 25)
                nc.gpsimd.dma_start(out=G3d[p][64:128, a0: a1],
                                  in_=G2[64 * p:64 * p + 64, a0 + 8: a1 + 8])
                (a0, a1) = (13, CB) if half == 0 else (CB, Tc + 21)
                nc.sync.dma_start(out=EG3d[p][0:64, a0: a1],
                                  in_=E2[64 * p:64 * p + 64, a0: a1])
                nc.sync.dma_start(out=EG3d[p][64:128, a0: a1],
                                  in_=G2[64 * p:64 * p + 64, a0 + 4: a1 + 4])

        def l3(half):  # G3 + E3
            G3d, EG3d = tiles['G3d'], tiles['EG3d']
            tiles['G3'] = G3 = [getst("G3a"), getst("G3b")]
            for p in range(2):
                conv_pass(G3[p], (21, Tc + 29),
                          [(lhsT(6), G3d[p], 0, 128, -4),
                           (lhsT(7, 64), G3d[p], 0, 64, 0)],
                          evac_for(f'G3_{p}', 0.2, 1, half), pool_for(f'G3_{p}'), half, psg_for(f'G3_{p}'))
            tiles['E3'] = E3 = [getst("E3a"), getst("E3b")]
            for p in range(2):
                conv_pass(E3[p], (13, Tc + 21),
                          [(lhsT(8), EG3d[p], 0, 128, 0)],
                          evac_for('E3', 0.2, 1, half), pool_for('E3'), half, psg_for('E3'))

        def l4(half):  # H
            G3, E3 = tiles['G3'], tiles['E3']
            tiles['H'] = H = [getst("Ha"), getst("Hb")]
            for p in range(2):
                conv_pass(H[p], (13, Tc + 21),
                          [(lhsT(9), E3[p], 0, 128, 0),
                           (lhsT(10), G3[p], 0, 128, 8)],
                          evac_for('H', 0.2, 2, half), pool_for('H'), half, psg_for('H'))

        def l5(half):  # H1 (fc1)
            H = tiles['H']
            tiles['H1'] = H1 = [getst("H1" + str(c)) for c in range(4)]
            for cidx in range(4):
                p, ph = cidx // 2, cidx % 2
                conv_pass(H1[cidx], (13, Tc + 21),
                          [(lhsT(11, 64, base=64 * ph), H[p],
                            64 * ph, 64 * ph + 64, 0)],
                          evac_for(f'H1_{cidx}', 0.02, 3, half), pool_for(f'H1_{cidx}'), half, psg_for(f'H1_{cidx}'))

        def l6(half):  # A2 (fc2)
            H1 = tiles['H1']
            tiles['A2'] = A2 = [getst("A2a"), getst("A2b")]
            for p in range(2):
                conv_pass(A2[p], (13, Tc + 21),
                          [(lhsT(12), H1[2 * p], 0, 128, 0),
                           (lhsT(13), H1[2 * p + 1], 0, 128, 0)],
                          evac_for('A2', 0.02, 4, half), pool_for('A2'), half, psg_for('A2'))

        def l7(half):  # FFC (fc3 + tanh)
            A2 = tiles['A2']
            tiles['FFC'] = FFC = getst("FFC")

            def tanh_evac(ps, ot):
                nc.scalar.activation(ot, ps, AF.Tanh, bias=bias(5), scale=1.0)

            conv_pass(FFC, (13, Tc + 21),
                      [(lhsT(14), A2[0], 0, 128, 0),
                       (lhsT(15), A2[1], 0, 128, 0)],
                      tanh_evac, ppool_a, half, PSA_GROUP)
            # phantom edge values must read as zero in the window sum
            if half == 0:
                nc.gpsimd.memset(FFC[0:32, 13:16], 0.0)
            else:
                nc.gpsimd.memset(FFC[96:128, Tc + 16: Tc + 21], 0.0)

        def l8(half):  # window-sum tree (8-wide): piece-split for pipelining
            # half 0 owns cols [.., CB), half 1 [CB, ..); ops at the boundary
            # read a few columns across it (producer half finished earlier).
            FFC = tiles['FFC']
            S1 = getst("S1")
            tiles['S1'] = S1
            SSTEP = 512
            # Pool is slow (no 16-bit speedup); the final job's sums sit on
            # the critical tail, so run those on DVE instead.
            eng = nc.gpsimd if (b == 0 and half == 0) else nc.vector
            # staggered split points: each op's half-0 range ends before the
            # columns that would read the NEXT op stage across the boundary
            for (p0, p1) in _split(13 if half == 0 else CB + 3,
                                   CB + 3 if half == 0 else Tc + 19, SSTEP):
                eng.tensor_tensor(S1[:, p0: p1], FFC[:, p0: p1],
                                  FFC[:, p0 + 1: p1 + 1], OP.add)
            for (p0, p1) in _split(13 if half == 0 else CB + 1,
                                   CB + 1 if half == 0 else Tc + 17, SSTEP):
                eng.tensor_tensor(FFC[:, p0: p1], S1[:, p0: p1],
                                  S1[:, p0 + 2: p1 + 2], OP.add)
            for (p0, p1) in _split(16 if half == 0 else CB,
                                   CB if half == 0 else Tc + 16, SSTEP):
                eng.tensor_tensor(S1[:, p0: p1], FFC[:, p0 - 3: p1 - 3],
                                  FFC[:, p0 + 1: p1 + 1], OP.add)

        def l9(half):  # final fc + clip + output DMA
            S1 = tiles['S1']
            for (glo, ghi) in _split(16 if half == 0 else CB,
                                     CB if half == 0 else Tc + 16, PS_GROUP):
                ps = ppool_d.tile([128, PS_GROUP], F32, tag=f"ps{PS_GROUP}", name="ps",
                                  bufs=PS_BUFS)
                for (lo, hi) in _split(glo, ghi, 512):
                    n, off = hi - lo, lo - glo
                    nc.tensor.matmul(ps[0:64, off:off + n], lhsT(16, 128, 64),
                                     S1[:, lo: hi], start=True, stop=True)
                    nc.vector.tensor_scalar(
                        Y[:, b * Tc + lo - 16: b * Tc + hi - 16],
                        ps[0:64, off:off + n], 0.0, 1.0, OP.max, OP.min)
            (h0, h1) = (0, CB - 16) if half == 0 else (CB - 16, Tc)
            nc.sync.dma_start(out=o_d[b, :, :, h0:h1],
                              in_=Y[0:64, b * Tc + h0: b * Tc + h1])

        return [l0, l1, l2, l3, l4, l5, l6, l7, l8, l9]

    batch_layers = [make_layers(b) for b in range(BPC)]
    jobs = [(0, 0), (0, 1), (1, 0), (1, 1)]
    L = len(batch_layers[0])
    for k in range(L + SKEW * (len(jobs) - 1)):
        # emit deeper-pipelined jobs first: a half-0 layer reads a few
        # boundary columns from the next job's previous layer, which must
        # appear earlier in program order for the dependency to register
        for j, (b, h) in reversed(list(enumerate(jobs))):
            kk = k - SKEW * j
            if 0 <= kk < L:
                batch_layers[b][kk](h)


def _get_program(reps=1):
    global _PROG
    if _PROG is None:
        _PROG = {}
    if reps not in _PROG:
        _PROG[reps] = _build_program(reps)
    return _PROG[reps]


def _prepare_inputs(inputs):
    x = np.asarray(inputs['speech_features'], np.float32)
    xp = np.zeros((B, C + 1, T + 22), np.float32)
    xp[:, :C, 10:10 + T] = x
    xp[:, C, :] = 1.0
    # chunk-major: [B, 4, 30, Tc+22]; chunk c covers padded cols [c*Tc, c*Tc+Tc+22)
    xa = np.zeros((B, 4, C + 1, Tc + 22), np.float32)
    for c in range(4):
        xa[:, c] = xp[:, :, c * Tc: c * Tc + Tc + 22]
    xa = xa.astype(BF)
    wp, wb = _pack_weights({k: np.asarray(v, np.float32)
                            for k, v in inputs.items() if k != 'speech_features'})
    return [{"x": xa[i * BPC:(i + 1) * BPC], "wpack": wp, "wbias": wb}
            for i in range(NCORES)]


def kernel(**inputs):
    from concourse.bass_utils import run_bass_kernel_spmd

    in_maps = _prepare_inputs(inputs)
    nc = _get_program()
    res = run_bass_kernel_spmd(nc, in_maps, core_ids=list(range(NCORES)))
    outs = []
    for r in res.results:
        o = np.asarray(r["out"]).astype(np.float32)      # [BPC, 4, 16, Tc]
        o = o.transpose(0, 1, 3, 2).reshape(BPC, T, 16)  # [BPC, T, 16]
        outs.append(o)
    return np.ascontiguousarray(np.concatenate(outs, axis=0))


# revision 43
# speedup vs baseline: 1.0203x; 1.0066x over previous
"""Trainium2 Bass kernel for nn_CNNGenerator (frame CNN + FC + window-sum + FC).

Key algebraic facts exploited (validated vs the reference):
  * softmax over a size-1 axis == 1.0, so the whole attention_conv stack is
    dead code; the bmm reduces to an 8-wide sliding-window sum of ffc.
  * The per-window stride-2 conv stack collapses into global conv streams:
    an "interior" stream g{1,2,3} and a "left-edge" stream e{1,2,3} per
    layer, plus a 2-tap combine (z).  Per t:
      g1[s] = b1 + sum_k W1k x[s+k-8]          e1[t] = b1 + W11 x[t-7] + W12 x[t-6]
      g2[s] = b2 + V0 G1[s-2] + V1 G1[s] + V2 G1[s+2]
      e2[t] = b2 + V1 E1[t] + V2 G1[t+2]
      g3[s] = b3 + U0 G2[s-4] + U1 G2[s] + U2 G2[s+4]
      e3[t] = b3 + U1 E2[t] + U2 G2[t+4]
      z[t]  = b4 + T1 E3[t] + T2 G3[t+8]
    (capitals = leaky-activated streams), then fc1/fc2/fc3+tanh,
    ws[t] = sum_{d=-3..4} ffc[t+d], out = clip(fcw @ ws, 0, 1).

Sharding: pure data parallel, 2 batch elements per core on 8 cores.
On-chip layout: time axis split in 4 chunks of 2048; 32-channel streams pack
4 chunks x 32ch on the 128 partitions, 64-channel streams pack 2 chunks x 64ch
(two tiles).  All matmul operands are bf16 (fp32 PSUM accumulate); evacuation
work is split between the Activation engine (Prelu/Tanh) and DVE (one-op
leaky via scalar_tensor_tensor).  G3 runs as 2 matmuls (not 3) and E3 as 1
(not 2) against DMA-assembled tiles that stack a stream with a column-shifted
copy on the partition axis, doubling effective contraction per column.
"""
import sys

if '/opt/trn_rl_repo' not in sys.path:
    sys.path.insert(0, '/opt/trn_rl_repo')

import numpy as np
import ml_dtypes

BF = ml_dtypes.bfloat16

B, C, T = 16, 29, 8192
NCORES = 8
BPC = B // NCORES          # batch per core
Tc = T // 4                # time chunk
HL = 16                    # left halo: tile col u <-> global idx c*Tc + u - HL
W = Tc + 40                # per-batch stream tile width
NSLOT = 17                 # 128-col lhsT slots in the weight pack

_PROG = {}
PS_GROUP = 1024
PS_BUFS = 4
ST_BUFS = 22

# evacuation routing: which streams drain on DVE instead of Activation.
# 'dve1' = one-op leaky (psum already contains bias), 'dve2' = bias-add +
# leaky (two DVE ops), 'act' = Activation Prelu.
ROUTE = {
    'G1': 'act', 'E1': 'act',
    'G2': 'act', 'E2': 'dve2',
    'G3_0': 'dve2', 'G3_1': 'act', 'E3': 'act', 'H': 'act',
    'H1_0': 'dve2', 'H1_1': 'act', 'H1_2': 'dve2', 'H1_3': 'act',
    'A2': 'act', 'FFC': 'act',
}
SKEW = 3


def _blockdiag(blocks):
    k = sum(b.shape[0] for b in blocks)
    m = sum(b.shape[1] for b in blocks)
    out = np.zeros((k, m), np.float32)
    i = j = 0
    for b in blocks:
        out[i:i + b.shape[0], j:j + b.shape[1]] = b
        i += b.shape[0]
        j += b.shape[1]
    return out


def _pack_weights(inp):
    wp = np.zeros((128, NSLOT * 128), np.float32)

    def put(slot, mat):
        wp[:mat.shape[0], slot * 128: slot * 128 + mat.shape[1]] = mat

    w1 = inp['w1'].astype(np.float32)  # [32, 29, 3]
    b1 = inp['b1'].astype(np.float32)
    # slots 0-2: g1 taps: blockdiag4 of [30, 32]: 29 in-ch rows + bias row
    for k in range(3):
        blk = np.zeros((30, 32), np.float32)
        blk[:29] = w1[:, :, k].T
        if k == 1:
            blk[29] = b1
        put(k, _blockdiag([blk] * 4))
    w2 = inp['w2'].astype(np.float32)
    for k in range(3):  # slots 3-5
        put(3 + k, _blockdiag([w2[:, :, k].T.astype(np.float32)] * 4))
    w3 = inp['w3'].astype(np.float32)
    w3t = [_blockdiag([w3[:, :, k].T.astype(np.float32)] * 2) for k in range(3)]
    # slot 6: G3 merged tap: [G2p ; G2p<<8] -> U0 on rows 0:64, U2 on 64:128
    put(6, np.concatenate([w3t[0], w3t[2]], axis=0))
    # slot 7: G3 center tap U1 on G3d rows 0:64
    put(7, w3t[1])
    # slot 8: E3 merged tap: [E2p ; G2p<<4] -> U1 rows 0:64, U2 rows 64:128
    put(8, np.concatenate([w3t[1], w3t[2]], axis=0))
    w4 = inp['w4'].astype(np.float32)
    for j in (1, 2):    # slots 9-10: [128, 128]
        put(9 + j - 1, _blockdiag([w4[:, :, j].T.astype(np.float32)] * 2))
    fw1t = inp['fw1'].T.astype(np.float32)
    put(11, np.concatenate([fw1t, fw1t], axis=0))               # [64,128] x2 rows
    # fc2 / fc3 as M=128 with zero column-halves: psum accumulation composes
    # the two chunk-halves onto partitions 0:64 / 64:128 without col-tiling.
    fw2t = inp['fw2'].T.astype(np.float32)          # [128, 64]
    z64 = np.zeros_like(fw2t)
    put(12, np.concatenate([fw2t, z64], axis=1))    # fc2_lo [128, 128]
    put(13, np.concatenate([z64, fw2t], axis=1))    # fc2_hi
    fw3t = _blockdiag([inp['fw3'].T.astype(np.float32)] * 2)       # [128, 64]
    z64b = np.zeros_like(fw3t)
    put(14, np.concatenate([fw3t, z64b], axis=1))   # fc3_lo
    put(15, np.concatenate([z64b, fw3t], axis=1))   # fc3_hi
    put(16, _blockdiag([inp['fcw'].T.astype(np.float32)] * 4))     # [128, 64]

    wb = np.zeros((128, 8), np.float32)
    wb[:, 0] = np.tile(inp['b2'], 4)
    wb[:, 1] = np.tile(inp['b3'], 2)
    wb[:, 2] = np.tile(inp['b4'], 2)
    wb[:, 3] = inp['fb1']
    wb[:, 4] = np.tile(inp['fb2'], 2)
    wb[:, 5] = np.tile(inp['fb3'], 4)
    return wp.astype(BF), wb


def _split(lo, hi, step=512):
    return [(a, min(a + step, hi)) for a in range(lo, hi, step)]


def _build_program(reps=1):
    import concourse.bacc as bacc
    import concourse.mybir as mybir
    import concourse.tile as tile

    F32 = mybir.dt.float32
    BF16 = mybir.dt.bfloat16

    nc = bacc.Bacc("TRN2", target_bir_lowering=False, debug=False)
    x_d = nc.dram_tensor("x", [BPC, 4, C + 1, Tc + 22], BF16, kind="ExternalInput").ap()
    w_d = nc.dram_tensor("wpack", [128, NSLOT * 128], BF16, kind="ExternalInput").ap()
    wb_d = nc.dram_tensor("wbias", [128, 8], F32, kind="ExternalInput").ap()
    o_d = nc.dram_tensor("out", [BPC, 4, 16, Tc], BF16, kind="ExternalOutput").ap()

    with tile.TileContext(nc) as tc:
        with tc.tile_pool(name="wp", bufs=1) as wpool, \
             tc.tile_pool(name="xp", bufs=1) as xpool, \
             tc.tile_pool(name="yp", bufs=1) as ypool, \
             tc.tile_pool(name="st", bufs=ST_BUFS) as spool, \
             tc.tile_pool(name="psa", bufs=PS_BUFS, space="PSUM") as ppool_a:

            wsb = wpool.tile([128, NSLOT * 128], BF16, tag="w")
            wbb = wpool.tile([128, 8], F32, tag="wb")

            for _rep in range(reps):
                _emit_body(nc, tc, mybir, F32, BF16, wsb, wbb,
                           xpool, ypool, spool, (ppool_a, ppool_a), x_d, o_d,
                           w_d, wb_d)
    nc.finalize()
    return nc


def _emit_body(nc, tc, mybir, F32, BF16, wsb, wbb,
               xpool, ypool, spool, ppool, x_d, o_d):
    AF = mybir.ActivationFunctionType
    OP = mybir.AluOpType

    def lhsT(slot, k=128, m=128, base=0):
        return wsb[base:base + k, slot * 128: slot * 128 + m]

    def bias(i):
        return wbb[:, i: i + 1]

    # ---------------- input load: per batch [120p = 4 x (29ch + ones), W]
    # host pre-pads x to [4, 30, Tc+22] (chunk-major so one DMA fills all 120
    # partitions); each load covers tile cols [6, Tc+28) in 4 column pieces.
    X = [None] * BPC
    for b in range(BPC):
        X[b] = xpool.tile([120, W], BF16, tag="x", name=f"x{b}")
    XP = [(0, 1050), (1050, Tc + 22)]
    nc.sync.dma_start(out=wsb[:, 0:384], in_=w_d[:, 0:384])
    for (s0, s1) in XP:
        nc.sync.dma_start(out=X[0][0:120, 6 + s0: 6 + s1],
                          in_=x_d[0, :, :, s0: s1])
    nc.sync.dma_start(out=wbb[:], in_=wb_d[:])
    nc.sync.dma_start(out=wsb[:, 384:], in_=w_d[:, 384:])
    for (s0, s1) in XP:
        nc.sync.dma_start(out=X[1][0:120, 6 + s0: 6 + s1],
                          in_=x_d[1, :, :, s0: s1])

    Y = ypool.tile([64, BPC * Tc], BF16, tag="y")

    CB = 1037  # half boundary (tile col)

    def make_layers(b):
        ST = lambda nm: spool.tile([128, W], BF16, tag="st", name=f"{nm}_{b}")  # noqa: E731

        def conv_pass(out_tile, rng, taps, evac, pool, half):
            """taps: list of (lhsT_ap, rhs_tile, rp0, rp1, delta)."""
            lo = rng[0] if half == 0 else CB
            hi = CB if half == 0 else rng[1]
            for (glo, ghi) in _split(lo, hi, PS_GROUP):
                gn = ghi - glo
                ps = pool.tile([128, PS_GROUP], F32, tag="ps", name="ps")
                for (lo, hi) in _split(glo, ghi, 512):
                    n, off = hi - lo, lo - glo
                    for i, (lw, rt, rp0, rp1, d) in enumerate(taps):
                        nc.tensor.matmul(
                            ps[0:lw.shape[-1], off:off + n], lw,
                            rt[rp0:rp1, lo + d: hi + d],
                            start=(i == 0), stop=(i == len(taps) - 1))
                evac(ps[:, 0:gn], out_tile[:, glo:ghi])

        def pool_for(name):
            return ppool_a if ROUTE.get(name, 'act') == 'act' else ppool_d

        def psg_for(name):
            return PSA_GROUP if ROUTE.get(name, 'act') == 'act' else PS_GROUP

        def evac_for(name, alpha, bias_i, half=0):
            route = ROUTE.get(name, 'act')
            # tail: the last job's DVE-routed drains serialize behind the
            # window-sum/clip chain while Act idles; shift them to Act
            if TAILACT and b == BPC - 1 and half == len(CBS) and route in ('dve2',):
                route = 'act'

            def act(ps, ot):
                nc.scalar.activation(ot, ps, AF.Prelu,
                                     bias=bias(bias_i) if bias_i is not None else 0.0,
                                     scale=1.0, alpha=alpha)

            def dve1(ps, ot):
                # one PSUM read max per instruction: copy out, then leaky in SBUF
                nc.vector.tensor_scalar(ot, ps, 0.0, None, OP.add)
                nc.vector.scalar_tensor_tensor(ot, ot, alpha, ot, OP.mult, OP.max)

            def dve2(ps, ot):
                nc.vector.tensor_scalar(ot, ps, bias(bias_i), None, OP.add)
                nc.vector.scalar_tensor_tensor(ot, ot, alpha, ot, OP.mult, OP.max)

            def dver(ps, ot):
                nc.vector.tensor_scalar(
                    ot, ps, bias(bias_i) if bias_i is not None else 0.0, 0.0,
                    OP.add, OP.max)

            return {'act': act, 'dve1': dve1, 'dve2': dve2, 'dver': dver}[route]

        tiles = {}

        def getst(nm):
            if nm not in tiles:
                tiles[nm] = ST(nm)
            return tiles[nm]

        def l0(half):  # G1 + E1
            G1 = getst("G1")
            conv_pass(G1, (14, Tc + 34),
                      [(lhsT(k, 120), X[b], 0, 120, k - 8) for k in range(3)],
                      evac_for('G1', 0.02, None, half), pool_for('G1'), half, psg_for('G1'))
            E1 = getst("E1")
            conv_pass(E1, (13, Tc + 21),
                      [(lhsT(k, 120), X[b], 0, 120, k - 8) for k in (1, 2)],
                      evac_for('E1', 0.02, None, half), pool_for('E1'), half, psg_for('E1'))

        def l1(half):  # G2 + E2
            G1, E1 = tiles['G1'], tiles['E1']
            G2 = getst("G2")
            conv_pass(G2, (17, Tc + 33),
                      [(lhsT(3 + k), G1, 0, 128, 2 * (k - 1)) for k in range(3)],
                      evac_for('G2', 0.02, 0, half), pool_for('G2'), half, psg_for('G2'))
            E2 = getst("E2")
            conv_pass(E2, (13, Tc + 21),
                      [(lhsT(4), E1, 0, 128, 0), (lhsT(5), G1, 0, 128, 2)],
                      evac_for('E2', 0.02, 0, half), pool_for('E2'), half, psg_for('E2'))

        def l2(half):  # assemble [stream ; shifted-copy] tiles
            G2, E2 = tiles['G2'], tiles['E2']
            G3d = [getst("G3d0"), getst("G3d1")]
            EG3d = [getst("EG3d0"), getst("EG3d1")]
            tiles['G3d'], tiles['EG3d'] = G3d, EG3d
            for p in range(2):
                (a0, a1) = (17, CB) if half == 0 else (CB, Tc + 29)
                nc.gpsimd.dma_start(out=G3d[p][0:64, a0: a1],
                                  in_=G2[64 * p:64 * p + 64, a0: a1])
                (a0, a1) = (17, CB) if half == 0 else (CB, Tc + 25)
                nc.gpsimd.dma_start(out=G3d[p][64:128, a0: a1],
                                  in_=G2[64 * p:64 * p + 64, a0 + 8: a1 + 8])
                (a0, a1) = (13, CB) if half == 0 else (CB, Tc + 21)
                nc.sync.dma_start(out=EG3d[p][0:64, a0: a1],
                                  in_=E2[64 * p:64 * p + 64, a0: a1])
                nc.sync.dma_start(out=EG3d[p][64:128, a0: a1],
                                  in_=G2[64 * p:64 * p + 64, a0 + 4: a1 + 4])

        def l3(half):  # G3 + E3
            G3d, EG3d = tiles['G3d'], tiles['EG3d']
            tiles['G3'] = G3 = [getst("G3a"), getst("G3b")]
            for p in range(2):
                conv_pass(G3[p], (21, Tc + 29),
                          [(lhsT(6), G3d[p], 0, 128, -4),
                           (lhsT(7, 64), G3d[p], 0, 64, 0)],
                          evac_for(f'G3_{p}', 0.2, 1, half), pool_for(f'G3_{p}'), half, psg_for(f'G3_{p}'))
            tiles['E3'] = E3 = [getst("E3a"), getst("E3b")]
            for p in range(2):
                conv_pass(E3[p], (13, Tc + 21),
                          [(lhsT(8), EG3d[p], 0, 128, 0)],
                          evac_for('E3', 0.2, 1, half), pool_for('E3'), half, psg_for('E3'))

        def l4(half):  # H
            G3, E3 = tiles['G3'], tiles['E3']
            tiles['H'] = H = [getst("Ha"), getst("Hb")]
            for p in range(2):
                conv_pass(H[p], (13, Tc + 21),
                          [(lhsT(9), E3[p], 0, 128, 0),
                           (lhsT(10), G3[p], 0, 128, 8)],
                          evac_for('H', 0.2, 2, half), pool_for('H'), half, psg_for('H'))

        def l5(half):  # H1 (fc1)
            H = tiles['H']
            tiles['H1'] = H1 = [getst("H1" + str(c)) for c in range(4)]
            for cidx in range(4):
                p, ph = cidx // 2, cidx % 2
                conv_pass(H1[cidx], (13, Tc + 21),
                          [(lhsT(11, 64, base=64 * ph), H[p],
                            64 * ph, 64 * ph + 64, 0)],
                          evac_for(f'H1_{cidx}', 0.02, 3, half), pool_for(f'H1_{cidx}'), half, psg_for(f'H1_{cidx}'))

        def l6(half):  # A2 (fc2)
            H1 = tiles['H1']
            tiles['A2'] = A2 = [getst("A2a"), getst("A2b")]
            for p in range(2):
                conv_pass(A2[p], (13, Tc + 21),
                          [(lhsT(12), H1[2 * p], 0, 128, 0),
                           (lhsT(13), H1[2 * p + 1], 0, 128, 0)],
                          evac_for('A2', 0.02, 4, half), pool_for('A2'), half, psg_for('A2'))

        def l7(half):  # FFC (fc3 + tanh)
            A2 = tiles['A2']
            tiles['FFC'] = FFC = getst("FFC")

            def tanh_evac(ps, ot):
                nc.scalar.activation(ot, ps, AF.Tanh, bias=bias(5), scale=1.0)

            conv_pass(FFC, (13, Tc + 21),
                      [(lhsT(14), A2[0], 0, 128, 0),
                       (lhsT(15), A2[1], 0, 128, 0)],
                      tanh_evac, ppool_a, half, PSA_GROUP)
            # phantom edge values must read as zero in the window sum
            if half == 0:
                nc.gpsimd.memset(FFC[0:32, 13:16], 0.0)
            else:
                nc.gpsimd.memset(FFC[96:128, Tc + 16: Tc + 21], 0.0)

        def l8(half):  # window-sum tree (8-wide): piece-split for pipelining
            # half 0 owns cols [.., CB), half 1 [CB, ..); ops at the boundary
            # read a few columns across it (producer half finished earlier).
            FFC = tiles['FFC']
            S1 = getst("S1")
            tiles['S1'] = S1
            SSTEP = 512
            # Pool is slow (no 16-bit speedup); the final job's sums sit on
            # the critical tail, so run those on DVE instead.
            eng = nc.vector
            # staggered split points: each op's half-0 range ends before the
            # columns that would read the NEXT op stage across the boundary
            for (p0, p1) in _split(13 if half == 0 else CB + 3,
                                   CB + 3 if half == 0 else Tc + 19, SSTEP):
                eng.tensor_tensor(S1[:, p0: p1], FFC[:, p0: p1],
                                  FFC[:, p0 + 1: p1 + 1], OP.add)
            for (p0, p1) in _split(13 if half == 0 else CB + 1,
                                   CB + 1 if half == 0 else Tc + 17, SSTEP):
                eng.tensor_tensor(FFC[:, p0: p1], S1[:, p0: p1],
                                  S1[:, p0 + 2: p1 + 2], OP.add)
            for (p0, p1) in _split(16 if half == 0 else CB,
                                   CB if half == 0 else Tc + 16, SSTEP):
                eng.tensor_tensor(S1[:, p0: p1], FFC[:, p0 - 3: p1 - 3],
                                  FFC[:, p0 + 1: p1 + 1], OP.add)

        def l9(half):  # final fc + clip + output DMA
            S1 = tiles['S1']
            for (glo, ghi) in _split(16 if half == 0 else CB,
                                     CB if half == 0 else Tc + 16, PS_GROUP):
                ps = ppool_d.tile([128, PS_GROUP], F32, tag=f"ps{PS_GROUP}", name="ps",
                                  bufs=PS_BUFS)
                for (lo, hi) in _split(glo, ghi, 512):
                    n, off = hi - lo, lo - glo
                    nc.tensor.matmul(ps[0:64, off:off + n], lhsT(16, 128, 64),
                                     S1[:, lo: hi], start=True, stop=True)
                    nc.vector.tensor_scalar(
                        Y[:, b * Tc + lo - 16: b * Tc + hi - 16],
                        ps[0:64, off:off + n], 0.0, 1.0, OP.max, OP.min)
            (h0, h1) = (0, CB - 16) if half == 0 else (CB - 16, Tc)
            nc.sync.dma_start(out=o_d[b, :, :, h0:h1],
                              in_=Y[0:64, b * Tc + h0: b * Tc + h1])

        return [l0, l1, l2, l3, l4, l5, l6, l7, l8, l9]

    batch_layers = [make_layers(b) for b in range(BPC)]
    jobs = [(0, 0), (0, 1), (1, 0), (1, 1)]
    L = len(batch_layers[0])
    for k in range(L + SKEW * (len(jobs) - 1)):
        # emit deeper-pipelined jobs first: a half-0 layer reads a few
        # boundary columns from the next job's previous layer, which must
        # appear earlier in program order for the dependency to register
        for j, (b, h) in reversed(list(enumerate(jobs))):
            kk = k - SKEW * j
            if 0 <= kk < L:
                batch_layers[b][kk](h)


def _get_program(reps=1):
    global _PROG
    if _PROG is None:
        _PROG = {}
    if reps not in _PROG:
        _PROG[reps] = _build_program(reps)
    return _PROG[reps]


def _prepare_inputs(inputs):
    x = np.asarray(inputs['speech_features'], np.float32)
    xp = np.zeros((B, C + 1, T + 22), np.float32)
    xp[:, :C, 10:10 + T] = x
    xp[:, C, :] = 1.0
    # chunk-major: [B, 4, 30, Tc+22]; chunk c covers padded cols [c*Tc, c*Tc+Tc+22)
    xa = np.zeros((B, 4, C + 1, Tc + 22), np.float32)
    for c in range(4):
        xa[:, c] = xp[:, :, c * Tc: c * Tc + Tc + 22]
    xa = xa.astype(BF)
    wp, wb = _pack_weights({k: np.asarray(v, np.float32)
                            for k, v in inputs.items() if k != 'speech_features'})
    return [{"x": xa[i * BPC:(i + 1) * BPC], "wpack": wp, "wbias": wb}
            for i in range(NCORES)]


def kernel(**inputs):
    from concourse.bass_utils import run_bass_kernel_spmd

    in_maps = _prepare_inputs(inputs)
    nc = _get_program()
    res = run_bass_kernel_spmd(nc, in_maps, core_ids=list(range(NCORES)))
    outs = []
    for r in res.results:
        o = np.asarray(r["out"]).astype(np.float32)      # [BPC, 4, 16, Tc]
        o = o.transpose(0, 1, 3, 2).reshape(BPC, T, 16)  # [BPC, T, 16]
        outs.append(o)
    return np.ascontiguousarray(np.concatenate(outs, axis=0))


# revision 46
# speedup vs baseline: 1.0287x; 1.0082x over previous
"""Trainium2 Bass kernel for nn_CNNGenerator (frame CNN + FC + window-sum + FC).

Key algebraic facts exploited (validated vs the reference):
  * softmax over a size-1 axis == 1.0, so the whole attention_conv stack is
    dead code; the bmm reduces to an 8-wide sliding-window sum of ffc.
  * The per-window stride-2 conv stack collapses into global conv streams:
    an "interior" stream g{1,2,3} and a "left-edge" stream e{1,2,3} per
    layer, plus a 2-tap combine (z).  Per t:
      g1[s] = b1 + sum_k W1k x[s+k-8]          e1[t] = b1 + W11 x[t-7] + W12 x[t-6]
      g2[s] = b2 + V0 G1[s-2] + V1 G1[s] + V2 G1[s+2]
      e2[t] = b2 + V1 E1[t] + V2 G1[t+2]
      g3[s] = b3 + U0 G2[s-4] + U1 G2[s] + U2 G2[s+4]
      e3[t] = b3 + U1 E2[t] + U2 G2[t+4]
      z[t]  = b4 + T1 E3[t] + T2 G3[t+8]
    (capitals = leaky-activated streams), then fc1/fc2/fc3+tanh,
    ws[t] = sum_{d=-3..4} ffc[t+d], out = clip(fcw @ ws, 0, 1).

Sharding: pure data parallel, 2 batch elements per core on 8 cores.
On-chip layout: time axis split in 4 chunks of 2048; 32-channel streams pack
4 chunks x 32ch on the 128 partitions, 64-channel streams pack 2 chunks x 64ch
(two tiles).  All matmul operands are bf16 (fp32 PSUM accumulate); evacuation
work is split between the Activation engine (Prelu/Tanh) and DVE (one-op
leaky via scalar_tensor_tensor).  G3 runs as 2 matmuls (not 3) and E3 as 1
(not 2) against DMA-assembled tiles that stack a stream with a column-shifted
copy on the partition axis, doubling effective contraction per column.
"""
import sys

if '/opt/trn_rl_repo' not in sys.path:
    sys.path.insert(0, '/opt/trn_rl_repo')

import numpy as np
import ml_dtypes

BF = ml_dtypes.bfloat16

B, C, T = 16, 29, 8192
NCORES = 8
BPC = B // NCORES          # batch per core
Tc = T // 4                # time chunk
HL = 16                    # left halo: tile col u <-> global idx c*Tc + u - HL
W = Tc + 40                # per-batch stream tile width
NSLOT = 17                 # 128-col lhsT slots in the weight pack

_PROG = {}
PS_GROUP = 1024
PS_BUFS = 4
ST_BUFS = 22

# evacuation routing: which streams drain on DVE instead of Activation.
# 'dve1' = one-op leaky (psum already contains bias), 'dve2' = bias-add +
# leaky (two DVE ops), 'act' = Activation Prelu.
ROUTE = {
    'G1': 'act', 'E1': 'act',
    'G2': 'act', 'E2': 'dve2',
    'G3_0': 'dve2', 'G3_1': 'act', 'E3': 'act', 'H': 'act',
    'H1_0': 'dve2', 'H1_1': 'act', 'H1_2': 'dve2', 'H1_3': 'act',
    'A2': 'act', 'FFC': 'act',
}
SKEW = 3


def _blockdiag(blocks):
    k = sum(b.shape[0] for b in blocks)
    m = sum(b.shape[1] for b in blocks)
    out = np.zeros((k, m), np.float32)
    i = j = 0
    for b in blocks:
        out[i:i + b.shape[0], j:j + b.shape[1]] = b
        i += b.shape[0]
        j += b.shape[1]
    return out


def _pack_weights(inp):
    wp = np.zeros((128, NSLOT * 128), np.float32)

    def put(slot, mat):
        wp[:mat.shape[0], slot * 128: slot * 128 + mat.shape[1]] = mat

    w1 = inp['w1'].astype(np.float32)  # [32, 29, 3]
    b1 = inp['b1'].astype(np.float32)
    # slots 0-2: g1 taps: blockdiag4 of [30, 32]: 29 in-ch rows + bias row
    for k in range(3):
        blk = np.zeros((30, 32), np.float32)
        blk[:29] = w1[:, :, k].T
        if k == 1:
            blk[29] = b1
        put(k, _blockdiag([blk] * 4))
    w2 = inp['w2'].astype(np.float32)
    for k in range(3):  # slots 3-5
        put(3 + k, _blockdiag([w2[:, :, k].T.astype(np.float32)] * 4))
    w3 = inp['w3'].astype(np.float32)
    w3t = [_blockdiag([w3[:, :, k].T.astype(np.float32)] * 2) for k in range(3)]
    # slot 6: G3 merged tap: [G2p ; G2p<<8] -> U0 on rows 0:64, U2 on 64:128
    put(6, np.concatenate([w3t[0], w3t[2]], axis=0))
    # slot 7: G3 center tap U1 on G3d rows 0:64
    put(7, w3t[1])
    # slot 8: E3 merged tap: [E2p ; G2p<<4] -> U1 rows 0:64, U2 rows 64:128
    put(8, np.concatenate([w3t[1], w3t[2]], axis=0))
    w4 = inp['w4'].astype(np.float32)
    for j in (1, 2):    # slots 9-10: [128, 128]
        put(9 + j - 1, _blockdiag([w4[:, :, j].T.astype(np.float32)] * 2))
    fw1t = inp['fw1'].T.astype(np.float32)
    put(11, np.concatenate([fw1t, fw1t], axis=0))               # [64,128] x2 rows
    # fc2 / fc3 as M=128 with zero column-halves: psum accumulation composes
    # the two chunk-halves onto partitions 0:64 / 64:128 without col-tiling.
    fw2t = inp['fw2'].T.astype(np.float32)          # [128, 64]
    z64 = np.zeros_like(fw2t)
    put(12, np.concatenate([fw2t, z64], axis=1))    # fc2_lo [128, 128]
    put(13, np.concatenate([z64, fw2t], axis=1))    # fc2_hi
    fw3t = _blockdiag([inp['fw3'].T.astype(np.float32)] * 2)       # [128, 64]
    z64b = np.zeros_like(fw3t)
    put(14, np.concatenate([fw3t, z64b], axis=1))   # fc3_lo
    put(15, np.concatenate([z64b, fw3t], axis=1))   # fc3_hi
    put(16, _blockdiag([inp['fcw'].T.astype(np.float32)] * 4))     # [128, 64]

    wb = np.zeros((128, 8), np.float32)
    wb[:, 0] = np.tile(inp['b2'], 4)
    wb[:, 1] = np.tile(inp['b3'], 2)
    wb[:, 2] = np.tile(inp['b4'], 2)
    wb[:, 3] = inp['fb1']
    wb[:, 4] = np.tile(inp['fb2'], 2)
    wb[:, 5] = np.tile(inp['fb3'], 4)
    return wp.astype(BF), wb


def _split(lo, hi, step=512):
    return [(a, min(a + step, hi)) for a in range(lo, hi, step)]


def _build_program(reps=1):
    import concourse.bacc as bacc
    import concourse.mybir as mybir
    import concourse.tile as tile

    F32 = mybir.dt.float32
    BF16 = mybir.dt.bfloat16

    nc = bacc.Bacc("TRN2", target_bir_lowering=False, debug=False)
    x_d = nc.dram_tensor("x", [BPC, 4, C + 1, Tc + 22], BF16, kind="ExternalInput").ap()
    w_d = nc.dram_tensor("wpack", [128, NSLOT * 128], BF16, kind="ExternalInput").ap()
    wb_d = nc.dram_tensor("wbias", [128, 8], F32, kind="ExternalInput").ap()
    o_d = nc.dram_tensor("out", [BPC, 4, 16, Tc], BF16, kind="ExternalOutput").ap()

    with tile.TileContext(nc) as tc:
        with tc.tile_pool(name="wp", bufs=1) as wpool, \
             tc.tile_pool(name="xp", bufs=1) as xpool, \
             tc.tile_pool(name="yp", bufs=1) as ypool, \
             tc.tile_pool(name="st", bufs=ST_BUFS) as spool, \
             tc.tile_pool(name="psa", bufs=PS_BUFS, space="PSUM") as ppool_a:

            wsb = wpool.tile([128, NSLOT * 128], BF16, tag="w")
            wbb = wpool.tile([128, 8], F32, tag="wb")

            for _rep in range(reps):
                _emit_body(nc, tc, mybir, F32, BF16, wsb, wbb,
                           xpool, ypool, spool, (ppool_a, ppool_a), x_d, o_d,
                           w_d, wb_d)
    nc.finalize()
    return nc


def _emit_body(nc, tc, mybir, F32, BF16, wsb, wbb,
               xpool, ypool, spool, ppool, x_d, o_d):
    AF = mybir.ActivationFunctionType
    OP = mybir.AluOpType

    def lhsT(slot, k=128, m=128, base=0):
        return wsb[base:base + k, slot * 128: slot * 128 + m]

    def bias(i):
        return wbb[:, i: i + 1]

    # ---------------- input load: per batch [120p = 4 x (29ch + ones), W]
    # host pre-pads x to [4, 30, Tc+22] (chunk-major so one DMA fills all 120
    # partitions); each load covers tile cols [6, Tc+28) in 4 column pieces.
    X = [None] * BPC
    for b in range(BPC):
        X[b] = xpool.tile([120, W], BF16, tag="x", name=f"x{b}")
    XP = [(0, 800), (800, 1500), (1500, Tc + 22)]
    nc.sync.dma_start(out=wsb[:, 0:384], in_=w_d[:, 0:384])
    for (s0, s1) in XP:
        nc.sync.dma_start(out=X[0][0:120, 6 + s0: 6 + s1],
                          in_=x_d[0, :, :, s0: s1])
    nc.sync.dma_start(out=wbb[:], in_=wb_d[:])
    nc.sync.dma_start(out=wsb[:, 384:], in_=w_d[:, 384:])
    for (s0, s1) in XP:
        nc.sync.dma_start(out=X[1][0:120, 6 + s0: 6 + s1],
                          in_=x_d[1, :, :, s0: s1])

    Y = ypool.tile([64, BPC * Tc], BF16, tag="y")

    CB = 1037  # half boundary (tile col)

    def make_layers(b):
        ST = lambda nm: spool.tile([128, W], BF16, tag="st", name=f"{nm}_{b}")  # noqa: E731

        def conv_pass(out_tile, rng, taps, evac, pool, half):
            """taps: list of (lhsT_ap, rhs_tile, rp0, rp1, delta)."""
            lo = rng[0] if half == 0 else CB
            hi = CB if half == 0 else rng[1]
            for (glo, ghi) in _split(lo, hi, PS_GROUP):
                gn = ghi - glo
                ps = pool.tile([128, PS_GROUP], F32, tag="ps", name="ps")
                for (lo, hi) in _split(glo, ghi, 512):
                    n, off = hi - lo, lo - glo
                    for i, (lw, rt, rp0, rp1, d) in enumerate(taps):
                        nc.tensor.matmul(
                            ps[0:lw.shape[-1], off:off + n], lw,
                            rt[rp0:rp1, lo + d: hi + d],
                            start=(i == 0), stop=(i == len(taps) - 1))
                evac(ps[:, 0:gn], out_tile[:, glo:ghi])

        def pool_for(name):
            return ppool_a if ROUTE.get(name, 'act') == 'act' else ppool_d

        def psg_for(name):
            return PSA_GROUP if ROUTE.get(name, 'act') == 'act' else PS_GROUP

        def evac_for(name, alpha, bias_i, half=0):
            route = ROUTE.get(name, 'act')
            # tail: the last job's DVE-routed drains serialize behind the
            # window-sum/clip chain while Act idles; shift them to Act
            if TAILACT and b == BPC - 1 and half == len(CBS) and route in ('dve2',):
                route = 'act'

            def act(ps, ot):
                nc.scalar.activation(ot, ps, AF.Prelu,
                                     bias=bias(bias_i) if bias_i is not None else 0.0,
                                     scale=1.0, alpha=alpha)

            def dve1(ps, ot):
                # one PSUM read max per instruction: copy out, then leaky in SBUF
                nc.vector.tensor_scalar(ot, ps, 0.0, None, OP.add)
                nc.vector.scalar_tensor_tensor(ot, ot, alpha, ot, OP.mult, OP.max)

            def dve2(ps, ot):
                nc.vector.tensor_scalar(ot, ps, bias(bias_i), None, OP.add)
                nc.vector.scalar_tensor_tensor(ot, ot, alpha, ot, OP.mult, OP.max)

            def dver(ps, ot):
                nc.vector.tensor_scalar(
                    ot, ps, bias(bias_i) if bias_i is not None else 0.0, 0.0,
                    OP.add, OP.max)

            return {'act': act, 'dve1': dve1, 'dve2': dve2, 'dver': dver}[route]

        tiles = {}

        def getst(nm):
            if nm not in tiles:
                tiles[nm] = ST(nm)
            return tiles[nm]

        def l0(half):  # G1 + E1
            G1 = getst("G1")
            conv_pass(G1, (14, Tc + 34),
                      [(lhsT(k, 120), X[b], 0, 120, k - 8) for k in range(3)],
                      evac_for('G1', 0.02, None, half), pool_for('G1'), half, psg_for('G1'))
            E1 = getst("E1")
            conv_pass(E1, (13, Tc + 21),
                      [(lhsT(k, 120), X[b], 0, 120, k - 8) for k in (1, 2)],
                      evac_for('E1', 0.02, None, half), pool_for('E1'), half, psg_for('E1'))

        def l1(half):  # G2 + E2
            G1, E1 = tiles['G1'], tiles['E1']
            G2 = getst("G2")
            conv_pass(G2, (17, Tc + 33),
                      [(lhsT(3 + k), G1, 0, 128, 2 * (k - 1)) for k in range(3)],
                      evac_for('G2', 0.02, 0, half), pool_for('G2'), half, psg_for('G2'))
            E2 = getst("E2")
            conv_pass(E2, (13, Tc + 21),
                      [(lhsT(4), E1, 0, 128, 0), (lhsT(5), G1, 0, 128, 2)],
                      evac_for('E2', 0.02, 0, half), pool_for('E2'), half, psg_for('E2'))

        def l2(half):  # assemble [stream ; shifted-copy] tiles
            G2, E2 = tiles['G2'], tiles['E2']
            G3d = [getst("G3d0"), getst("G3d1")]
            EG3d = [getst("EG3d0"), getst("EG3d1")]
            tiles['G3d'], tiles['EG3d'] = G3d, EG3d
            for p in range(2):
                (a0, a1) = (17, CB) if half == 0 else (CB, Tc + 29)
                nc.gpsimd.dma_start(out=G3d[p][0:64, a0: a1],
                                  in_=G2[64 * p:64 * p + 64, a0: a1])
                (a0, a1) = (17, CB) if half == 0 else (CB, Tc + 25)
                nc.gpsimd.dma_start(out=G3d[p][64:128, a0: a1],
                                  in_=G2[64 * p:64 * p + 64, a0 + 8: a1 + 8])
                (a0, a1) = (13, CB) if half == 0 else (CB, Tc + 21)
                nc.sync.dma_start(out=EG3d[p][0:64, a0: a1],
                                  in_=E2[64 * p:64 * p + 64, a0: a1])
                nc.sync.dma_start(out=EG3d[p][64:128, a0: a1],
                                  in_=G2[64 * p:64 * p + 64, a0 + 4: a1 + 4])

        def l3(half):  # G3 + E3
            G3d, EG3d = tiles['G3d'], tiles['EG3d']
            tiles['G3'] = G3 = [getst("G3a"), getst("G3b")]
            for p in range(2):
                conv_pass(G3[p], (21, Tc + 29),
                          [(lhsT(6), G3d[p], 0, 128, -4),
                           (lhsT(7, 64), G3d[p], 0, 64, 0)],
                          evac_for(f'G3_{p}', 0.2, 1, half), pool_for(f'G3_{p}'), half, psg_for(f'G3_{p}'))
            tiles['E3'] = E3 = [getst("E3a"), getst("E3b")]
            for p in range(2):
                conv_pass(E3[p], (13, Tc + 21),
                          [(lhsT(8), EG3d[p], 0, 128, 0)],
                          evac_for('E3', 0.2, 1, half), pool_for('E3'), half, psg_for('E3'))

        def l4(half):  # H
            G3, E3 = tiles['G3'], tiles['E3']
            tiles['H'] = H = [getst("Ha"), getst("Hb")]
            for p in range(2):
                conv_pass(H[p], (13, Tc + 21),
                          [(lhsT(9), E3[p], 0, 128, 0),
                           (lhsT(10), G3[p], 0, 128, 8)],
                          evac_for('H', 0.2, 2, half), pool_for('H'), half, psg_for('H'))

        def l5(half):  # H1 (fc1)
            H = tiles['H']
            tiles['H1'] = H1 = [getst("H1" + str(c)) for c in range(4)]
            for cidx in range(4):
                p, ph = cidx // 2, cidx % 2
                conv_pass(H1[cidx], (13, Tc + 21),
                          [(lhsT(11, 64, base=64 * ph), H[p],
                            64 * ph, 64 * ph + 64, 0)],
                          evac_for(f'H1_{cidx}', 0.02, 3, half), pool_for(f'H1_{cidx}'), half, psg_for(f'H1_{cidx}'))

        def l6(half):  # A2 (fc2)
            H1 = tiles['H1']
            tiles['A2'] = A2 = [getst("A2a"), getst("A2b")]
            for p in range(2):
                conv_pass(A2[p], (13, Tc + 21),
                          [(lhsT(12), H1[2 * p], 0, 128, 0),
                           (lhsT(13), H1[2 * p + 1], 0, 128, 0)],
                          evac_for('A2', 0.02, 4, half), pool_for('A2'), half, psg_for('A2'))

        def l7(half):  # FFC (fc3 + tanh)
            A2 = tiles['A2']
            tiles['FFC'] = FFC = getst("FFC")

            def tanh_evac(ps, ot):
                nc.scalar.activation(ot, ps, AF.Tanh, bias=bias(5), scale=1.0)

            conv_pass(FFC, (13, Tc + 21),
                      [(lhsT(14), A2[0], 0, 128, 0),
                       (lhsT(15), A2[1], 0, 128, 0)],
                      tanh_evac, ppool_a, half, PSA_GROUP)
            # phantom edge values must read as zero in the window sum
            if half == 0:
                nc.gpsimd.memset(FFC[0:32, 13:16], 0.0)
            else:
                nc.gpsimd.memset(FFC[96:128, Tc + 16: Tc + 21], 0.0)

        def l8(half):  # window-sum tree (8-wide): piece-split for pipelining
            # half 0 owns cols [.., CB), half 1 [CB, ..); ops at the boundary
            # read a few columns across it (producer half finished earlier).
            FFC = tiles['FFC']
            S1 = getst("S1")
            tiles['S1'] = S1
            SSTEP = 512
            # Pool is slow (no 16-bit speedup); the final job's sums sit on
            # the critical tail, so run those on DVE instead.
            eng = nc.vector
            # staggered split points: each op's half-0 range ends before the
            # columns that would read the NEXT op stage across the boundary
            for (p0, p1) in _split(13 if half == 0 else CB + 3,
                                   CB + 3 if half == 0 else Tc + 19, SSTEP):
                eng.tensor_tensor(S1[:, p0: p1], FFC[:, p0: p1],
                                  FFC[:, p0 + 1: p1 + 1], OP.add)
            for (p0, p1) in _split(13 if half == 0 else CB + 1,
                                   CB + 1 if half == 0 else Tc + 17, SSTEP):
                eng.tensor_tensor(FFC[:, p0: p1], S1[:, p0: p1],
                                  S1[:, p0 + 2: p1 + 2], OP.add)
            for (p0, p1) in _split(16 if half == 0 else CB,
                                   CB if half == 0 else Tc + 16, SSTEP):
                eng.tensor_tensor(S1[:, p0: p1], FFC[:, p0 - 3: p1 - 3],
                                  FFC[:, p0 + 1: p1 + 1], OP.add)

        def l9(half):  # final fc + clip + output DMA
            S1 = tiles['S1']
            for (glo, ghi) in _split(16 if half == 0 else CB,
                                     CB if half == 0 else Tc + 16, PS_GROUP):
                ps = ppool_d.tile([128, PS_GROUP], F32, tag=f"ps{PS_GROUP}", name="ps",
                                  bufs=PS_BUFS)
                for (lo, hi) in _split(glo, ghi, 512):
                    n, off = hi - lo, lo - glo
                    nc.tensor.matmul(ps[0:64, off:off + n], lhsT(16, 128, 64),
                                     S1[:, lo: hi], start=True, stop=True)
                    nc.vector.tensor_scalar(
                        Y[:, b * Tc + lo - 16: b * Tc + hi - 16],
                        ps[0:64, off:off + n], 0.0, 1.0, OP.max, OP.min)
            (h0, h1) = (0, CB - 16) if half == 0 else (CB - 16, Tc)
            nc.sync.dma_start(out=o_d[b, :, :, h0:h1],
                              in_=Y[0:64, b * Tc + h0: b * Tc + h1])

        return [l0, l1, l2, l3, l4, l5, l6, l7, l8, l9]

    batch_layers = [make_layers(b) for b in range(BPC)]
    jobs = [(0, 0), (0, 1), (1, 0), (1, 1)]
    L = len(batch_layers[0])
    for k in range(L + SKEW * (len(jobs) - 1)):
        # emit deeper-pipelined jobs first: a half-0 layer reads a few
        # boundary columns from the next job's previous layer, which must
        # appear earlier in program order for the dependency to register
        for j, (b, h) in reversed(list(enumerate(jobs))):
            kk = k - SKEW * j
            if 0 <= kk < L:
                batch_layers[b][kk](h)


def _get_program(reps=1):
    global _PROG
    if _PROG is None:
        _PROG = {}
    if reps not in _PROG:
        _PROG[reps] = _build_program(reps)
    return _PROG[reps]


def _prepare_inputs(inputs):
    x = np.asarray(inputs['speech_features'], np.float32)
    xp = np.zeros((B, C + 1, T + 22), np.float32)
    xp[:, :C, 10:10 + T] = x
    xp[:, C, :] = 1.0
    # chunk-major: [B, 4, 30, Tc+22]; chunk c covers padded cols [c*Tc, c*Tc+Tc+22)
    xa = np.zeros((B, 4, C + 1, Tc + 22), np.float32)
    for c in range(4):
        xa[:, c] = xp[:, :, c * Tc: c * Tc + Tc + 22]
    xa = xa.astype(BF)
    wp, wb = _pack_weights({k: np.asarray(v, np.float32)
                            for k, v in inputs.items() if k != 'speech_features'})
    return [{"x": xa[i * BPC:(i + 1) * BPC], "wpack": wp, "wbias": wb}
            for i in range(NCORES)]


def kernel(**inputs):
    from concourse.bass_utils import run_bass_kernel_spmd

    in_maps = _prepare_inputs(inputs)
    nc = _get_program()
    res = run_bass_kernel_spmd(nc, in_maps, core_ids=list(range(NCORES)))
    outs = []
    for r in res.results:
        o = np.asarray(r["out"]).astype(np.float32)      # [BPC, 4, 16, Tc]
        o = o.transpose(0, 1, 3, 2).reshape(BPC, T, 16)  # [BPC, T, 16]
        outs.append(o)
    return np.ascontiguousarray(np.concatenate(outs, axis=0))


# revision 47
# speedup vs baseline: 1.0289x; 1.0002x over previous
"""Trainium2 Bass kernel for nn_CNNGenerator (frame CNN + FC + window-sum + FC).

Key algebraic facts exploited (validated vs the reference):
  * softmax over a size-1 axis == 1.0, so the whole attention_conv stack is
    dead code; the bmm reduces to an 8-wide sliding-window sum of ffc.
  * The per-window stride-2 conv stack collapses into global conv streams:
    an "interior" stream g{1,2,3} and a "left-edge" stream e{1,2,3} per
    layer, plus a 2-tap combine (z).  Per t:
      g1[s] = b1 + sum_k W1k x[s+k-8]          e1[t] = b1 + W11 x[t-7] + W12 x[t-6]
      g2[s] = b2 + V0 G1[s-2] + V1 G1[s] + V2 G1[s+2]
      e2[t] = b2 + V1 E1[t] + V2 G1[t+2]
      g3[s] = b3 + U0 G2[s-4] + U1 G2[s] + U2 G2[s+4]
      e3[t] = b3 + U1 E2[t] + U2 G2[t+4]
      z[t]  = b4 + T1 E3[t] + T2 G3[t+8]
    (capitals = leaky-activated streams), then fc1/fc2/fc3+tanh,
    ws[t] = sum_{d=-3..4} ffc[t+d], out = clip(fcw @ ws, 0, 1).

Sharding: pure data parallel, 2 batch elements per core on 8 cores.
On-chip layout: time axis split in 4 chunks of 2048; 32-channel streams pack
4 chunks x 32ch on the 128 partitions, 64-channel streams pack 2 chunks x 64ch
(two tiles).  All matmul operands are bf16 (fp32 PSUM accumulate); evacuation
work is split between the Activation engine (Prelu/Tanh) and DVE (one-op
leaky via scalar_tensor_tensor).  G3 runs as 2 matmuls (not 3) and E3 as 1
(not 2) against DMA-assembled tiles that stack a stream with a column-shifted
copy on the partition axis, doubling effective contraction per column.
"""
import sys

if '/opt/trn_rl_repo' not in sys.path:
    sys.path.insert(0, '/opt/trn_rl_repo')

import numpy as np
import ml_dtypes

BF = ml_dtypes.bfloat16

B, C, T = 16, 29, 8192
NCORES = 8
BPC = B // NCORES          # batch per core
Tc = T // 4                # time chunk
HL = 16                    # left halo: tile col u <-> global idx c*Tc + u - HL
W = Tc + 40                # per-batch stream tile width
NSLOT = 17                 # 128-col lhsT slots in the weight pack

_PROG = {}
PS_GROUP = 1024
PS_BUFS = 4
ST_BUFS = 22

# evacuation routing: which streams drain on DVE instead of Activation.
# 'dve1' = one-op leaky (psum already contains bias), 'dve2' = bias-add +
# leaky (two DVE ops), 'act' = Activation Prelu.
ROUTE = {
    'G1': 'act', 'E1': 'act',
    'G2': 'act', 'E2': 'dve2',
    'G3_0': 'dve2', 'G3_1': 'act', 'E3': 'act', 'H': 'act',
    'H1_0': 'dve2', 'H1_1': 'act', 'H1_2': 'dve2', 'H1_3': 'act',
    'A2': 'act', 'FFC': 'act',
}
SKEW = 3


def _blockdiag(blocks):
    k = sum(b.shape[0] for b in blocks)
    m = sum(b.shape[1] for b in blocks)
    out = np.zeros((k, m), np.float32)
    i = j = 0
    for b in blocks:
        out[i:i + b.shape[0], j:j + b.shape[1]] = b
        i += b.shape[0]
        j += b.shape[1]
    return out


def _pack_weights(inp):
    wp = np.zeros((128, NSLOT * 128), np.float32)

    def put(slot, mat):
        wp[:mat.shape[0], slot * 128: slot * 128 + mat.shape[1]] = mat

    w1 = inp['w1'].astype(np.float32)  # [32, 29, 3]
    b1 = inp['b1'].astype(np.float32)
    # slots 0-2: g1 taps: blockdiag4 of [30, 32]: 29 in-ch rows + bias row
    for k in range(3):
        blk = np.zeros((30, 32), np.float32)
        blk[:29] = w1[:, :, k].T
        if k == 1:
            blk[29] = b1
        put(k, _blockdiag([blk] * 4))
    w2 = inp['w2'].astype(np.float32)
    for k in range(3):  # slots 3-5
        put(3 + k, _blockdiag([w2[:, :, k].T.astype(np.float32)] * 4))
    w3 = inp['w3'].astype(np.float32)
    w3t = [_blockdiag([w3[:, :, k].T.astype(np.float32)] * 2) for k in range(3)]
    # slot 6: G3 merged tap: [G2p ; G2p<<8] -> U0 on rows 0:64, U2 on 64:128
    put(6, np.concatenate([w3t[0], w3t[2]], axis=0))
    # slot 7: G3 center tap U1 on G3d rows 0:64
    put(7, w3t[1])
    # slot 8: E3 merged tap: [E2p ; G2p<<4] -> U1 rows 0:64, U2 rows 64:128
    put(8, np.concatenate([w3t[1], w3t[2]], axis=0))
    w4 = inp['w4'].astype(np.float32)
    for j in (1, 2):    # slots 9-10: [128, 128]
        put(9 + j - 1, _blockdiag([w4[:, :, j].T.astype(np.float32)] * 2))
    fw1t = inp['fw1'].T.astype(np.float32)
    put(11, np.concatenate([fw1t, fw1t], axis=0))               # [64,128] x2 rows
    # fc2 / fc3 as M=128 with zero column-halves: psum accumulation composes
    # the two chunk-halves onto partitions 0:64 / 64:128 without col-tiling.
    fw2t = inp['fw2'].T.astype(np.float32)          # [128, 64]
    z64 = np.zeros_like(fw2t)
    put(12, np.concatenate([fw2t, z64], axis=1))    # fc2_lo [128, 128]
    put(13, np.concatenate([z64, fw2t], axis=1))    # fc2_hi
    fw3t = _blockdiag([inp['fw3'].T.astype(np.float32)] * 2)       # [128, 64]
    z64b = np.zeros_like(fw3t)
    put(14, np.concatenate([fw3t, z64b], axis=1))   # fc3_lo
    put(15, np.concatenate([z64b, fw3t], axis=1))   # fc3_hi
    put(16, _blockdiag([inp['fcw'].T.astype(np.float32)] * 4))     # [128, 64]

    wb = np.zeros((128, 8), np.float32)
    wb[:, 0] = np.tile(inp['b2'], 4)
    wb[:, 1] = np.tile(inp['b3'], 2)
    wb[:, 2] = np.tile(inp['b4'], 2)
    wb[:, 3] = inp['fb1']
    wb[:, 4] = np.tile(inp['fb2'], 2)
    wb[:, 5] = np.tile(inp['fb3'], 4)
    return wp.astype(BF), wb


def _split(lo, hi, step=512):
    return [(a, min(a + step, hi)) for a in range(lo, hi, step)]


def _build_program(reps=1):
    import concourse.bacc as bacc
    import concourse.mybir as mybir
    import concourse.tile as tile

    F32 = mybir.dt.float32
    BF16 = mybir.dt.bfloat16

    nc = bacc.Bacc("TRN2", target_bir_lowering=False, debug=False)
    x_d = nc.dram_tensor("x", [BPC, 4, C + 1, Tc + 22], BF16, kind="ExternalInput").ap()
    w_d = nc.dram_tensor("wpack", [128, NSLOT * 128], BF16, kind="ExternalInput").ap()
    wb_d = nc.dram_tensor("wbias", [128, 8], F32, kind="ExternalInput").ap()
    o_d = nc.dram_tensor("out", [BPC, 4, 16, Tc], BF16, kind="ExternalOutput").ap()

    with tile.TileContext(nc) as tc:
        with tc.tile_pool(name="wp", bufs=1) as wpool, \
             tc.tile_pool(name="xp", bufs=1) as xpool, \
             tc.tile_pool(name="yp", bufs=1) as ypool, \
             tc.tile_pool(name="st", bufs=ST_BUFS) as spool, \
             tc.tile_pool(name="psa", bufs=PS_BUFS, space="PSUM") as ppool_a:

            wsb = wpool.tile([128, NSLOT * 128], BF16, tag="w")
            wbb = wpool.tile([128, 8], F32, tag="wb")

            for _rep in range(reps):
                _emit_body(nc, tc, mybir, F32, BF16, wsb, wbb,
                           xpool, ypool, spool, (ppool_a, ppool_a), x_d, o_d,
                           w_d, wb_d)
    nc.finalize()
    return nc


def _emit_body(nc, tc, mybir, F32, BF16, wsb, wbb,
               xpool, ypool, spool, ppool, x_d, o_d):
    AF = mybir.ActivationFunctionType
    OP = mybir.AluOpType

    def lhsT(slot, k=128, m=128, base=0):
        return wsb[base:base + k, slot * 128: slot * 128 + m]

    def bias(i):
        return wbb[:, i: i + 1]

    # ---------------- input load: per batch [120p = 4 x (29ch + ones), W]
    # host pre-pads x to [4, 30, Tc+22] (chunk-major so one DMA fills all 120
    # partitions); each load covers tile cols [6, Tc+28) in 4 column pieces.
    X = [None] * BPC
    for b in range(BPC):
        X[b] = xpool.tile([120, W], BF16, tag="x", name=f"x{b}")
    XP = [(0, 800), (800, 1500), (1500, Tc + 22)]
    nc.sync.dma_start(out=wsb[:, 0:768], in_=w_d[:, 0:768])
    for (s0, s1) in XP:
        nc.sync.dma_start(out=X[0][0:120, 6 + s0: 6 + s1],
                          in_=x_d[0, :, :, s0: s1])
    nc.sync.dma_start(out=wbb[:], in_=wb_d[:])
    nc.sync.dma_start(out=wsb[:, 768:], in_=w_d[:, 768:])
    for (s0, s1) in XP:
        nc.sync.dma_start(out=X[1][0:120, 6 + s0: 6 + s1],
                          in_=x_d[1, :, :, s0: s1])

    Y = ypool.tile([64, BPC * Tc], BF16, tag="y")

    CB = 1037  # half boundary (tile col)

    def make_layers(b):
        ST = lambda nm: spool.tile([128, W], BF16, tag="st", name=f"{nm}_{b}")  # noqa: E731

        def conv_pass(out_tile, rng, taps, evac, pool, half):
            """taps: list of (lhsT_ap, rhs_tile, rp0, rp1, delta)."""
            lo = rng[0] if half == 0 else CB
            hi = CB if half == 0 else rng[1]
            for (glo, ghi) in _split(lo, hi, PS_GROUP):
                gn = ghi - glo
                ps = pool.tile([128, PS_GROUP], F32, tag="ps", name="ps")
                for (lo, hi) in _split(glo, ghi, 512):
                    n, off = hi - lo, lo - glo
                    for i, (lw, rt, rp0, rp1, d) in enumerate(taps):
                        nc.tensor.matmul(
                            ps[0:lw.shape[-1], off:off + n], lw,
                            rt[rp0:rp1, lo + d: hi + d],
                            start=(i == 0), stop=(i == len(taps) - 1))
                evac(ps[:, 0:gn], out_tile[:, glo:ghi])

        def pool_for(name):
            return ppool_a if ROUTE.get(name, 'act') == 'act' else ppool_d

        def psg_for(name):
            return PSA_GROUP if ROUTE.get(name, 'act') == 'act' else PS_GROUP

        def evac_for(name, alpha, bias_i, half=0):
            route = ROUTE.get(name, 'act')
            # tail: the last job's DVE-routed drains serialize behind the
            # window-sum/clip chain while Act idles; shift them to Act
            if TAILACT and b == BPC - 1 and half == len(CBS) and route in ('dve2',):
                route = 'act'

            def act(ps, ot):
                nc.scalar.activation(ot, ps, AF.Prelu,
                                     bias=bias(bias_i) if bias_i is not None else 0.0,
                                     scale=1.0, alpha=alpha)

            def dve1(ps, ot):
                # one PSUM read max per instruction: copy out, then leaky in SBUF
                nc.vector.tensor_scalar(ot, ps, 0.0, None, OP.add)
                nc.vector.scalar_tensor_tensor(ot, ot, alpha, ot, OP.mult, OP.max)

            def dve2(ps, ot):
                nc.vector.tensor_scalar(ot, ps, bias(bias_i), None, OP.add)
                nc.vector.scalar_tensor_tensor(ot, ot, alpha, ot, OP.mult, OP.max)

            def dver(ps, ot):
                nc.vector.tensor_scalar(
                    ot, ps, bias(bias_i) if bias_i is not None else 0.0, 0.0,
                    OP.add, OP.max)

            return {'act': act, 'dve1': dve1, 'dve2': dve2, 'dver': dver}[route]

        tiles = {}

        def getst(nm):
            if nm not in tiles:
                tiles[nm] = ST(nm)
            return tiles[nm]

        def l0(half):  # G1 + E1
            G1 = getst("G1")
            conv_pass(G1, (14, Tc + 34),
                      [(lhsT(k, 120), X[b], 0, 120, k - 8) for k in range(3)],
                      evac_for('G1', 0.02, None, half), pool_for('G1'), half, psg_for('G1'))
            E1 = getst("E1")
            conv_pass(E1, (13, Tc + 21),
                      [(lhsT(k, 120), X[b], 0, 120, k - 8) for k in (1, 2)],
                      evac_for('E1', 0.02, None, half), pool_for('E1'), half, psg_for('E1'))

        def l1(half):  # G2 + E2
            G1, E1 = tiles['G1'], tiles['E1']
            G2 = getst("G2")
            conv_pass(G2, (17, Tc + 33),
                      [(lhsT(3 + k), G1, 0, 128, 2 * (k - 1)) for k in range(3)],
                      evac_for('G2', 0.02, 0, half), pool_for('G2'), half, psg_for('G2'))
            E2 = getst("E2")
            conv_pass(E2, (13, Tc + 21),
                      [(lhsT(4), E1, 0, 128, 0), (lhsT(5), G1, 0, 128, 2)],
                      evac_for('E2', 0.02, 0, half), pool_for('E2'), half, psg_for('E2'))

        def l2(half):  # assemble [stream ; shifted-copy] tiles
            G2, E2 = tiles['G2'], tiles['E2']
            G3d = [getst("G3d0"), getst("G3d1")]
            EG3d = [getst("EG3d0"), getst("EG3d1")]
            tiles['G3d'], tiles['EG3d'] = G3d, EG3d
            for p in range(2):
                (a0, a1) = (17, CB) if half == 0 else (CB, Tc + 29)
                nc.gpsimd.dma_start(out=G3d[p][0:64, a0: a1],
                                  in_=G2[64 * p:64 * p + 64, a0: a1])
                (a0, a1) = (17, CB) if half == 0 else (CB, Tc + 25)
                nc.gpsimd.dma_start(out=G3d[p][64:128, a0: a1],
                                  in_=G2[64 * p:64 * p + 64, a0 + 8: a1 + 8])
                (a0, a1) = (13, CB) if half == 0 else (CB, Tc + 21)
                nc.sync.dma_start(out=EG3d[p][0:64, a0: a1],
                                  in_=E2[64 * p:64 * p + 64, a0: a1])
                nc.sync.dma_start(out=EG3d[p][64:128, a0: a1],
                                  in_=G2[64 * p:64 * p + 64, a0 + 4: a1 + 4])

        def l3(half):  # G3 + E3
            G3d, EG3d = tiles['G3d'], tiles['EG3d']
            tiles['G3'] = G3 = [getst("G3a"), getst("G3b")]
            for p in range(2):
                conv_pass(G3[p], (21, Tc + 29),
                          [(lhsT(6), G3d[p], 0, 128, -4),
                           (lhsT(7, 64), G3d[p], 0, 64, 0)],
                          evac_for(f'G3_{p}', 0.2, 1, half), pool_for(f'G3_{p}'), half, psg_for(f'G3_{p}'))
            tiles['E3'] = E3 = [getst("E3a"), getst("E3b")]
            for p in range(2):
                conv_pass(E3[p], (13, Tc + 21),
                          [(lhsT(8), EG3d[p], 0, 128, 0)],
                          evac_for('E3', 0.2, 1, half), pool_for('E3'), half, psg_for('E3'))

        def l4(half):  # H
            G3, E3 = tiles['G3'], tiles['E3']
            tiles['H'] = H = [getst("Ha"), getst("Hb")]
            for p in range(2):
                conv_pass(H[p], (13, Tc + 21),
                          [(lhsT(9), E3[p], 0, 128, 0),
                           (lhsT(10), G3[p], 0, 128, 8)],
                          evac_for('H', 0.2, 2, half), pool_for('H'), half, psg_for('H'))

        def l5(half):  # H1 (fc1)
            H = tiles['H']
            tiles['H1'] = H1 = [getst("H1" + str(c)) for c in range(4)]
            for cidx in range(4):
                p, ph = cidx // 2, cidx % 2
                conv_pass(H1[cidx], (13, Tc + 21),
                          [(lhsT(11, 64, base=64 * ph), H[p],
                            64 * ph, 64 * ph + 64, 0)],
                          evac_for(f'H1_{cidx}', 0.02, 3, half), pool_for(f'H1_{cidx}'), half, psg_for(f'H1_{cidx}'))

        def l6(half):  # A2 (fc2)
            H1 = tiles['H1']
            tiles['A2'] = A2 = [getst("A2a"), getst("A2b")]
            for p in range(2):
                conv_pass(A2[p], (13, Tc + 21),
                          [(lhsT(12), H1[2 * p], 0, 128, 0),
                           (lhsT(13), H1[2 * p + 1], 0, 128, 0)],
                          evac_for('A2', 0.02, 4, half), pool_for('A2'), half, psg_for('A2'))

        def l7(half):  # FFC (fc3 + tanh)
            A2 = tiles['A2']
            tiles['FFC'] = FFC = getst("FFC")

            def tanh_evac(ps, ot):
                nc.scalar.activation(ot, ps, AF.Tanh, bias=bias(5), scale=1.0)

            conv_pass(FFC, (13, Tc + 21),
                      [(lhsT(14), A2[0], 0, 128, 0),
                       (lhsT(15), A2[1], 0, 128, 0)],
                      tanh_evac, ppool_a, half, PSA_GROUP)
            # phantom edge values must read as zero in the window sum
            if half == 0:
                nc.gpsimd.memset(FFC[0:32, 13:16], 0.0)
            else:
                nc.gpsimd.memset(FFC[96:128, Tc + 16: Tc + 21], 0.0)

        def l8(half):  # window-sum tree (8-wide): piece-split for pipelining
            # half 0 owns cols [.., CB), half 1 [CB, ..); ops at the boundary
            # read a few columns across it (producer half finished earlier).
            FFC = tiles['FFC']
            S1 = getst("S1")
            tiles['S1'] = S1
            SSTEP = 512
            # Pool is slow (no 16-bit speedup); the final job's sums sit on
            # the critical tail, so run those on DVE instead.
            eng = nc.vector
            # staggered split points: each op's half-0 range ends before the
            # columns that would read the NEXT op stage across the boundary
            for (p0, p1) in _split(13 if half == 0 else CB + 3,
                                   CB + 3 if half == 0 else Tc + 19, SSTEP):
                eng.tensor_tensor(S1[:, p0: p1], FFC[:, p0: p1],
                                  FFC[:, p0 + 1: p1 + 1], OP.add)
            for (p0, p1) in _split(13 if half == 0 else CB + 1,
                                   CB + 1 if half == 0 else Tc + 17, SSTEP):
                eng.tensor_tensor(FFC[:, p0: p1], S1[:, p0: p1],
                                  S1[:, p0 + 2: p1 + 2], OP.add)
            for (p0, p1) in _split(16 if half == 0 else CB,
                                   CB if half == 0 else Tc + 16, SSTEP):
                eng.tensor_tensor(S1[:, p0: p1], FFC[:, p0 - 3: p1 - 3],
                                  FFC[:, p0 + 1: p1 + 1], OP.add)

        def l9(half):  # final fc + clip + output DMA
            S1 = tiles['S1']
            for (glo, ghi) in _split(16 if half == 0 else CB,
                                     CB if half == 0 else Tc + 16, PS_GROUP):
                ps = ppool_d.tile([128, PS_GROUP], F32, tag=f"ps{PS_GROUP}", name="ps",
                                  bufs=PS_BUFS)
                for (lo, hi) in _split(glo, ghi, 512):
                    n, off = hi - lo, lo - glo
                    nc.tensor.matmul(ps[0:64, off:off + n], lhsT(16, 128, 64),
                                     S1[:, lo: hi], start=True, stop=True)
                    nc.vector.tensor_scalar(
                        Y[:, b * Tc + lo - 16: b * Tc + hi - 16],
                        ps[0:64, off:off + n], 0.0, 1.0, OP.max, OP.min)
            (h0, h1) = (0, CB - 16) if half == 0 else (CB - 16, Tc)
            nc.sync.dma_start(out=o_d[b, :, :, h0:h1],
                              in_=Y[0:64, b * Tc + h0: b * Tc + h1])

        return [l0, l1, l2, l3, l4, l5, l6, l7, l8, l9]

    batch_layers = [make_layers(b) for b in range(BPC)]
    jobs = [(0, 0), (0, 1), (1, 0), (1, 1)]
    L = len(batch_layers[0])
    for k in range(L + SKEW * (len(jobs) - 1)):
        # emit deeper-pipelined jobs first: a half-0 layer reads a few
        # boundary columns from the next job's previous layer, which must
        # appear earlier in program order for the dependency to register
        for j, (b, h) in reversed(list(enumerate(jobs))):
            kk = k - SKEW * j
            if 0 <= kk < L:
                batch_layers[b][kk](h)


def _get_program(reps=1):
    global _PROG
    if _PROG is None:
        _PROG = {}
    if reps not in _PROG:
        _PROG[reps] = _build_program(reps)
    return _PROG[reps]


def _prepare_inputs(inputs):
    x = np.asarray(inputs['speech_features'], np.float32)
    xp = np.zeros((B, C + 1, T + 22), np.float32)
    xp[:, :C, 10:10 + T] = x
    xp[:, C, :] = 1.0
    # chunk-major: [B, 4, 30, Tc+22]; chunk c covers padded cols [c*Tc, c*Tc+Tc+22)
    xa = np.zeros((B, 4, C + 1, Tc + 22), np.float32)
    for c in range(4):
        xa[:, c] = xp[:, :, c * Tc: c * Tc + Tc + 22]
    xa = xa.astype(BF)
    wp, wb = _pack_weights({k: np.asarray(v, np.float32)
                            for k, v in inputs.items() if k != 'speech_features'})
    return [{"x": xa[i * BPC:(i + 1) * BPC], "wpack": wp, "wbias": wb}
            for i in range(NCORES)]


def kernel(**inputs):
    from concourse.bass_utils import run_bass_kernel_spmd

    in_maps = _prepare_inputs(inputs)
    nc = _get_program()
    res = run_bass_kernel_spmd(nc, in_maps, core_ids=list(range(NCORES)))
    outs = []
    for r in res.results:
        o = np.asarray(r["out"]).astype(np.float32)      # [BPC, 4, 16, Tc]
        o = o.transpose(0, 1, 3, 2).reshape(BPC, T, 16)  # [BPC, T, 16]
        outs.append(o)
    return np.ascontiguousarray(np.concatenate(outs, axis=0))


# revision 48
# speedup vs baseline: 1.0342x; 1.0052x over previous
"""Trainium2 Bass kernel for nn_CNNGenerator (frame CNN + FC + window-sum + FC).

Key algebraic facts exploited (validated vs the reference):
  * softmax over a size-1 axis == 1.0, so the whole attention_conv stack is
    dead code; the bmm reduces to an 8-wide sliding-window sum of ffc.
  * The per-window stride-2 conv stack collapses into global conv streams:
    an "interior" stream g{1,2,3} and a "left-edge" stream e{1,2,3} per
    layer, plus a 2-tap combine (z).  Per t:
      g1[s] = b1 + sum_k W1k x[s+k-8]          e1[t] = b1 + W11 x[t-7] + W12 x[t-6]
      g2[s] = b2 + V0 G1[s-2] + V1 G1[s] + V2 G1[s+2]
      e2[t] = b2 + V1 E1[t] + V2 G1[t+2]
      g3[s] = b3 + U0 G2[s-4] + U1 G2[s] + U2 G2[s+4]
      e3[t] = b3 + U1 E2[t] + U2 G2[t+4]
      z[t]  = b4 + T1 E3[t] + T2 G3[t+8]
    (capitals = leaky-activated streams), then fc1/fc2/fc3+tanh,
    ws[t] = sum_{d=-3..4} ffc[t+d], out = clip(fcw @ ws, 0, 1).

Sharding: pure data parallel, 2 batch elements per core on 8 cores.
On-chip layout: time axis split in 4 chunks of 2048; 32-channel streams pack
4 chunks x 32ch on the 128 partitions, 64-channel streams pack 2 chunks x 64ch
(two tiles).  All matmul operands are bf16 (fp32 PSUM accumulate); evacuation
work is split between the Activation engine (Prelu/Tanh) and DVE (one-op
leaky via scalar_tensor_tensor).  G3 runs as 2 matmuls (not 3) and E3 as 1
(not 2) against DMA-assembled tiles that stack a stream with a column-shifted
copy on the partition axis, doubling effective contraction per column.
"""
import sys

if '/opt/trn_rl_repo' not in sys.path:
    sys.path.insert(0, '/opt/trn_rl_repo')

import numpy as np
import ml_dtypes

BF = ml_dtypes.bfloat16

B, C, T = 16, 29, 8192
NCORES = 8
BPC = B // NCORES          # batch per core
Tc = T // 4                # time chunk
HL = 16                    # left halo: tile col u <-> global idx c*Tc + u - HL
W = Tc + 40                # per-batch stream tile width
NSLOT = 17                 # 128-col lhsT slots in the weight pack

_PROG = {}
PS_GROUP = 1024
PS_BUFS = 4
ST_BUFS = 22

# evacuation routing: which streams drain on DVE instead of Activation.
# 'dve1' = one-op leaky (psum already contains bias), 'dve2' = bias-add +
# leaky (two DVE ops), 'act' = Activation Prelu.
ROUTE = {
    'G1': 'act', 'E1': 'act',
    'G2': 'act', 'E2': 'dve2',
    'G3_0': 'dve2', 'G3_1': 'act', 'E3': 'act', 'H': 'act',
    'H1_0': 'dve2', 'H1_1': 'act', 'H1_2': 'dve2', 'H1_3': 'act',
    'A2': 'act', 'FFC': 'act',
}
SKEW = 3


def _blockdiag(blocks):
    k = sum(b.shape[0] for b in blocks)
    m = sum(b.shape[1] for b in blocks)
    out = np.zeros((k, m), np.float32)
    i = j = 0
    for b in blocks:
        out[i:i + b.shape[0], j:j + b.shape[1]] = b
        i += b.shape[0]
        j += b.shape[1]
    return out


def _pack_weights(inp):
    wp = np.zeros((128, NSLOT * 128), np.float32)

    def put(slot, mat):
        wp[:mat.shape[0], slot * 128: slot * 128 + mat.shape[1]] = mat

    w1 = inp['w1'].astype(np.float32)  # [32, 29, 3]
    b1 = inp['b1'].astype(np.float32)
    # slots 0-2: g1 taps: blockdiag4 of [30, 32]: 29 in-ch rows + bias row
    for k in range(3):
        blk = np.zeros((30, 32), np.float32)
        blk[:29] = w1[:, :, k].T
        if k == 1:
            blk[29] = b1
        put(k, _blockdiag([blk] * 4))
    w2 = inp['w2'].astype(np.float32)
    for k in range(3):  # slots 3-5
        put(3 + k, _blockdiag([w2[:, :, k].T.astype(np.float32)] * 4))
    w3 = inp['w3'].astype(np.float32)
    w3t = [_blockdiag([w3[:, :, k].T.astype(np.float32)] * 2) for k in range(3)]
    # slot 6: G3 merged tap: [G2p ; G2p<<8] -> U0 on rows 0:64, U2 on 64:128
    put(6, np.concatenate([w3t[0], w3t[2]], axis=0))
    # slot 7: G3 center tap U1 on G3d rows 0:64
    put(7, w3t[1])
    # slot 8: E3 merged tap: [E2p ; G2p<<4] -> U1 rows 0:64, U2 rows 64:128
    put(8, np.concatenate([w3t[1], w3t[2]], axis=0))
    w4 = inp['w4'].astype(np.float32)
    for j in (1, 2):    # slots 9-10: [128, 128]
        put(9 + j - 1, _blockdiag([w4[:, :, j].T.astype(np.float32)] * 2))
    fw1t = inp['fw1'].T.astype(np.float32)
    put(11, np.concatenate([fw1t, fw1t], axis=0))               # [64,128] x2 rows
    # fc2 / fc3 as M=128 with zero column-halves: psum accumulation composes
    # the two chunk-halves onto partitions 0:64 / 64:128 without col-tiling.
    fw2t = inp['fw2'].T.astype(np.float32)          # [128, 64]
    z64 = np.zeros_like(fw2t)
    put(12, np.concatenate([fw2t, z64], axis=1))    # fc2_lo [128, 128]
    put(13, np.concatenate([z64, fw2t], axis=1))    # fc2_hi
    fw3t = _blockdiag([inp['fw3'].T.astype(np.float32)] * 2)       # [128, 64]
    z64b = np.zeros_like(fw3t)
    put(14, np.concatenate([fw3t, z64b], axis=1))   # fc3_lo
    put(15, np.concatenate([z64b, fw3t], axis=1))   # fc3_hi
    put(16, _blockdiag([inp['fcw'].T.astype(np.float32)] * 4))     # [128, 64]

    wb = np.zeros((128, 8), np.float32)
    wb[:, 0] = np.tile(inp['b2'], 4)
    wb[:, 1] = np.tile(inp['b3'], 2)
    wb[:, 2] = np.tile(inp['b4'], 2)
    wb[:, 3] = inp['fb1']
    wb[:, 4] = np.tile(inp['fb2'], 2)
    wb[:, 5] = np.tile(inp['fb3'], 4)
    return wp.astype(BF), wb


def _split(lo, hi, step=512):
    return [(a, min(a + step, hi)) for a in range(lo, hi, step)]


def _build_program(reps=1):
    import concourse.bacc as bacc
    import concourse.mybir as mybir
    import concourse.tile as tile

    F32 = mybir.dt.float32
    BF16 = mybir.dt.bfloat16

    nc = bacc.Bacc("TRN2", target_bir_lowering=False, debug=False)
    x_d = nc.dram_tensor("x", [BPC, 4, C + 1, Tc + 22], BF16, kind="ExternalInput").ap()
    w_d = nc.dram_tensor("wpack", [128, NSLOT * 128], BF16, kind="ExternalInput").ap()
    wb_d = nc.dram_tensor("wbias", [128, 8], F32, kind="ExternalInput").ap()
    o_d = nc.dram_tensor("out", [BPC, 4, 16, Tc], BF16, kind="ExternalOutput").ap()

    with tile.TileContext(nc) as tc:
        with tc.tile_pool(name="wp", bufs=1) as wpool, \
             tc.tile_pool(name="xp", bufs=1) as xpool, \
             tc.tile_pool(name="yp", bufs=1) as ypool, \
             tc.tile_pool(name="st", bufs=ST_BUFS) as spool, \
             tc.tile_pool(name="psa", bufs=PS_BUFS, space="PSUM") as ppool_a:

            wsb = wpool.tile([128, NSLOT * 128], BF16, tag="w")
            wbb = wpool.tile([128, 8], F32, tag="wb")

            for _rep in range(reps):
                _emit_body(nc, tc, mybir, F32, BF16, wsb, wbb,
                           xpool, ypool, spool, (ppool_a, ppool_a), x_d, o_d,
                           w_d, wb_d)
    nc.finalize()
    return nc


def _emit_body(nc, tc, mybir, F32, BF16, wsb, wbb,
               xpool, ypool, spool, ppool, x_d, o_d):
    AF = mybir.ActivationFunctionType
    OP = mybir.AluOpType

    def lhsT(slot, k=128, m=128, base=0):
        return wsb[base:base + k, slot * 128: slot * 128 + m]

    def bias(i):
        return wbb[:, i: i + 1]

    # ---------------- input load: per batch [120p = 4 x (29ch + ones), W]
    # host pre-pads x to [4, 30, Tc+22] (chunk-major so one DMA fills all 120
    # partitions); each load covers tile cols [6, Tc+28) in 4 column pieces.
    X = [None] * BPC
    for b in range(BPC):
        X[b] = xpool.tile([120, W], BF16, tag="x", name=f"x{b}")
    XP = [(0, 800), (800, 1500), (1500, Tc + 22)]
    nc.sync.dma_start(out=wsb[:, 0:768], in_=w_d[:, 0:768])
    for (s0, s1) in XP:
        nc.sync.dma_start(out=X[0][0:120, 6 + s0: 6 + s1],
                          in_=x_d[0, :, :, s0: s1])
    nc.sync.dma_start(out=wbb[:], in_=wb_d[:])
    nc.sync.dma_start(out=wsb[:, 768:], in_=w_d[:, 768:])
    for (s0, s1) in XP:
        nc.sync.dma_start(out=X[1][0:120, 6 + s0: 6 + s1],
                          in_=x_d[1, :, :, s0: s1])

    Y = ypool.tile([64, BPC * Tc], BF16, tag="y")

    CB = 1037  # half boundary (tile col)

    def make_layers(b):
        ST = lambda nm: spool.tile([128, W], BF16, tag="st", name=f"{nm}_{b}")  # noqa: E731

        def conv_pass(out_tile, rng, taps, evac, pool, half):
            """taps: list of (lhsT_ap, rhs_tile, rp0, rp1, delta)."""
            lo = rng[0] if half == 0 else CB
            hi = CB if half == 0 else rng[1]
            for (glo, ghi) in _split(lo, hi, PS_GROUP):
                gn = ghi - glo
                ps = pool.tile([128, PS_GROUP], F32, tag="ps", name="ps")
                for (lo, hi) in _split(glo, ghi, 512):
                    n, off = hi - lo, lo - glo
                    for i, (lw, rt, rp0, rp1, d) in enumerate(taps):
                        nc.tensor.matmul(
                            ps[0:lw.shape[-1], off:off + n], lw,
                            rt[rp0:rp1, lo + d: hi + d],
                            start=(i == 0), stop=(i == len(taps) - 1))
                evac(ps[:, 0:gn], out_tile[:, glo:ghi])

        def pool_for(name):
            return ppool_a if ROUTE.get(name, 'act') == 'act' else ppool_d

        def psg_for(name):
            return PSA_GROUP if ROUTE.get(name, 'act') == 'act' else PS_GROUP

        def evac_for(name, alpha, bias_i, half=0):
            route = ROUTE.get(name, 'act')
            # tail: the last job's DVE-routed drains serialize behind the
            # window-sum/clip chain while Act idles; shift them to Act
            if TAILACT and b == BPC - 1 and half == len(CBS) and route in ('dve2',):
                route = 'act'

            def act(ps, ot):
                nc.scalar.activation(ot, ps, AF.Prelu,
                                     bias=bias(bias_i) if bias_i is not None else 0.0,
                                     scale=1.0, alpha=alpha)

            def dve1(ps, ot):
                # one PSUM read max per instruction: copy out, then leaky in SBUF
                nc.vector.tensor_scalar(ot, ps, 0.0, None, OP.add)
                nc.vector.scalar_tensor_tensor(ot, ot, alpha, ot, OP.mult, OP.max)

            def dve2(ps, ot):
                nc.vector.tensor_scalar(ot, ps, bias(bias_i), None, OP.add)
                nc.vector.scalar_tensor_tensor(ot, ot, alpha, ot, OP.mult, OP.max)

            def dver(ps, ot):
                nc.vector.tensor_scalar(
                    ot, ps, bias(bias_i) if bias_i is not None else 0.0, 0.0,
                    OP.add, OP.max)

            return {'act': act, 'dve1': dve1, 'dve2': dve2, 'dver': dver}[route]

        tiles = {}

        def getst(nm):
            if nm not in tiles:
                tiles[nm] = ST(nm)
            return tiles[nm]

        def l0(half):  # G1 + E1
            G1 = getst("G1")
            conv_pass(G1, (14, Tc + 34),
                      [(lhsT(k, 120), X[b], 0, 120, k - 8) for k in range(3)],
                      evac_for('G1', 0.02, None, half), pool_for('G1'), half, psg_for('G1'))
            E1 = getst("E1")
            conv_pass(E1, (13, Tc + 21),
                      [(lhsT(k, 120), X[b], 0, 120, k - 8) for k in (1, 2)],
                      evac_for('E1', 0.02, None, half), pool_for('E1'), half, psg_for('E1'))

        def l1(half):  # G2 + E2
            G1, E1 = tiles['G1'], tiles['E1']
            G2 = getst("G2")
            conv_pass(G2, (17, Tc + 33),
                      [(lhsT(3 + k), G1, 0, 128, 2 * (k - 1)) for k in range(3)],
                      evac_for('G2', 0.02, 0, half), pool_for('G2'), half, psg_for('G2'))
            E2 = getst("E2")
            conv_pass(E2, (13, Tc + 21),
                      [(lhsT(4), E1, 0, 128, 0), (lhsT(5), G1, 0, 128, 2)],
                      evac_for('E2', 0.02, 0, half), pool_for('E2'), half, psg_for('E2'))

        def l2(half):  # assemble [stream ; shifted-copy] tiles
            G2, E2 = tiles['G2'], tiles['E2']
            G3d = [getst("G3d0"), getst("G3d1")]
            EG3d = [getst("EG3d0"), getst("EG3d1")]
            tiles['G3d'], tiles['EG3d'] = G3d, EG3d
            for p in range(2):
                (a0, a1) = (17, CB) if half == 0 else (CB, Tc + 29)
                nc.gpsimd.dma_start(out=G3d[p][0:64, a0: a1],
                                  in_=G2[64 * p:64 * p + 64, a0: a1])
                (a0, a1) = (17, CB) if half == 0 else (CB, Tc + 25)
                nc.gpsimd.dma_start(out=G3d[p][64:128, a0: a1],
                                  in_=G2[64 * p:64 * p + 64, a0 + 8: a1 + 8])
                (a0, a1) = (13, CB) if half == 0 else (CB, Tc + 21)
                nc.sync.dma_start(out=EG3d[p][0:64, a0: a1],
                                  in_=E2[64 * p:64 * p + 64, a0: a1])
                nc.sync.dma_start(out=EG3d[p][64:128, a0: a1],
                                  in_=G2[64 * p:64 * p + 64, a0 + 4: a1 + 4])

        def l3(half):  # G3 + E3
            G3d, EG3d = tiles['G3d'], tiles['EG3d']
            tiles['G3'] = G3 = [getst("G3a"), getst("G3b")]
            tiles['E3'] = E3 = [getst("E3a"), getst("E3b")]
            for p in range(2):
                conv_pass(G3[p], (21, Tc + 29),
                          [(lhsT(6), G3d[p], 0, 128, -4),
                           (lhsT(7, 64), G3d[p], 0, 64, 0)],
                          evac_for(f'G3_{p}', 0.2, 1, half), pool_for(f'G3_{p}'), half, psg_for(f'G3_{p}'))
                conv_pass(E3[p], (13, Tc + 21),
                          [(lhsT(8), EG3d[p], 0, 128, 0)],
                          evac_for('E3', 0.2, 1, half), pool_for('E3'), half, psg_for('E3'))

        def l4(half):  # H
            G3, E3 = tiles['G3'], tiles['E3']
            tiles['H'] = H = [getst("Ha"), getst("Hb")]
            for p in range(2):
                conv_pass(H[p], (13, Tc + 21),
                          [(lhsT(9), E3[p], 0, 128, 0),
                           (lhsT(10), G3[p], 0, 128, 8)],
                          evac_for('H', 0.2, 2, half), pool_for('H'), half, psg_for('H'))

        def l5(half):  # H1 (fc1)
            H = tiles['H']
            tiles['H1'] = H1 = [getst("H1" + str(c)) for c in range(4)]
            for cidx in range(4):
                p, ph = cidx // 2, cidx % 2
                conv_pass(H1[cidx], (13, Tc + 21),
                          [(lhsT(11, 64, base=64 * ph), H[p],
                            64 * ph, 64 * ph + 64, 0)],
                          evac_for(f'H1_{cidx}', 0.02, 3, half), pool_for(f'H1_{cidx}'), half, psg_for(f'H1_{cidx}'))

        def l6(half):  # A2 (fc2)
            H1 = tiles['H1']
            tiles['A2'] = A2 = [getst("A2a"), getst("A2b")]
            for p in range(2):
                conv_pass(A2[p], (13, Tc + 21),
                          [(lhsT(12), H1[2 * p], 0, 128, 0),
                           (lhsT(13), H1[2 * p + 1], 0, 128, 0)],
                          evac_for('A2', 0.02, 4, half), pool_for('A2'), half, psg_for('A2'))

        def l7(half):  # FFC (fc3 + tanh)
            A2 = tiles['A2']
            tiles['FFC'] = FFC = getst("FFC")

            def tanh_evac(ps, ot):
                nc.scalar.activation(ot, ps, AF.Tanh, bias=bias(5), scale=1.0)

            conv_pass(FFC, (13, Tc + 21),
                      [(lhsT(14), A2[0], 0, 128, 0),
                       (lhsT(15), A2[1], 0, 128, 0)],
                      tanh_evac, ppool_a, half, PSA_GROUP)
            # phantom edge values must read as zero in the window sum
            if half == 0:
                nc.gpsimd.memset(FFC[0:32, 13:16], 0.0)
            else:
                nc.gpsimd.memset(FFC[96:128, Tc + 16: Tc + 21], 0.0)

        def l8(half):  # window-sum tree (8-wide): piece-split for pipelining
            # half 0 owns cols [.., CB), half 1 [CB, ..); ops at the boundary
            # read a few columns across it (producer half finished earlier).
            FFC = tiles['FFC']
            S1 = getst("S1")
            tiles['S1'] = S1
            SSTEP = 512
            # Pool is slow (no 16-bit speedup); the final job's sums sit on
            # the critical tail, so run those on DVE instead.
            eng = nc.vector
            # staggered split points: each op's half-0 range ends before the
            # columns that would read the NEXT op stage across the boundary
            for (p0, p1) in _split(13 if half == 0 else CB + 3,
                                   CB + 3 if half == 0 else Tc + 19, SSTEP):
                eng.tensor_tensor(S1[:, p0: p1], FFC[:, p0: p1],
                                  FFC[:, p0 + 1: p1 + 1], OP.add)
            for (p0, p1) in _split(13 if half == 0 else CB + 1,
                                   CB + 1 if half == 0 else Tc + 17, SSTEP):
                eng.tensor_tensor(FFC[:, p0: p1], S1[:, p0: p1],
                                  S1[:, p0 + 2: p1 + 2], OP.add)
            for (p0, p1) in _split(16 if half == 0 else CB,
                                   CB if half == 0 else Tc + 16, SSTEP):
                eng.tensor_tensor(S1[:, p0: p1], FFC[:, p0 - 3: p1 - 3],
                                  FFC[:, p0 + 1: p1 + 1], OP.add)

        def l9(half):  # final fc + clip + output DMA
            S1 = tiles['S1']
            for (glo, ghi) in _split(16 if half == 0 else CB,
                                     CB if half == 0 else Tc + 16, PS_GROUP):
                ps = ppool_d.tile([128, PS_GROUP], F32, tag=f"ps{PS_GROUP}", name="ps",
                                  bufs=PS_BUFS)
                for (lo, hi) in _split(glo, ghi, 512):
                    n, off = hi - lo, lo - glo
                    nc.tensor.matmul(ps[0:64, off:off + n], lhsT(16, 128, 64),
                                     S1[:, lo: hi], start=True, stop=True)
                    nc.vector.tensor_scalar(
                        Y[:, b * Tc + lo - 16: b * Tc + hi - 16],
                        ps[0:64, off:off + n], 0.0, 1.0, OP.max, OP.min)
            (h0, h1) = (0, CB - 16) if half == 0 else (CB - 16, Tc)
            nc.sync.dma_start(out=o_d[b, :, :, h0:h1],
                              in_=Y[0:64, b * Tc + h0: b * Tc + h1])

        return [l0, l1, l2, l3, l4, l5, l6, l7, l8, l9]

    batch_layers = [make_layers(b) for b in range(BPC)]
    jobs = [(0, 0), (0, 1), (1, 0), (1, 1)]
    L = len(batch_layers[0])
    for k in range(L + SKEW * (len(jobs) - 1)):
        # emit deeper-pipelined jobs first: a half-0 layer reads a few
        # boundary columns from the next job's previous layer, which must
        # appear earlier in program order for the dependency to register
        for j, (b, h) in reversed(list(enumerate(jobs))):
            kk = k - SKEW * j
            if 0 <= kk < L:
                batch_layers[b][kk](h)


def _get_program(reps=1):
    global _PROG
    if _PROG is None:
        _PROG = {}
    if reps not in _PROG:
        _PROG[reps] = _build_program(reps)
    return _PROG[reps]


def _prepare_inputs(inputs):
    x = np.asarray(inputs['speech_features'], np.float32)
    xp = np.zeros((B, C + 1, T + 22), np.float32)
    xp[:, :C, 10:10 + T] = x
    xp[:, C, :] = 1.0
    # chunk-major: [B, 4, 30, Tc+22]; chunk c covers padded cols [c*Tc, c*Tc+Tc+22)
    xa = np.zeros((B, 4, C + 1, Tc + 22), np.float32)
    for c in range(4):
        xa[:, c] = xp[:, :, c * Tc: c * Tc + Tc + 22]
    xa = xa.astype(BF)
    wp, wb = _pack_weights({k: np.asarray(v, np.float32)
                            for k, v in inputs.items() if k != 'speech_features'})
    return [{"x": xa[i * BPC:(i + 1) * BPC], "wpack": wp, "wbias": wb}
            for i in range(NCORES)]


def kernel(**inputs):
    from concourse.bass_utils import run_bass_kernel_spmd

    in_maps = _prepare_inputs(inputs)
    nc = _get_program()
    res = run_bass_kernel_spmd(nc, in_maps, core_ids=list(range(NCORES)))
    outs = []
    for r in res.results:
        o = np.asarray(r["out"]).astype(np.float32)      # [BPC, 4, 16, Tc]
        o = o.transpose(0, 1, 3, 2).reshape(BPC, T, 16)  # [BPC, T, 16]
        outs.append(o)
    return np.ascontiguousarray(np.concatenate(outs, axis=0))


# revision 49
# speedup vs baseline: 1.0521x; 1.0172x over previous
"""Trainium2 Bass kernel for nn_CNNGenerator (frame CNN + FC + window-sum + FC).

Key algebraic facts exploited (validated vs the reference):
  * softmax over a size-1 axis == 1.0, so the whole attention_conv stack is
    dead code; the bmm reduces to an 8-wide sliding-window sum of ffc.
  * The per-window stride-2 conv stack collapses into global conv streams:
    an "interior" stream g{1,2,3} and a "left-edge" stream e{1,2,3} per
    layer, plus a 2-tap combine (z).  Per t:
      g1[s] = b1 + sum_k W1k x[s+k-8]          e1[t] = b1 + W11 x[t-7] + W12 x[t-6]
      g2[s] = b2 + V0 G1[s-2] + V1 G1[s] + V2 G1[s+2]
      e2[t] = b2 + V1 E1[t] + V2 G1[t+2]
      g3[s] = b3 + U0 G2[s-4] + U1 G2[s] + U2 G2[s+4]
      e3[t] = b3 + U1 E2[t] + U2 G2[t+4]
      z[t]  = b4 + T1 E3[t] + T2 G3[t+8]
    (capitals = leaky-activated streams), then fc1/fc2/fc3+tanh,
    ws[t] = sum_{d=-3..4} ffc[t+d], out = clip(fcw @ ws, 0, 1).

Sharding: pure data parallel, 2 batch elements per core on 8 cores.
On-chip layout: time axis split in 4 chunks of 2048; 32-channel streams pack
4 chunks x 32ch on the 128 partitions, 64-channel streams pack 2 chunks x 64ch
(two tiles).  All matmul operands are bf16 (fp32 PSUM accumulate); evacuation
work is split between the Activation engine (Prelu/Tanh) and DVE (one-op
leaky via scalar_tensor_tensor).  G3 runs as 2 matmuls (not 3) and E3 as 1
(not 2) against DMA-assembled tiles that stack a stream with a column-shifted
copy on the partition axis, doubling effective contraction per column.
"""
import sys

if '/opt/trn_rl_repo' not in sys.path:
    sys.path.insert(0, '/opt/trn_rl_repo')

import numpy as np
import ml_dtypes

BF = ml_dtypes.bfloat16

B, C, T = 16, 29, 8192
NCORES = 8
BPC = B // NCORES          # batch per core
Tc = T // 4                # time chunk
HL = 16                    # left halo: tile col u <-> global idx c*Tc + u - HL
W = Tc + 40                # per-batch stream tile width
NSLOT = 17                 # 128-col lhsT slots in the weight pack

_PROG = {}
PS_GROUP = 1024
PS_BUFS = 4
ST_BUFS = 22

# evacuation routing: which streams drain on DVE instead of Activation.
# 'dve1' = one-op leaky (psum already contains bias), 'dve2' = bias-add +
# leaky (two DVE ops), 'act' = Activation Prelu.
ROUTE = {
    'G1': 'act', 'E1': 'act',
    'G2': 'act', 'E2': 'dve2',
    'G3_0': 'dve2', 'G3_1': 'act', 'E3': 'act', 'H': 'act',
    'H1_0': 'dve2', 'H1_1': 'act', 'H1_2': 'dve2', 'H1_3': 'act',
    'A2': 'act', 'FFC': 'act',
}
SKEW = 3


def _blockdiag(blocks):
    k = sum(b.shape[0] for b in blocks)
    m = sum(b.shape[1] for b in blocks)
    out = np.zeros((k, m), np.float32)
    i = j = 0
    for b in blocks:
        out[i:i + b.shape[0], j:j + b.shape[1]] = b
        i += b.shape[0]
        j += b.shape[1]
    return out


def _pack_weights(inp):
    wp = np.zeros((128, NSLOT * 128), np.float32)

    def put(slot, mat):
        wp[:mat.shape[0], slot * 128: slot * 128 + mat.shape[1]] = mat

    w1 = inp['w1'].astype(np.float32)  # [32, 29, 3]
    b1 = inp['b1'].astype(np.float32)
    # slots 0-2: g1 taps: blockdiag4 of [30, 32]: 29 in-ch rows + bias row
    for k in range(3):
        blk = np.zeros((30, 32), np.float32)
        blk[:29] = w1[:, :, k].T
        if k == 1:
            blk[29] = b1
        put(k, _blockdiag([blk] * 4))
    w2 = inp['w2'].astype(np.float32)
    for k in range(3):  # slots 3-5
        put(3 + k, _blockdiag([w2[:, :, k].T.astype(np.float32)] * 4))
    w3 = inp['w3'].astype(np.float32)
    w3t = [_blockdiag([w3[:, :, k].T.astype(np.float32)] * 2) for k in range(3)]
    # slot 6: G3 merged tap: [G2p ; G2p<<8] -> U0 on rows 0:64, U2 on 64:128
    put(6, np.concatenate([w3t[0], w3t[2]], axis=0))
    # slot 7: G3 center tap U1 on G3d rows 0:64
    put(7, w3t[1])
    # slot 8: E3 merged tap: [E2p ; G2p<<4] -> U1 rows 0:64, U2 rows 64:128
    put(8, np.concatenate([w3t[1], w3t[2]], axis=0))
    w4 = inp['w4'].astype(np.float32)
    for j in (1, 2):    # slots 9-10: [128, 128]
        put(9 + j - 1, _blockdiag([w4[:, :, j].T.astype(np.float32)] * 2))
    fw1t = inp['fw1'].T.astype(np.float32)
    put(11, np.concatenate([fw1t, fw1t], axis=0))               # [64,128] x2 rows
    # fc2 / fc3 as M=128 with zero column-halves: psum accumulation composes
    # the two chunk-halves onto partitions 0:64 / 64:128 without col-tiling.
    fw2t = inp['fw2'].T.astype(np.float32)          # [128, 64]
    z64 = np.zeros_like(fw2t)
    put(12, np.concatenate([fw2t, z64], axis=1))    # fc2_lo [128, 128]
    put(13, np.concatenate([z64, fw2t], axis=1))    # fc2_hi
    fw3t = _blockdiag([inp['fw3'].T.astype(np.float32)] * 2)       # [128, 64]
    z64b = np.zeros_like(fw3t)
    put(14, np.concatenate([fw3t, z64b], axis=1))   # fc3_lo
    put(15, np.concatenate([z64b, fw3t], axis=1))   # fc3_hi
    put(16, _blockdiag([inp['fcw'].T.astype(np.float32)] * 4))     # [128, 64]

    wb = np.zeros((128, 8), np.float32)
    wb[:, 0] = np.tile(inp['b2'], 4)
    wb[:, 1] = np.tile(inp['b3'], 2)
    wb[:, 2] = np.tile(inp['b4'], 2)
    wb[:, 3] = inp['fb1']
    wb[:, 4] = np.tile(inp['fb2'], 2)
    wb[:, 5] = np.tile(inp['fb3'], 4)
    return wp.astype(BF), wb


def _split(lo, hi, step=512):
    return [(a, min(a + step, hi)) for a in range(lo, hi, step)]


def _build_program(reps=1):
    import concourse.bacc as bacc
    import concourse.mybir as mybir
    import concourse.tile as tile

    F32 = mybir.dt.float32
    BF16 = mybir.dt.bfloat16

    nc = bacc.Bacc("TRN2", target_bir_lowering=False, debug=False)
    x_d = nc.dram_tensor("x", [BPC, 4, C + 1, Tc + 22], BF16, kind="ExternalInput").ap()
    w_d = nc.dram_tensor("wpack", [128, NSLOT * 128], BF16, kind="ExternalInput").ap()
    wb_d = nc.dram_tensor("wbias", [128, 8], F32, kind="ExternalInput").ap()
    o_d = nc.dram_tensor("out", [BPC, 4, 16, Tc], BF16, kind="ExternalOutput").ap()

    with tile.TileContext(nc) as tc:
        with tc.tile_pool(name="wp", bufs=1) as wpool, \
             tc.tile_pool(name="xp", bufs=1) as xpool, \
             tc.tile_pool(name="yp", bufs=1) as ypool, \
             tc.tile_pool(name="st", bufs=ST_BUFS) as spool, \
             tc.tile_pool(name="psa", bufs=PS_BUFS, space="PSUM") as ppool_a:

            wsb = wpool.tile([128, NSLOT * 128], BF16, tag="w")
            wbb = wpool.tile([128, 8], F32, tag="wb")

            for _rep in range(reps):
                _emit_body(nc, tc, mybir, F32, BF16, wsb, wbb,
                           xpool, ypool, spool, (ppool_a, ppool_a), x_d, o_d,
                           w_d, wb_d)
    nc.finalize()
    return nc


def _emit_body(nc, tc, mybir, F32, BF16, wsb, wbb,
               xpool, ypool, spool, ppool, x_d, o_d):
    AF = mybir.ActivationFunctionType
    OP = mybir.AluOpType

    def lhsT(slot, k=128, m=128, base=0):
        return wsb[base:base + k, slot * 128: slot * 128 + m]

    def bias(i):
        return wbb[:, i: i + 1]

    # ---------------- input load: per batch [120p = 4 x (29ch + ones), W]
    # host pre-pads x to [4, 30, Tc+22] (chunk-major so one DMA fills all 120
    # partitions); each load covers tile cols [6, Tc+28) in 4 column pieces.
    X = [None] * BPC
    for b in range(BPC):
        X[b] = xpool.tile([120, W], BF16, tag="x", name=f"x{b}")
    XP = [(0, 800), (800, 1500), (1500, Tc + 22)]
    nc.sync.dma_start(out=wsb[:, 0:768], in_=w_d[:, 0:768])
    for (s0, s1) in XP:
        nc.sync.dma_start(out=X[0][0:120, 6 + s0: 6 + s1],
                          in_=x_d[0, :, :, s0: s1])
    nc.sync.dma_start(out=wbb[:], in_=wb_d[:])
    nc.sync.dma_start(out=wsb[:, 768:], in_=w_d[:, 768:])
    for (s0, s1) in XP:
        nc.sync.dma_start(out=X[1][0:120, 6 + s0: 6 + s1],
                          in_=x_d[1, :, :, s0: s1])

    Y = ypool.tile([64, BPC * Tc], BF16, tag="y")

    CB = 1037  # half boundary (tile col)

    def make_layers(b):
        ST = lambda nm: spool.tile([128, W], BF16, tag="st", name=f"{nm}_{b}")  # noqa: E731

        def conv_pass(out_tile, rng, taps, evac, pool, half):
            """taps: list of (lhsT_ap, rhs_tile, rp0, rp1, delta)."""
            lo = rng[0] if half == 0 else CB
            hi = CB if half == 0 else rng[1]
            for (glo, ghi) in _split(lo, hi, PS_GROUP):
                gn = ghi - glo
                ps = pool.tile([128, PS_GROUP], F32, tag="ps", name="ps")
                for (lo, hi) in _split(glo, ghi, 512):
                    n, off = hi - lo, lo - glo
                    for i, (lw, rt, rp0, rp1, d) in enumerate(taps):
                        nc.tensor.matmul(
                            ps[0:lw.shape[-1], off:off + n], lw,
                            rt[rp0:rp1, lo + d: hi + d],
                            start=(i == 0), stop=(i == len(taps) - 1))
                evac(ps[:, 0:gn], out_tile[:, glo:ghi])

        def pool_for(name):
            return ppool_a if ROUTE.get(name, 'act') == 'act' else ppool_d

        def psg_for(name):
            return PSA_GROUP if ROUTE.get(name, 'act') == 'act' else PS_GROUP

        def evac_for(name, alpha, bias_i, half=0):
            route = ROUTE.get(name, 'act')
            # tail: the last job's DVE-routed drains serialize behind the
            # window-sum/clip chain while Act idles; shift them to Act
            if TAILACT and b == BPC - 1 and half == len(CBS) and route in ('dve2',):
                route = 'act'

            def act(ps, ot):
                nc.scalar.activation(ot, ps, AF.Prelu,
                                     bias=bias(bias_i) if bias_i is not None else 0.0,
                                     scale=1.0, alpha=alpha)

            def dve1(ps, ot):
                # one PSUM read max per instruction: copy out, then leaky in SBUF
                nc.vector.tensor_scalar(ot, ps, 0.0, None, OP.add)
                nc.vector.scalar_tensor_tensor(ot, ot, alpha, ot, OP.mult, OP.max)

            def dve2(ps, ot):
                nc.vector.tensor_scalar(ot, ps, bias(bias_i), None, OP.add)
                nc.vector.scalar_tensor_tensor(ot, ot, alpha, ot, OP.mult, OP.max)

            def dver(ps, ot):
                nc.vector.tensor_scalar(
                    ot, ps, bias(bias_i) if bias_i is not None else 0.0, 0.0,
                    OP.add, OP.max)

            return {'act': act, 'dve1': dve1, 'dve2': dve2, 'dver': dver}[route]

        tiles = {}

        def getst(nm):
            if nm not in tiles:
                tiles[nm] = ST(nm)
            return tiles[nm]

        def l0(half):  # G1 + E1
            G1 = getst("G1")
            conv_pass(G1, (14, Tc + 34),
                      [(lhsT(k, 120), X[b], 0, 120, k - 8) for k in range(3)],
                      evac_for('G1', 0.02, None, half), pool_for('G1'), half, psg_for('G1'))
            E1 = getst("E1")
            conv_pass(E1, (13, Tc + 21),
                      [(lhsT(k, 120), X[b], 0, 120, k - 8) for k in (1, 2)],
                      evac_for('E1', 0.02, None, half), pool_for('E1'), half, psg_for('E1'))

        def l1(half):  # G2 + E2
            G1, E1 = tiles['G1'], tiles['E1']
            G2 = getst("G2")
            conv_pass(G2, (17, Tc + 33),
                      [(lhsT(3 + k), G1, 0, 128, 2 * (k - 1)) for k in range(3)],
                      evac_for('G2', 0.02, 0, half), pool_for('G2'), half, psg_for('G2'))
            E2 = getst("E2")
            conv_pass(E2, (13, Tc + 21),
                      [(lhsT(4), E1, 0, 128, 0), (lhsT(5), G1, 0, 128, 2)],
                      evac_for('E2', 0.02, 0, half), pool_for('E2'), half, psg_for('E2'))

        def l2(half):  # assemble [stream ; shifted-copy] tiles
            G2, E2 = tiles['G2'], tiles['E2']
            G3d = [getst("G3d0"), getst("G3d1")]
            EG3d = [getst("EG3d0"), getst("EG3d1")]
            tiles['G3d'], tiles['EG3d'] = G3d, EG3d
            for p in range(2):
                (a0, a1) = (17, CB) if half == 0 else (CB, Tc + 29)
                nc.gpsimd.dma_start(out=G3d[p][0:64, a0: a1],
                                  in_=G2[64 * p:64 * p + 64, a0: a1])
                (a0, a1) = (17, CB) if half == 0 else (CB, Tc + 25)
                nc.gpsimd.dma_start(out=G3d[p][64:128, a0: a1],
                                  in_=G2[64 * p:64 * p + 64, a0 + 8: a1 + 8])
                (a0, a1) = (13, CB) if half == 0 else (CB, Tc + 21)
                nc.sync.dma_start(out=EG3d[p][0:64, a0: a1],
                                  in_=E2[64 * p:64 * p + 64, a0: a1])
                nc.sync.dma_start(out=EG3d[p][64:128, a0: a1],
                                  in_=G2[64 * p:64 * p + 64, a0 + 4: a1 + 4])

        def l3(half):  # G3 + E3
            G3d, EG3d = tiles['G3d'], tiles['EG3d']
            tiles['G3'] = G3 = [getst("G3a"), getst("G3b")]
            tiles['E3'] = E3 = [getst("E3a"), getst("E3b")]
            for p in range(2):
                conv_pass(G3[p], (21, Tc + 29),
                          [(lhsT(6), G3d[p], 0, 128, -4),
                           (lhsT(7, 64), G3d[p], 0, 64, 0)],
                          evac_for(f'G3_{p}', 0.2, 1, half), pool_for(f'G3_{p}'), half, psg_for(f'G3_{p}'))
                conv_pass(E3[p], (13, Tc + 21),
                          [(lhsT(8), EG3d[p], 0, 128, 0)],
                          evac_for('E3', 0.2, 1, half), pool_for('E3'), half, psg_for('E3'))

        def l4(half):  # H
            G3, E3 = tiles['G3'], tiles['E3']
            tiles['H'] = H = [getst("Ha"), getst("Hb")]
            for p in range(2):
                conv_pass(H[p], (13, Tc + 21),
                          [(lhsT(9), E3[p], 0, 128, 0),
                           (lhsT(10), G3[p], 0, 128, 8)],
                          evac_for('H', 0.2, 2, half), pool_for('H'), half, psg_for('H'))

        def l5(half):  # H1 (fc1) - interleave pair halves
            H = tiles['H']
            tiles['H1'] = H1 = [getst("H1" + str(c)) for c in range(4)]
            for cidx in (0, 2, 1, 3):
                p, ph = cidx // 2, cidx % 2
                conv_pass(H1[cidx], (13, Tc + 21),
                          [(lhsT(11, 64, base=64 * ph), H[p],
                            64 * ph, 64 * ph + 64, 0)],
                          evac_for(f'H1_{cidx}', 0.02, 3, half), pool_for(f'H1_{cidx}'), half, psg_for(f'H1_{cidx}'))

        def l6(half):  # A2 (fc2)
            H1 = tiles['H1']
            tiles['A2'] = A2 = [getst("A2a"), getst("A2b")]
            for p in range(2):
                conv_pass(A2[p], (13, Tc + 21),
                          [(lhsT(12), H1[2 * p], 0, 128, 0),
                           (lhsT(13), H1[2 * p + 1], 0, 128, 0)],
                          evac_for('A2', 0.02, 4, half), pool_for('A2'), half, psg_for('A2'))

        def l7(half):  # FFC (fc3 + tanh)
            A2 = tiles['A2']
            tiles['FFC'] = FFC = getst("FFC")

            def tanh_evac(ps, ot):
                nc.scalar.activation(ot, ps, AF.Tanh, bias=bias(5), scale=1.0)

            conv_pass(FFC, (13, Tc + 21),
                      [(lhsT(14), A2[0], 0, 128, 0),
                       (lhsT(15), A2[1], 0, 128, 0)],
                      tanh_evac, ppool_a, half, PSA_GROUP)
            # phantom edge values must read as zero in the window sum
            if half == 0:
                nc.gpsimd.memset(FFC[0:32, 13:16], 0.0)
            else:
                nc.gpsimd.memset(FFC[96:128, Tc + 16: Tc + 21], 0.0)

        def l8(half):  # window-sum tree (8-wide): piece-split for pipelining
            # half 0 owns cols [.., CB), half 1 [CB, ..); ops at the boundary
            # read a few columns across it (producer half finished earlier).
            FFC = tiles['FFC']
            S1 = getst("S1")
            tiles['S1'] = S1
            SSTEP = 512
            # Pool is slow (no 16-bit speedup); the final job's sums sit on
            # the critical tail, so run those on DVE instead.
            eng = nc.vector
            # staggered split points: each op's half-0 range ends before the
            # columns that would read the NEXT op stage across the boundary
            for (p0, p1) in _split(13 if half == 0 else CB + 3,
                                   CB + 3 if half == 0 else Tc + 19, SSTEP):
                eng.tensor_tensor(S1[:, p0: p1], FFC[:, p0: p1],
                                  FFC[:, p0 + 1: p1 + 1], OP.add)
            for (p0, p1) in _split(13 if half == 0 else CB + 1,
                                   CB + 1 if half == 0 else Tc + 17, SSTEP):
                eng.tensor_tensor(FFC[:, p0: p1], S1[:, p0: p1],
                                  S1[:, p0 + 2: p1 + 2], OP.add)
            for (p0, p1) in _split(16 if half == 0 else CB,
                                   CB if half == 0 else Tc + 16, SSTEP):
                eng.tensor_tensor(S1[:, p0: p1], FFC[:, p0 - 3: p1 - 3],
                                  FFC[:, p0 + 1: p1 + 1], OP.add)

        def l9(half):  # final fc + clip + output DMA
            S1 = tiles['S1']
            for (glo, ghi) in _split(16 if half == 0 else CB,
                                     CB if half == 0 else Tc + 16, PS_GROUP):
                ps = ppool_d.tile([128, PS_GROUP], F32, tag=f"ps{PS_GROUP}", name="ps",
                                  bufs=PS_BUFS)
                for (lo, hi) in _split(glo, ghi, 512):
                    n, off = hi - lo, lo - glo
                    nc.tensor.matmul(ps[0:64, off:off + n], lhsT(16, 128, 64),
                                     S1[:, lo: hi], start=True, stop=True)
                    nc.vector.tensor_scalar(
                        Y[:, b * Tc + lo - 16: b * Tc + hi - 16],
                        ps[0:64, off:off + n], 0.0, 1.0, OP.max, OP.min)
            (h0, h1) = (0, CB - 16) if half == 0 else (CB - 16, Tc)
            nc.sync.dma_start(out=o_d[b, :, :, h0:h1],
                              in_=Y[0:64, b * Tc + h0: b * Tc + h1])

        return [l0, l1, l2, l3, l4, l5, l6, l7, l8, l9]

    batch_layers = [make_layers(b) for b in range(BPC)]
    jobs = [(0, 0), (0, 1), (1, 0), (1, 1)]
    L = len(batch_layers[0])
    for k in range(L + SKEW * (len(jobs) - 1)):
        # emit deeper-pipelined jobs first: a half-0 layer reads a few
        # boundary columns from the next job's previous layer, which must
        # appear earlier in program order for the dependency to register
        for j, (b, h) in reversed(list(enumerate(jobs))):
            kk = k - SKEW * j
            if 0 <= kk < L:
                batch_layers[b][kk](h)


def _get_program(reps=1):
    global _PROG
    if _PROG is None:
        _PROG = {}
    if reps not in _PROG:
        _PROG[reps] = _build_program(reps)
    return _PROG[reps]


def _prepare_inputs(inputs):
    x = np.asarray(inputs['speech_features'], np.float32)
    xp = np.zeros((B, C + 1, T + 22), np.float32)
    xp[:, :C, 10:10 + T] = x
    xp[:, C, :] = 1.0
    # chunk-major: [B, 4, 30, Tc+22]; chunk c covers padded cols [c*Tc, c*Tc+Tc+22)
    xa = np.zeros((B, 4, C + 1, Tc + 22), np.float32)
    for c in range(4):
        xa[:, c] = xp[:, :, c * Tc: c * Tc + Tc + 22]
    xa = xa.astype(BF)
    wp, wb = _pack_weights({k: np.asarray(v, np.float32)
                            for k, v in inputs.items() if k != 'speech_features'})
    return [{"x": xa[i * BPC:(i + 1) * BPC], "wpack": wp, "wbias": wb}
            for i in range(NCORES)]


def kernel(**inputs):
    from concourse.bass_utils import run_bass_kernel_spmd

    in_maps = _prepare_inputs(inputs)
    nc = _get_program()
    res = run_bass_kernel_spmd(nc, in_maps, core_ids=list(range(NCORES)))
    outs = []
    for r in res.results:
        o = np.asarray(r["out"]).astype(np.float32)      # [BPC, 4, 16, Tc]
        o = o.transpose(0, 1, 3, 2).reshape(BPC, T, 16)  # [BPC, T, 16]
        outs.append(o)
    return np.ascontiguousarray(np.concatenate(outs, axis=0))


# revision 54
# speedup vs baseline: 1.0543x; 1.0021x over previous
"""Trainium2 Bass kernel for nn_CNNGenerator (frame CNN + FC + window-sum + FC).

Key algebraic facts exploited (validated vs the reference):
  * softmax over a size-1 axis == 1.0, so the whole attention_conv stack is
    dead code; the bmm reduces to an 8-wide sliding-window sum of ffc.
  * The per-window stride-2 conv stack collapses into global conv streams:
    an "interior" stream g{1,2,3} and a "left-edge" stream e{1,2,3} per
    layer, plus a 2-tap combine (z).  Per t:
      g1[s] = b1 + sum_k W1k x[s+k-8]          e1[t] = b1 + W11 x[t-7] + W12 x[t-6]
      g2[s] = b2 + V0 G1[s-2] + V1 G1[s] + V2 G1[s+2]
      e2[t] = b2 + V1 E1[t] + V2 G1[t+2]
      g3[s] = b3 + U0 G2[s-4] + U1 G2[s] + U2 G2[s+4]
      e3[t] = b3 + U1 E2[t] + U2 G2[t+4]
      z[t]  = b4 + T1 E3[t] + T2 G3[t+8]
    (capitals = leaky-activated streams), then fc1/fc2/fc3+tanh,
    ws[t] = sum_{d=-3..4} ffc[t+d], out = clip(fcw @ ws, 0, 1).

Sharding: pure data parallel, 2 batch elements per core on 8 cores.
On-chip layout: time axis split in 4 chunks of 2048; 32-channel streams pack
4 chunks x 32ch on the 128 partitions, 64-channel streams pack 2 chunks x 64ch
(two tiles).  All matmul operands are bf16 (fp32 PSUM accumulate); evacuation
work is split between the Activation engine (Prelu/Tanh) and DVE (one-op
leaky via scalar_tensor_tensor).  G3 runs as 2 matmuls (not 3) and E3 as 1
(not 2) against DMA-assembled tiles that stack a stream with a column-shifted
copy on the partition axis, doubling effective contraction per column.
"""
import sys

if '/opt/trn_rl_repo' not in sys.path:
    sys.path.insert(0, '/opt/trn_rl_repo')

import numpy as np
import ml_dtypes

BF = ml_dtypes.bfloat16

B, C, T = 16, 29, 8192
NCORES = 8
BPC = B // NCORES          # batch per core
Tc = T // 4                # time chunk
HL = 16                    # left halo: tile col u <-> global idx c*Tc + u - HL
W = Tc + 40                # per-batch stream tile width
NSLOT = 17                 # 128-col lhsT slots in the weight pack

_PROG = {}
PS_GROUP = 1024
PS_BUFS = 4
ST_BUFS = 22

# evacuation routing: which streams drain on DVE instead of Activation.
# 'dve1' = one-op leaky (psum already contains bias), 'dve2' = bias-add +
# leaky (two DVE ops), 'act' = Activation Prelu.
ROUTE = {
    'G1': 'act', 'E1': 'act',
    'G2': 'act', 'E2': 'dve2',
    'G3_0': 'dve2', 'G3_1': 'act', 'E3': 'act', 'H': 'act',
    'H1_0': 'dve2', 'H1_1': 'act', 'H1_2': 'dve2', 'H1_3': 'act',
    'A2': 'act', 'FFC': 'act',
}
SKEW = 3


def _blockdiag(blocks):
    k = sum(b.shape[0] for b in blocks)
    m = sum(b.shape[1] for b in blocks)
    out = np.zeros((k, m), np.float32)
    i = j = 0
    for b in blocks:
        out[i:i + b.shape[0], j:j + b.shape[1]] = b
        i += b.shape[0]
        j += b.shape[1]
    return out


def _pack_weights(inp):
    wp = np.zeros((128, NSLOT * 128), np.float32)

    def put(slot, mat):
        wp[:mat.shape[0], slot * 128: slot * 128 + mat.shape[1]] = mat

    w1 = inp['w1'].astype(np.float32)  # [32, 29, 3]
    b1 = inp['b1'].astype(np.float32)
    # slots 0-2: g1 taps: blockdiag4 of [30, 32]: 29 in-ch rows + bias row
    for k in range(3):
        blk = np.zeros((30, 32), np.float32)
        blk[:29] = w1[:, :, k].T
        if k == 1:
            blk[29] = b1
        put(k, _blockdiag([blk] * 4))
    w2 = inp['w2'].astype(np.float32)
    for k in range(3):  # slots 3-5
        put(3 + k, _blockdiag([w2[:, :, k].T.astype(np.float32)] * 4))
    w3 = inp['w3'].astype(np.float32)
    w3t = [_blockdiag([w3[:, :, k].T.astype(np.float32)] * 2) for k in range(3)]
    # slot 6: G3 merged tap: [G2p ; G2p<<8] -> U0 on rows 0:64, U2 on 64:128
    put(6, np.concatenate([w3t[0], w3t[2]], axis=0))
    # slot 7: G3 center tap U1 on G3d rows 0:64
    put(7, w3t[1])
    # slot 8: E3 merged tap: [E2p ; G2p<<4] -> U1 rows 0:64, U2 rows 64:128
    put(8, np.concatenate([w3t[1], w3t[2]], axis=0))
    w4 = inp['w4'].astype(np.float32)
    for j in (1, 2):    # slots 9-10: [128, 128]
        put(9 + j - 1, _blockdiag([w4[:, :, j].T.astype(np.float32)] * 2))
    fw1t = inp['fw1'].T.astype(np.float32)
    put(11, np.concatenate([fw1t, fw1t], axis=0))               # [64,128] x2 rows
    # fc2 / fc3 as M=128 with zero column-halves: psum accumulation composes
    # the two chunk-halves onto partitions 0:64 / 64:128 without col-tiling.
    fw2t = inp['fw2'].T.astype(np.float32)          # [128, 64]
    z64 = np.zeros_like(fw2t)
    put(12, np.concatenate([fw2t, z64], axis=1))    # fc2_lo [128, 128]
    put(13, np.concatenate([z64, fw2t], axis=1))    # fc2_hi
    fw3t = _blockdiag([inp['fw3'].T.astype(np.float32)] * 2)       # [128, 64]
    z64b = np.zeros_like(fw3t)
    put(14, np.concatenate([fw3t, z64b], axis=1))   # fc3_lo
    put(15, np.concatenate([z64b, fw3t], axis=1))   # fc3_hi
    put(16, _blockdiag([inp['fcw'].T.astype(np.float32)] * 4))     # [128, 64]

    wb = np.zeros((128, 8), np.float32)
    wb[:, 0] = np.tile(inp['b2'], 4)
    wb[:, 1] = np.tile(inp['b3'], 2)
    wb[:, 2] = np.tile(inp['b4'], 2)
    wb[:, 3] = inp['fb1']
    wb[:, 4] = np.tile(inp['fb2'], 2)
    wb[:, 5] = np.tile(inp['fb3'], 4)
    return wp.astype(BF), wb


def _split(lo, hi, step=512):
    return [(a, min(a + step, hi)) for a in range(lo, hi, step)]


def _build_program(reps=1):
    import concourse.bacc as bacc
    import concourse.mybir as mybir
    import concourse.tile as tile

    F32 = mybir.dt.float32
    BF16 = mybir.dt.bfloat16

    nc = bacc.Bacc("TRN2", target_bir_lowering=False, debug=False)
    x_d = nc.dram_tensor("x", [BPC, 4, C + 1, Tc + 22], BF16, kind="ExternalInput").ap()
    w_d = nc.dram_tensor("wpack", [128, NSLOT * 128], BF16, kind="ExternalInput").ap()
    wb_d = nc.dram_tensor("wbias", [128, 8], F32, kind="ExternalInput").ap()
    o_d = nc.dram_tensor("out", [BPC, 4, 16, Tc], BF16, kind="ExternalOutput").ap()

    with tile.TileContext(nc) as tc:
        with tc.tile_pool(name="wp", bufs=1) as wpool, \
             tc.tile_pool(name="xp", bufs=1) as xpool, \
             tc.tile_pool(name="yp", bufs=1) as ypool, \
             tc.tile_pool(name="st", bufs=ST_BUFS) as spool, \
             tc.tile_pool(name="psa", bufs=PS_BUFS, space="PSUM") as ppool_a:

            wsb = wpool.tile([128, NSLOT * 128], BF16, tag="w")
            wbb = wpool.tile([128, 8], F32, tag="wb")

            for _rep in range(reps):
                _emit_body(nc, tc, mybir, F32, BF16, wsb, wbb,
                           xpool, ypool, spool, (ppool_a, ppool_a), x_d, o_d,
                           w_d, wb_d)
    nc.finalize()
    return nc


def _emit_body(nc, tc, mybir, F32, BF16, wsb, wbb,
               xpool, ypool, spool, ppool, x_d, o_d):
    AF = mybir.ActivationFunctionType
    OP = mybir.AluOpType

    def lhsT(slot, k=128, m=128, base=0):
        return wsb[base:base + k, slot * 128: slot * 128 + m]

    def bias(i):
        return wbb[:, i: i + 1]

    # ---------------- input load: per batch [120p = 4 x (29ch + ones), W]
    # host pre-pads x to [4, 30, Tc+22] (chunk-major so one DMA fills all 120
    # partitions); each load covers tile cols [6, Tc+28) in 4 column pieces.
    X = [None] * BPC
    for b in range(BPC):
        X[b] = xpool.tile([120, W], BF16, tag="x", name=f"x{b}")
    XP = [(0, 800), (800, 1500), (1500, Tc + 22)]
    nc.sync.dma_start(out=wsb[:, 0:768], in_=w_d[:, 0:768])
    for (s0, s1) in XP:
        nc.sync.dma_start(out=X[0][0:120, 6 + s0: 6 + s1],
                          in_=x_d[0, :, :, s0: s1])
    nc.sync.dma_start(out=wbb[:], in_=wb_d[:])
    nc.sync.dma_start(out=wsb[:, 768:], in_=w_d[:, 768:])
    for (s0, s1) in XP:
        nc.sync.dma_start(out=X[1][0:120, 6 + s0: 6 + s1],
                          in_=x_d[1, :, :, s0: s1])

    Y = ypool.tile([64, BPC * Tc], BF16, tag="y")

    CB = 1037  # half boundary (tile col)

    def make_layers(b):
        ST = lambda nm: spool.tile([128, W], BF16, tag="st", name=f"{nm}_{b}")  # noqa: E731

        def conv_pass(out_tile, rng, taps, evac, pool, half):
            """taps: list of (lhsT_ap, rhs_tile, rp0, rp1, delta)."""
            lo = rng[0] if half == 0 else CB
            hi = CB if half == 0 else rng[1]
            for (glo, ghi) in _split(lo, hi, PS_GROUP):
                gn = ghi - glo
                ps = pool.tile([128, PS_GROUP], F32, tag="ps", name="ps")
                for (lo, hi) in _split(glo, ghi, 512):
                    n, off = hi - lo, lo - glo
                    for i, (lw, rt, rp0, rp1, d) in enumerate(taps):
                        nc.tensor.matmul(
                            ps[0:lw.shape[-1], off:off + n], lw,
                            rt[rp0:rp1, lo + d: hi + d],
                            start=(i == 0), stop=(i == len(taps) - 1))
                evac(ps[:, 0:gn], out_tile[:, glo:ghi])

        def pool_for(name):
            return ppool_a if ROUTE.get(name, 'act') == 'act' else ppool_d

        def psg_for(name):
            return PSA_GROUP if ROUTE.get(name, 'act') == 'act' else PS_GROUP

        def evac_for(name, alpha, bias_i, half=0):
            route = ROUTE.get(name, 'act')
            # tail: the last job's DVE-routed drains serialize behind the
            # window-sum/clip chain while Act idles; shift them to Act
            if TAILACT and b == BPC - 1 and half == len(CBS) and route in ('dve2',):
                route = 'act'

            def act(ps, ot):
                nc.scalar.activation(ot, ps, AF.Prelu,
                                     bias=bias(bias_i) if bias_i is not None else 0.0,
                                     scale=1.0, alpha=alpha)

            def dve1(ps, ot):
                # one PSUM read max per instruction: copy out, then leaky in SBUF
                nc.vector.tensor_scalar(ot, ps, 0.0, None, OP.add)
                nc.vector.scalar_tensor_tensor(ot, ot, alpha, ot, OP.mult, OP.max)

            def dve2(ps, ot):
                nc.vector.tensor_scalar(ot, ps, bias(bias_i), None, OP.add)
                nc.vector.scalar_tensor_tensor(ot, ot, alpha, ot, OP.mult, OP.max)

            def dver(ps, ot):
                nc.vector.tensor_scalar(
                    ot, ps, bias(bias_i) if bias_i is not None else 0.0, 0.0,
                    OP.add, OP.max)

            return {'act': act, 'dve1': dve1, 'dve2': dve2, 'dver': dver}[route]

        tiles = {}

        def getst(nm):
            if nm not in tiles:
                tiles[nm] = ST(nm)
            return tiles[nm]

        def l0(half):  # G1 + E1
            G1 = getst("G1")
            conv_pass(G1, (14, Tc + 34),
                      [(lhsT(k, 120), X[b], 0, 120, k - 8) for k in range(3)],
                      evac_for('G1', 0.02, None, half), pool_for('G1'), half, psg_for('G1'))
            E1 = getst("E1")
            conv_pass(E1, (13, Tc + 21),
                      [(lhsT(k, 120), X[b], 0, 120, k - 8) for k in (1, 2)],
                      evac_for('E1', 0.02, None, half), pool_for('E1'), half, psg_for('E1'))

        def l1(half):  # G2 + E2
            G1, E1 = tiles['G1'], tiles['E1']
            E2 = getst("E2")
            conv_pass(E2, (13, Tc + 21),
                      [(lhsT(4), E1, 0, 128, 0), (lhsT(5), G1, 0, 128, 2)],
                      evac_for('E2', 0.02, 0, half), pool_for('E2'), half, psg_for('E2'))
            G2 = getst("G2")
            conv_pass(G2, (17, Tc + 33),
                      [(lhsT(3 + k), G1, 0, 128, 2 * (k - 1)) for k in range(3)],
                      evac_for('G2', 0.02, 0, half), pool_for('G2'), half, psg_for('G2'))

        def l2(half):  # assemble [stream ; shifted-copy] tiles
            G2, E2 = tiles['G2'], tiles['E2']
            G3d = [getst("G3d0"), getst("G3d1")]
            EG3d = [getst("EG3d0"), getst("EG3d1")]
            tiles['G3d'], tiles['EG3d'] = G3d, EG3d
            for p in range(2):
                (a0, a1) = (17, CB) if half == 0 else (CB, Tc + 29)
                nc.gpsimd.dma_start(out=G3d[p][0:64, a0: a1],
                                  in_=G2[64 * p:64 * p + 64, a0: a1])
                (a0, a1) = (17, CB) if half == 0 else (CB, Tc + 25)
                nc.gpsimd.dma_start(out=G3d[p][64:128, a0: a1],
                                  in_=G2[64 * p:64 * p + 64, a0 + 8: a1 + 8])
                (a0, a1) = (13, CB) if half == 0 else (CB, Tc + 21)
                nc.sync.dma_start(out=EG3d[p][0:64, a0: a1],
                                  in_=E2[64 * p:64 * p + 64, a0: a1])
                nc.sync.dma_start(out=EG3d[p][64:128, a0: a1],
                                  in_=G2[64 * p:64 * p + 64, a0 + 4: a1 + 4])

        def l3(half):  # G3 + E3
            G3d, EG3d = tiles['G3d'], tiles['EG3d']
            tiles['G3'] = G3 = [getst("G3a"), getst("G3b")]
            tiles['E3'] = E3 = [getst("E3a"), getst("E3b")]
            for p in range(2):
                conv_pass(G3[p], (21, Tc + 29),
                          [(lhsT(6), G3d[p], 0, 128, -4),
                           (lhsT(7, 64), G3d[p], 0, 64, 0)],
                          evac_for(f'G3_{p}', 0.2, 1, half), pool_for(f'G3_{p}'), half, psg_for(f'G3_{p}'))
                conv_pass(E3[p], (13, Tc + 21),
                          [(lhsT(8), EG3d[p], 0, 128, 0)],
                          evac_for('E3', 0.2, 1, half), pool_for('E3'), half, psg_for('E3'))

        def l4(half):  # H
            G3, E3 = tiles['G3'], tiles['E3']
            tiles['H'] = H = [getst("Ha"), getst("Hb")]
            for p in range(2):
                conv_pass(H[p], (13, Tc + 21),
                          [(lhsT(9), E3[p], 0, 128, 0),
                           (lhsT(10), G3[p], 0, 128, 8)],
                          evac_for('H', 0.2, 2, half), pool_for('H'), half, psg_for('H'))

        def l5(half):  # H1 (fc1) - interleave pair halves
            H = tiles['H']
            tiles['H1'] = H1 = [getst("H1" + str(c)) for c in range(4)]
            for cidx in (0, 2, 1, 3):
                p, ph = cidx // 2, cidx % 2
                conv_pass(H1[cidx], (13, Tc + 21),
                          [(lhsT(11, 64, base=64 * ph), H[p],
                            64 * ph, 64 * ph + 64, 0)],
                          evac_for(f'H1_{cidx}', 0.02, 3, half), pool_for(f'H1_{cidx}'), half, psg_for(f'H1_{cidx}'))

        def l6(half):  # A2 (fc2)
            H1 = tiles['H1']
            tiles['A2'] = A2 = [getst("A2a"), getst("A2b")]
            for p in range(2):
                conv_pass(A2[p], (13, Tc + 21),
                          [(lhsT(12), H1[2 * p], 0, 128, 0),
                           (lhsT(13), H1[2 * p + 1], 0, 128, 0)],
                          evac_for('A2', 0.02, 4, half), pool_for('A2'), half, psg_for('A2'))

        def l7(half):  # FFC (fc3 + tanh)
            A2 = tiles['A2']
            tiles['FFC'] = FFC = getst("FFC")

            def tanh_evac(ps, ot):
                nc.scalar.activation(ot, ps, AF.Tanh, bias=bias(5), scale=1.0)

            conv_pass(FFC, (13, Tc + 21),
                      [(lhsT(14), A2[0], 0, 128, 0),
                       (lhsT(15), A2[1], 0, 128, 0)],
                      tanh_evac, ppool_a, half, PSA_GROUP)
            # phantom edge values must read as zero in the window sum
            if half == 0:
                nc.gpsimd.memset(FFC[0:32, 13:16], 0.0)
            else:
                nc.gpsimd.memset(FFC[96:128, Tc + 16: Tc + 21], 0.0)

        def l8(half):  # window-sum tree (8-wide): piece-split for pipelining
            # half 0 owns cols [.., CB), half 1 [CB, ..); ops at the boundary
            # read a few columns across it (producer half finished earlier).
            FFC = tiles['FFC']
            S1 = getst("S1")
            tiles['S1'] = S1
            SSTEP = 512
            # Pool is slow (no 16-bit speedup); the final job's sums sit on
            # the critical tail, so run those on DVE instead.
            eng = nc.vector
            # staggered split points: each op's half-0 range ends before the
            # columns that would read the NEXT op stage across the boundary
            for (p0, p1) in _split(13 if half == 0 else CB + 3,
                                   CB + 3 if half == 0 else Tc + 19, SSTEP):
                eng.tensor_tensor(S1[:, p0: p1], FFC[:, p0: p1],
                                  FFC[:, p0 + 1: p1 + 1], OP.add)
            for (p0, p1) in _split(13 if half == 0 else CB + 1,
                                   CB + 1 if half == 0 else Tc + 17, SSTEP):
                eng.tensor_tensor(FFC[:, p0: p1], S1[:, p0: p1],
                                  S1[:, p0 + 2: p1 + 2], OP.add)
            for (p0, p1) in _split(16 if half == 0 else CB,
                                   CB if half == 0 else Tc + 16, SSTEP):
                eng.tensor_tensor(S1[:, p0: p1], FFC[:, p0 - 3: p1 - 3],
                                  FFC[:, p0 + 1: p1 + 1], OP.add)

        def l9(half):  # final fc + clip + output DMA
            S1 = tiles['S1']
            for (glo, ghi) in _split(16 if half == 0 else CB,
                                     CB if half == 0 else Tc + 16, PS_GROUP):
                ps = ppool_d.tile([128, PS_GROUP], F32, tag=f"ps{PS_GROUP}", name="ps",
                                  bufs=PS_BUFS)
                for (lo, hi) in _split(glo, ghi, 512):
                    n, off = hi - lo, lo - glo
                    nc.tensor.matmul(ps[0:64, off:off + n], lhsT(16, 128, 64),
                                     S1[:, lo: hi], start=True, stop=True)
                    nc.vector.tensor_scalar(
                        Y[:, b * Tc + lo - 16: b * Tc + hi - 16],
                        ps[0:64, off:off + n], 0.0, 1.0, OP.max, OP.min)
            (h0, h1) = (0, CB - 16) if half == 0 else (CB - 16, Tc)
            nc.sync.dma_start(out=o_d[b, :, :, h0:h1],
                              in_=Y[0:64, b * Tc + h0: b * Tc + h1])

        return [l0, l1, l2, l3, l4, l5, l6, l7, l8, l9]

    batch_layers = [make_layers(b) for b in range(BPC)]
    jobs = [(0, 0), (0, 1), (1, 0), (1, 1)]
    L = len(batch_layers[0])
    for k in range(L + SKEW * (len(jobs) - 1)):
        # emit deeper-pipelined jobs first: a half-0 layer reads a few
        # boundary columns from the next job's previous layer, which must
        # appear earlier in program order for the dependency to register
        for j, (b, h) in reversed(list(enumerate(jobs))):
            kk = k - SKEW * j
            if 0 <= kk < L:
                batch_layers[b][kk](h)


def _get_program(reps=1):
    global _PROG
    if _PROG is None:
        _PROG = {}
    if reps not in _PROG:
        _PROG[reps] = _build_program(reps)
    return _PROG[reps]


def _prepare_inputs(inputs):
    x = np.asarray(inputs['speech_features'], np.float32)
    xp = np.zeros((B, C + 1, T + 22), np.float32)
    xp[:, :C, 10:10 + T] = x
    xp[:, C, :] = 1.0
    # chunk-major: [B, 4, 30, Tc+22]; chunk c covers padded cols [c*Tc, c*Tc+Tc+22)
    xa = np.zeros((B, 4, C + 1, Tc + 22), np.float32)
    for c in range(4):
        xa[:, c] = xp[:, :, c * Tc: c * Tc + Tc + 22]
    xa = xa.astype(BF)
    wp, wb = _pack_weights({k: np.asarray(v, np.float32)
                            for k, v in inputs.items() if k != 'speech_features'})
    return [{"x": xa[i * BPC:(i + 1) * BPC], "wpack": wp, "wbias": wb}
            for i in range(NCORES)]


def kernel(**inputs):
    from concourse.bass_utils import run_bass_kernel_spmd

    in_maps = _prepare_inputs(inputs)
    nc = _get_program()
    res = run_bass_kernel_spmd(nc, in_maps, core_ids=list(range(NCORES)))
    outs = []
    for r in res.results:
        o = np.asarray(r["out"]).astype(np.float32)      # [BPC, 4, 16, Tc]
        o = o.transpose(0, 1, 3, 2).reshape(BPC, T, 16)  # [BPC, T, 16]
        outs.append(o)
    return np.ascontiguousarray(np.concatenate(outs, axis=0))
